# revision 1
# baseline (speedup 1.0000x reference)
"""CFG dual self-attention kernel for 8 Trainium2 NeuronCores.

Strategy (tensor parallel on heads):
  - h = concat(hidden_cond, hidden_uncond) -> [4096 tokens, 5120]; host
    pre-transposes to hT [5120, 4096] so the QKV matmul contraction dim (5120)
    lands on SBUF partitions.
  - Each core owns 5 heads (640 of the 5120 q/k/v channels).  It computes
    qT/kT [640, 4096] (transposed layout: head-dim on partitions) and
    v [4096, 640] (natural layout) from hT with fp32r matmuls.
  - RMSNorm over the full 5120 dims needs a cross-core sum of squares:
    partial ssq per token is computed with ones-matmuls on the PE and
    allreduced across the 8 cores (32 KB collective, hidden under the V
    projection).
  - Attention per (batch, head) in scores-transposed layout
    scoresT[st, sq] = (rope(k) slice)^T @ rope(q): softmax denominators via
    ones-matmul column sums (interleaved PSUM accumulation groups), exp on
    the scalar engine, A@V accumulated with v-chunks stationary, and the
    1/colsum normalization folded into the PSUM->SBUF eviction of attn_outT.
  - Output projection: partial_out = attn_outT^T @ Wout[rows of this core]
    -> [4096, 5120] per-core partial; host sums the 8 partials (+ bout).
"""

import numpy as np
import ml_dtypes

import concourse.bass as bass  # noqa: F401  (bass types via bacc)
import concourse.mybir as mybir
import concourse.tile as tile
from concourse import bacc
from concourse.bass_utils import run_bass_kernel_spmd

F32 = mybir.dt.float32
F32R = mybir.dt.float32r
F8 = mybir.dt.float8e4
E4NP = ml_dtypes.float8_e4m3
DRM = mybir.MatmulPerfMode.DoubleRow

NCORES = 8
EPS = 1e-6

# fp8 hi/lo quantization scales (host-side split; 3-term DoubleRow matmuls)
SH = 16.0
SW = 1024.0
SA = 32.0
SWO = 1024.0
QKV_DESC = 1.0 / (SH * SW)
OUT_DESC = 1.0 / (SA * SWO)


def _vblocks(cw):
    # split the v output width into matmul N-blocks (>=256 keeps fp32r fast)
    if cw % 320 == 0:
        return [320] * (cw // 320)
    return [cw]


def build_program(S, DIM, H, collective=True, repeat=1):
    """Emit the per-core bass program (identical on all cores; per-core data
    differences come entirely from the input tensors)."""
    HD = 128
    assert DIM == H * HD
    HPC = H // NCORES          # heads per core
    CW = HPC * HD              # per-core channel width for q/k/v
    CT = HPC                   # 128-col tiles per group
    NT = 2 * S                 # tokens across both batches
    DC = DIM // 128            # contraction chunks
    TBS = 256                  # token block in phase 1
    NTB = NT // TBS
    SQB = min(512, S)          # sq block in attention
    NSQ = S // SQB
    NST = S // 128             # st (key) chunks per batch
    VNB = _vblocks(CW)
    ONB = DIM // 512           # out-proj N blocks
    # contraction sub-blocks for merged DMA (dep granularity)
    CSUB = 10 if DC % 10 == 0 else DC
    NCS = DC // CSUB

    nc = bacc.Bacc("TRN2", target_bir_lowering=False, debug=False,
                   num_devices=NCORES)

    hT8 = nc.dram_tensor("hT8", [DIM * 2, NT], F8, kind="ExternalInput")
    wq8 = nc.dram_tensor("wq8", [DIM * 2, CW], F8, kind="ExternalInput")
    wk8 = nc.dram_tensor("wk8", [DIM * 2, CW], F8, kind="ExternalInput")
    wv8 = nc.dram_tensor("wv8", [DIM * 2, CW], F8, kind="ExternalInput")
    bq = nc.dram_tensor("bq", [128, CT], F32, kind="ExternalInput")
    bk = nc.dram_tensor("bk", [128, CT], F32, kind="ExternalInput")
    wqn = nc.dram_tensor("wqn", [128, CT], F32, kind="ExternalInput")
    wkn = nc.dram_tensor("wkn", [128, CT], F32, kind="ExternalInput")
    cosT = nc.dram_tensor("cosT", [128, S], F32, kind="ExternalInput")
    sinrT = nc.dram_tensor("sinrT", [128, S], F32, kind="ExternalInput")
    wo8 = nc.dram_tensor("wo8", [6 * 2 * 128, DIM], F8, kind="ExternalInput")
    outp = nc.dram_tensor("outp", [NT, DIM], F32, kind="ExternalOutput")

    h4 = hT8.rearrange("(c j p) t -> p c j t", p=128, j=2)

    with tile.TileContext(nc) as tc:
        with (
            tc.tile_pool(name="dram", bufs=1, space="DRAM") as dram,
            tc.tile_pool(name="persist", bufs=1) as persist,
        ):
            for _rep in range(repeat):
              qsc = dram.tile([CW, NT], F32, tag="qsc")
              ksc = dram.tile([CW, NT], F32, tag="ksc")
              vsc = dram.tile([NT, CW], F32, tag="vsc")
              aosc8 = dram.tile([2 * CW, NT], F8, tag="aosc8")
              cc_in = dram.tile([2, NT], F32, tag="cc_in")
              cc_out = dram.tile([2, NT], F32, tag="cc_out")

              # constants
              ones_f = persist.tile([128, 1], F32, tag="ones_f")
              nc.vector.memset(ones_f[:], 1.0)
              ones = persist.tile([128, 1], F32R, tag="ones")
              nc.vector.tensor_copy(ones[:], ones_f[:])

              bq_t = persist.tile([128, CT], F32, tag="bq")
              nc.sync.dma_start(bq_t[:], bq[:])
              bk_t = persist.tile([128, CT], F32, tag="bk")
              nc.sync.dma_start(bk_t[:], bk[:])
              wqn_t = persist.tile([128, CT], F32, tag="wqn")
              nc.sync.dma_start(wqn_t[:], wqn[:])
              wkn_t = persist.tile([128, CT], F32, tag="wkn")
              nc.sync.dma_start(wkn_t[:], wkn[:])

              # pre-reserved pool for the b=0 norm-factor tiles: opened
              # before the phase-1 pools and closed after phase 3, so their
              # DMAs/broadcasts never WAR-depend on recycled phase-1
              # addresses and can run during the v group
              from contextlib import ExitStack as _ES
              _rb_ctx = _ES()
              rb0p = _rb_ctx.enter_context(tc.tile_pool(name="rb0p", bufs=1))

              # ---------------- phase 1: qkv projections + ssq partials -------
              with (
                  tc.tile_pool(name="wp", bufs=1) as wp,
                  tc.tile_pool(name="hp", bufs=2) as hp,
                  tc.tile_pool(name="ev", bufs=1) as evp,
              ):
                  def load_hall(tb):
                      hall = hp.tile([128, DC, 2, TBS], F8, tag="hall")
                      for cs in range(NCS):
                          nc.sync.dma_start(
                              hall[:, cs * CSUB:(cs + 1) * CSUB, :, :],
                              h4[:, cs * CSUB:(cs + 1) * CSUB, :,
                                 tb * TBS:(tb + 1) * TBS])
                      return hall

                  def load_wall(wdram, interleave_tb0=None, order="ct"):
                      # sub-loads ordered to match the consumer loop: ct-major
                      # for q/k (col-tile outer), chunk-major for v (chunk
                      # outer) — the first matmul group only waits for the
                      # first slice, not the whole W block
                      wall = wp.tile([128, DC, 2, CW], F8, tag="wall")
                      w3 = wdram.rearrange("(c j p) n -> p c j n", p=128,
                                           j=2)
                      hall0 = None
                      if interleave_tb0 is not None:
                          hall0 = hp.tile([128, DC, 2, TBS], F8, tag="hall")
                      if order == "ct":
                          wslices = [(slice(None), slice(ct * 128,
                                                         (ct + 1) * 128))
                                     for ct in range(CT)]
                      else:
                          wslices = [(slice(cs * CSUB, (cs + 1) * CSUB),
                                      slice(None)) for cs in range(NCS)]
                      for si, (csl, nsl) in enumerate(wslices):
                          nc.sync.dma_start(wall[:, csl, :, nsl],
                                            w3[:, csl, :, nsl])
                          if hall0 is not None and si < NCS:
                              nc.sync.dma_start(
                                  hall0[:, si * CSUB:(si + 1) * CSUB, :, :],
                                  h4[:, si * CSUB:(si + 1) * CSUB, :,
                                     0:TBS])
                      if hall0 is not None:
                          for cs in range(len(wslices), NCS):
                              nc.sync.dma_start(
                                  hall0[:, cs * CSUB:(cs + 1) * CSUB, :, :],
                                  h4[:, cs * CSUB:(cs + 1) * CSUB, :,
                                     0:TBS])
                      return wall, hall0

                  for gi, (wdram, bias_t, spill) in enumerate(
                      [(wq8, bq_t, qsc), (wk8, bk_t, ksc)]
                  ):
                      with (
                          tc.tile_pool(name=f"ps{gi}", bufs=4, space="PSUM") as psp,
                          tc.tile_pool(name=f"sq{gi}", bufs=2, space="PSUM") as sqp,
                          tc.tile_pool(name=f"aux{gi}", bufs=1) as st1,
                      ):
                          wall, hall0 = load_wall(wdram, interleave_tb0=True)
                          deferred = []
                          for tb in range(NTB):
                              hall = hall0 if tb == 0 else load_hall(tb)
                              hall0 = None
                              ssq_ps = sqp.tile([1, TBS], F32, tag="ssq")
                              evq = evp.tile([128, CT, TBS], F32R, tag="evq")
                              sqt = st1.tile([128, CT, TBS], F32R, tag="sqt")
                              for ct in range(CT):
                                  pq = psp.tile([128, TBS], F32, tag="acc")
                                  csl = slice(ct * 128, (ct + 1) * 128)
                                  for cp in range(DC // 2):
                                      nc.tensor.matmul(
                                          pq[:],
                                          wall[:, 2 * cp:2 * cp + 2, 0, csl],
                                          hall[:, 2 * cp:2 * cp + 2, 1, :],
                                          start=(cp == 0), stop=False,
                                          perf_mode=DRM)
                                  for ch in range(DC):
                                      nc.tensor.matmul(
                                          pq[:],
                                          wall[:, ch, :, csl],
                                          hall[:, ch, :, :],
                                          start=False, stop=(ch == DC - 1),
                                          perf_mode=DRM)
                                  # the ssq matmul of the PREVIOUS ct group goes
                                  # here so the PE never waits on the DVE square
                                  if deferred:
                                      deferred.pop(0)()
                                  nc.vector.tensor_scalar(
                                      evq[:, ct, :].bitcast(F32), pq[:],
                                      bias_t[:, ct:ct + 1], QKV_DESC,
                                      mybir.AluOpType.add,
                                      mybir.AluOpType.mult)
                                  nc.sync.dma_start(
                                      spill.rearrange("(c p) t -> p c t", p=128)
                                      .bitcast(F32R)
                                      [:, ct, tb * TBS:(tb + 1) * TBS],
                                      evq[:, ct, :])
                                  nc.vector.tensor_mul(
                                      sqt[:, ct, :], evq[:, ct, :].bitcast(F32),
                                      evq[:, ct, :].bitcast(F32))

                                  def emit_ssq(ssq_ps=ssq_ps, sqt=sqt, ct=ct,
                                               evq=evq, tb=tb, spill=spill,
                                               gi=gi):
                                      nc.tensor.matmul(
                                          ssq_ps[:], ones[:], sqt[:, ct, :],
                                          start=(ct == 0), stop=(ct == CT - 1))
                                      if ct == CT - 1:
                                          stg = st1.tile([1, TBS], F32,
                                                         tag="stg", name="stg")
                                          nc.vector.tensor_copy(stg[:],
                                                                ssq_ps[:])
                                          nc.gpsimd.dma_start(
                                              cc_in[gi:gi + 1,
                                                    tb * TBS:(tb + 1) * TBS],
                                              stg[:])
                                  deferred.append(emit_ssq)
                          while deferred:
                              deferred.pop(0)()

                  # allreduce the ssq partials (overlaps with the v group below)
                  if collective:
                      nc.gpsimd.collective_compute(
                          "AllReduce", mybir.AluOpType.add,
                          replica_groups=[list(range(NCORES))],
                          ins=[cc_in[:].opt()], outs=[cc_out[:].opt()])
                  else:
                      # single-core timing-sim variant: stand-in for the
                      # allreduce so TimelineSim (no collectives) can run
                      nc.sync.dma_start(cc_out[:], cc_in[:])

                  # v projection (natural layout, hT token-tiles stationary)
                  with tc.tile_pool(name="psv", bufs=2, space="PSUM") as psv:
                      wall, hall0 = load_wall(wv8, interleave_tb0=True,
                                              order="cs")
                      for tb in range(NTB):
                          hall = hall0 if tb == 0 else load_hall(tb)
                          hall0 = None
                          nsub = TBS // 128
                          pv = [[psv.tile([128, nb], F32, tag=f"pv{ts}_{i}",
                                          name=f"pv{ts}_{i}")
                                 for i, nb in enumerate(VNB)]
                                for ts in range(nsub)]
                          for ts in range(nsub):
                              tsl = slice(ts * 128, (ts + 1) * 128)
                              off = 0
                              for i, nb in enumerate(VNB):
                                  nsl = slice(off, off + nb)
                                  for cp in range(DC // 2):
                                      nc.tensor.matmul(
                                          pv[ts][i][:],
                                          hall[:, 2 * cp:2 * cp + 2, 1, tsl],
                                          wall[:, 2 * cp:2 * cp + 2, 0, nsl],
                                          start=(cp == 0), stop=False,
                                          perf_mode=DRM)
                                  for ch in range(DC):
                                      nc.tensor.matmul(
                                          pv[ts][i][:],
                                          hall[:, ch, :, tsl],
                                          wall[:, ch, :, nsl],
                                          start=False, stop=(ch == DC - 1),
                                          perf_mode=DRM)
                                  off += nb
                          evv = evp.tile([128, nsub, CW], F32R, tag="evq",
                                         name="evv")
                          for ts in range(nsub):
                              off = 0
                              for i, nb in enumerate(VNB):
                                  # v bias is folded into the host-side output
                                  # bias (softmax rows sum to 1), so this is a
                                  # pure PSUM->SBUF copy on the idle scalar
                                  # engine, keeping DVE free for the rinv/rope
                                  # chains that overlap the v group
                                  nc.scalar.activation(
                                      evv[:, ts, off:off + nb].bitcast(F32),
                                      pv[ts][i][:],
                                      mybir.ActivationFunctionType.Copy,
                                      scale=QKV_DESC)
                                  off += nb
                              nc.sync.dma_start(
                                  vsc.rearrange("(b p) n -> p b n", p=128)
                                  .bitcast(F32R)[:, tb * nsub + ts, :],
                                  evv[:, ts, :])

              # ---------------- phase 3: attention per (batch, head) ----------
              # prefix of Wout loaded into its own (non-overlapping) pool while
              # attention still runs, so phase 4 starts without a DMA stall
              WOSL = 1024 if DIM >= 2048 else 512
              w3o = wo8.rearrange("(s j p) n -> p s j n", p=128, j=2)
              with tc.tile_pool(name="wo0p", bufs=1) as wo0p:
                with (
                  tc.tile_pool(name="p3", bufs=2) as p3,
                  tc.tile_pool(name="p3e", bufs=3) as p3e,
                  tc.tile_pool(name="cs3", bufs=1) as cs3,
                  tc.tile_pool(name="ps_sc", bufs=3, space="PSUM") as ps_sc,
                  tc.tile_pool(name="ps_cs", bufs=2, space="PSUM") as ps_cs,
                  tc.tile_pool(name="ps_av", bufs=2, space="PSUM") as ps_av,
                ):
                  cosT_t = cs3.tile([128, S], F32, tag="cosT")
                  nc.sync.dma_start(cosT_t[:], cosT[:])
                  sinrT_t = cs3.tile([128, S], F32, tag="sinrT")
                  nc.sync.dma_start(sinrT_t[:], sinrT[:])
                  wot0 = wo0p.tile([128, 6, 2, WOSL], F8, tag="wot0")

                  # rinv = 1/sqrt(mean ssq + eps): broadcast the allreduced
                  # ssq rows first, then do the math 128-lane-parallel.
                  # Only the b=0 factors are on the critical path; b=1 is
                  # deferred into the first pair's attention.
                  rb = {}

                  def emit_rb(b):
                      for gi in range(2):
                          # fold the HD**-0.5 attention scale into the q side:
                          # s/sqrt(ssq/DIM+eps) == 1/sqrt(ssq*HD/DIM+HD*eps)
                          sc1 = (HD / DIM) if gi == 0 else (1.0 / DIM)
                          sc2 = (HD * EPS) if gi == 0 else EPS
                          row = p3.tile([1, S], F32, tag="rrow", name="rrow")
                          nc.sync.dma_start(
                              row[:], cc_out[gi:gi + 1, b * S:(b + 1) * S])
                          t = cs3.tile([128, S], F32, tag=f"rb{gi}{b}",
                                       name=f"rb{gi}{b}")
                          nc.gpsimd.partition_broadcast(t[:], row[:])
                          nc.vector.tensor_scalar(t[:], t[:], sc1, sc2,
                                                  mybir.AluOpType.mult,
                                                  mybir.AluOpType.add)
                          nc.scalar.activation(
                              t[:], t[:], mybir.ActivationFunctionType.Sqrt)
                          nc.vector.reciprocal(t[:], t[:])
                          rb[(gi, b)] = t

                  emit_rb(0)

                  bhs = [(b, hh) for b in range(2) for hh in range(HPC)]

                  def prep(i, nchunk=1):
                      """Load + norm + rope q/k and load v for pair i.  Emitted
                      one pair ahead so the DVE work overlaps the previous
                      pair's attention matmuls.  nchunk>1 slices the chain
                      column-wise so the first attention matmul only waits
                      for the first slice (used for the first pair, whose
                      chain cannot overlap anything earlier)."""
                      b, hh = bhs[i]
                      CS2 = S // nchunk
                      qkr = []
                      for gi, (spill, wn) in enumerate(
                              [(qsc, wqn_t), (ksc, wkn_t)]):
                          xt = p3.tile([128, S], F32, tag="xt", name="xt")
                          tmc = p3.tile([128, S], F32, tag="tmc", name="tmc")
                          tms = p3.tile([128, S], F32, tag="tms", name="tms")
                          xr = p3.tile([128, S], F32R, tag="xr", name="xr")
                          for cc in range(nchunk):
                              sl = slice(cc * CS2, (cc + 1) * CS2)
                              nc.sync.dma_start(
                                  xt[:, sl],
                                  spill[hh * 128:(hh + 1) * 128,
                                        b * S + cc * CS2:
                                        b * S + (cc + 1) * CS2])
                              nc.vector.tensor_mul(xt[:, sl], xt[:, sl],
                                                   rb[(gi, b)][:, sl])
                              nc.vector.tensor_scalar_mul(
                                  xt[:, sl], xt[:, sl], wn[:, hh:hh + 1])
                              nc.vector.tensor_mul(tmc[:, sl], xt[:, sl],
                                                   cosT_t[:, sl])
                              nc.vector.tensor_mul(
                                  tms[0:64, sl], xt[64:128, sl],
                                  sinrT_t[64:128, sl])
                              nc.vector.tensor_mul(
                                  tms[64:128, sl], xt[0:64, sl],
                                  sinrT_t[0:64, sl])
                              nc.vector.tensor_add(xr[:, sl], tmc[:, sl],
                                                   tms[:, sl])
                          qkr.append(xr)
                      vt = p3.tile([128, NST, 128], F32R, tag="vt", name="vt")
                      nc.gpsimd.dma_start(
                          vt[:], vsc[b * S:(b + 1) * S,
                                     hh * 128:(hh + 1) * 128]
                          .bitcast(F32R)
                          .rearrange("(c p) d -> p c d", p=128))
                      return qkr[0], qkr[1], vt

                  preps = {0: prep(0, nchunk=4)}
                  for i in range(len(bhs)):
                      b, hh = bhs[i]
                      if i + 1 < len(bhs) and i > 0:
                          preps[i + 1] = prep(i + 1)
                      qr, kr, vt = preps.pop(i)
                      if True:
                          aoh8 = p3.tile([128, S], F8, tag="aoh8",
                                         name="aoh8")
                          aol8 = p3.tile([128, S], F8, tag="aol8",
                                         name="aol8")
                          for sqb in range(NSQ):
                              cs = ps_cs.tile([1, SQB], F32, tag="cs")
                              av = ps_av.tile([128, SQB], F32, tag="av")
                              # one-step lookahead: scores MM for st+1 issues
                              # before the cs/av MMs of st, hiding exp latency
                              ets = {}
                              for st in range(NST + 1):
                                  if st < NST:
                                      sc = ps_sc.tile([128, SQB], F32, tag="sc")
                                      nc.tensor.matmul(
                                          sc[:], kr[:, st * 128:(st + 1) * 128],
                                          qr[:, sqb * SQB:(sqb + 1) * SQB],
                                          start=True, stop=True)
                                      et = p3e.tile([128, SQB], F32R, tag="et")
                                      nc.scalar.activation(
                                          et[:], sc[:],
                                          mybir.ActivationFunctionType.Exp)
                                      ets[st] = et
                                  if st >= 1:
                                      pst = st - 1
                                      et = ets.pop(pst)
                                      nc.tensor.matmul(
                                          cs[:], ones[:], et[:],
                                          start=(pst == 0),
                                          stop=(pst == NST - 1))
                                      nc.tensor.matmul(
                                          av[:], vt[:, pst, :], et[:],
                                          start=(pst == 0),
                                          stop=(pst == NST - 1))
                              rc = p3.tile([1, SQB], F32, tag="rc")
                              nc.vector.reciprocal(rc[:], cs[:])
                              nc.vector.tensor_scalar_mul(rc[:], rc[:], SA)
                              rb2 = p3.tile([128, SQB], F32, tag="rb2")
                              nc.gpsimd.partition_broadcast(rb2[:], rc[:])
                              sqsl = slice(sqb * SQB, (sqb + 1) * SQB)
                              ao32 = p3.tile([128, SQB], F32, tag="ao32")
                              nc.vector.tensor_mul(ao32[:], av[:], rb2[:])
                              nc.vector.tensor_copy(aoh8[:, sqsl], ao32[:])
                              nc.vector.tensor_tensor(
                                  aol8[:, sqsl], ao32[:], aoh8[:, sqsl],
                                  mybir.AluOpType.subtract)
                              if i == 0 and sqb == 0:
                                  # off the critical path: the b=1 norm
                                  # factors and the second pair's prep
                                  emit_rb(1)
                                  preps[1] = prep(1)
                                  # prefetch of the Wout prefix for phase 4
                                  for nb in range(WOSL // 512):
                                      nc.sync.dma_start(
                                          wot0[:, :, :,
                                               nb * 512:(nb + 1) * 512],
                                          w3o[:, :, :,
                                              nb * 512:(nb + 1) * 512])
                          nc.gpsimd.dma_start(
                              aosc8[hh * 256 + 128:hh * 256 + 256,
                                    b * S:(b + 1) * S], aoh8[:])
                          nc.gpsimd.dma_start(
                              aosc8[hh * 256:hh * 256 + 128,
                                    b * S:(b + 1) * S], aol8[:])

                # -------------- phase 4: partial output projection ----------
                NWO0 = WOSL // 512
                with (
                  tc.tile_pool(name="wo", bufs=1) as wo,
                  tc.tile_pool(name="p4", bufs=4) as p4,
                  tc.tile_pool(name="oe", bufs=2) as oep,
                  tc.tile_pool(name="ps4", bufs=4, space="PSUM") as ps4,
                ):
                  ao4 = aosc8.rearrange("(c j p) t -> p c j t", p=128,
                                        j=2)

                  def load_aot(tt):
                      aot = p4.tile([128, HPC, 2, 128], F8, tag="aot",
                                    name="aot")
                      nc.sync.dma_start(
                          aot[:], ao4[:, :, :, tt * 128:(tt + 1) * 128])
                      return aot

                  aot_next = load_aot(0)   # first attn-out tile before W bulk
                  wot = None
                  if DIM > WOSL:
                      wot = wo.tile([128, 6, 2, DIM - WOSL], F8, tag="wot")
                      for nb in range(NWO0, ONB):
                          nc.sync.dma_start(
                              wot[:, :, :, (nb - NWO0) * 512:
                                  (nb - NWO0 + 1) * 512],
                              w3o[:, :, :, nb * 512:(nb + 1) * 512])
                  for tt in range(NT // 128):
                      aot = aot_next
                      if tt + 1 < NT // 128:
                          aot_next = load_aot(tt + 1)
                      orow = oep.tile([128, DIM], F32, tag="orow")
                      for nb in range(ONB):
                          if nb < NWO0:
                              wsl = wot0[:, :, :, nb * 512:(nb + 1) * 512]
                          else:
                              wsl = wot[:, :, :, (nb - NWO0) * 512:
                                        (nb - NWO0 + 1) * 512]
                          po = ps4.tile([128, 512], F32, tag="po")
                          # 8 DoubleRow passes: 2 hi*hi chunk-pairs, 5
                          # crosses, 1 fused hi*hi+lo*lo for the odd chunk
                          nc.tensor.matmul(
                              po[:], aot[:, 0:2, 1, :], wsl[:, 0:2, 0, :],
                              start=True, stop=False, perf_mode=DRM)
                          nc.tensor.matmul(
                              po[:], aot[:, 2:4, 1, :], wsl[:, 2:4, 0, :],
                              start=False, stop=False, perf_mode=DRM)
                          for ch in range(HPC):
                              nc.tensor.matmul(
                                  po[:], aot[:, ch, :, :], wsl[:, ch, :, :],
                                  start=False, stop=False, perf_mode=DRM)
                          nc.tensor.matmul(
                              po[:], aot[:, 4, :, :], wsl[:, 5, :, :],
                              start=False, stop=True, perf_mode=DRM)
                          nc.vector.tensor_scalar_mul(
                              orow[:, nb * 512:(nb + 1) * 512], po[:],
                              OUT_DESC)
                      nc.sync.dma_start(
                          outp[tt * 128:(tt + 1) * 128, :], orow[:])
              _rb_ctx.close()
    nc.finalize()
    return nc


_PROGRAM_CACHE = {}


def _get_program(S, DIM, H):
    key = (S, DIM, H)
    if key not in _PROGRAM_CACHE:
        _PROGRAM_CACHE[key] = build_program(S, DIM, H)
    return _PROGRAM_CACHE[key]


def _split8(x, scale):
    xs = (np.asarray(x, np.float32) * np.float32(scale))
    hi = xs.astype(E4NP)
    lo = (xs - hi.astype(np.float32)).astype(E4NP)
    return hi, lo


def _pack_w(Wslice, scale):
    # [DIM, n] -> [(c j p), n] with j=(hi, lo)
    wh, wl = _split8(Wslice, scale)
    d, n = Wslice.shape
    w8 = np.empty((d // 128, 2, 128, n), E4NP)
    w8[:, 0] = wh.reshape(d // 128, 128, n)
    w8[:, 1] = wl.reshape(d // 128, 128, n)
    return np.ascontiguousarray(w8.reshape(d * 2, n))


def make_in_maps(S, DIM, H, hidden_cond, hidden_uncond, cos_freqs, sin_freqs,
                 Wqkv, bqkv, wq_norm, wk_norm, Wout, bout):
    HD = 128
    HPC = H // NCORES
    CW = HPC * HD
    NT = 2 * S
    h = np.concatenate([np.asarray(hidden_cond), np.asarray(hidden_uncond)],
                       axis=0).reshape(NT, DIM)
    hT = np.ascontiguousarray(h.T)
    hh, hl = _split8(hT, SH)
    hT8 = np.empty((DIM // 128, 2, 128, NT), E4NP)
    hT8[:, 0] = hl.reshape(DIM // 128, 128, NT)   # j=0: lo
    hT8[:, 1] = hh.reshape(DIM // 128, 128, NT)   # j=1: hi
    hT8 = np.ascontiguousarray(hT8.reshape(DIM * 2, NT))
    cosT = np.ascontiguousarray(np.asarray(cos_freqs).T.astype(np.float32))
    sinT = np.asarray(sin_freqs).T  # [128, S]
    HF = HD // 2
    sinrT = np.concatenate([sinT[HF:], -sinT[:HF]], axis=0)
    sinrT = np.ascontiguousarray(sinrT.astype(np.float32))
    Wqkv = np.asarray(Wqkv)
    bqkv = np.asarray(bqkv)
    wq_norm = np.asarray(wq_norm)
    wk_norm = np.asarray(wk_norm)
    Wout = np.asarray(Wout)

    in_maps = []
    for c in range(NCORES):
        sl = slice(c * CW, (c + 1) * CW)
        bq_c = (bqkv[0 * DIM:1 * DIM][sl] * (SH * SW)).astype(np.float32) \
            .reshape(HPC, HD).T
        bk_c = (bqkv[1 * DIM:2 * DIM][sl] * (SH * SW)).astype(np.float32) \
            .reshape(HPC, HD).T
        # out-proj rows (slot, j, p): slots 0..4 = chunk (hi,lo), slot 5 =
        # chunk 4 as (lo,hi) so its hi*hi+lo*lo fuses into one DoubleRow
        woh, wol = _split8(Wout[sl, :], SWO)
        wo8 = np.empty((6, 2, 128, DIM), E4NP)
        for s in range(HPC):
            wo8[s, 0] = woh[s * 128:(s + 1) * 128]
            wo8[s, 1] = wol[s * 128:(s + 1) * 128]
        wo8[5, 0] = wol[4 * 128:5 * 128]
        wo8[5, 1] = woh[4 * 128:5 * 128]
        in_maps.append({
            "hT8": hT8,
            "wq8": _pack_w(Wqkv[:, 0 * DIM:1 * DIM][:, sl], SW),
            "wk8": _pack_w(Wqkv[:, 1 * DIM:2 * DIM][:, sl], SW),
            "wv8": _pack_w(Wqkv[:, 2 * DIM:3 * DIM][:, sl], SW),
            "bq": np.ascontiguousarray(bq_c),
            "bk": np.ascontiguousarray(bk_c),
            "wqn": np.ascontiguousarray(wq_norm[sl].reshape(HPC, HD).T
                                        .astype(np.float32)),
            "wkn": np.ascontiguousarray(wk_norm[sl].reshape(HPC, HD).T
                                        .astype(np.float32)),
            "cosT": cosT,
            "sinrT": sinrT,
            "wo8": np.ascontiguousarray(wo8.reshape(6 * 2 * 128, DIM)),
        })
    return in_maps


def run(S, DIM, H, inputs):
    nc = _get_program(S, DIM, H)
    in_maps = make_in_maps(S, DIM, H, **inputs)
    res = run_bass_kernel_spmd(nc, in_maps, list(range(NCORES)))
    partial = np.zeros((2 * S, DIM), np.float64)
    for r in res.results:
        partial += r["outp"].astype(np.float64)
    # the v-bias contribution: softmax rows sum to 1, so attn(v + 1*bv) =
    # attn(v) + 1*bv, and bv flows through Wout as a constant per-channel term
    bv_full = np.asarray(inputs["bqkv"])[2 * DIM:3 * DIM].astype(np.float64)
    const_bias = bv_full @ np.asarray(inputs["Wout"]).astype(np.float64) \
        + np.asarray(inputs["bout"])
    out = (partial + const_bias[None, :]).astype(np.float32)
    out = out.reshape(2, 1, S, DIM)
    return out[0], out[1]


def kernel(hidden_cond, hidden_uncond, cos_freqs, sin_freqs,
           Wqkv, bqkv, wq_norm, wk_norm, Wout, bout):
    B, S, DIM = np.asarray(hidden_cond).shape
    assert B == 1
    H = DIM // 128
    return run(S, DIM, H, dict(
        hidden_cond=hidden_cond, hidden_uncond=hidden_uncond,
        cos_freqs=cos_freqs, sin_freqs=sin_freqs, Wqkv=Wqkv, bqkv=bqkv,
        wq_norm=wq_norm, wk_norm=wk_norm, Wout=Wout, bout=bout))



# revision 21
# speedup vs baseline: 1.1605x; 1.1605x over previous
"""CFG dual self-attention kernel for 8 Trainium2 NeuronCores.

Strategy (tensor parallel on heads):
  - h = concat(hidden_cond, hidden_uncond) -> [4096 tokens, 5120]; host
    pre-transposes AND pre-tiles to hP [128, tb, chunk, hi/lo, tok] so every
    DMA line is a single 20 KB contiguous run per partition (full 360 GB/s;
    the naive transposed layout ran at half speed on 256 B descriptor lines).
  - Each core owns 5 heads (640 of the 5120 q/k/v channels).  One fused pass
    over hP computes qT/kT [640, 4096] (head-dim on partitions, spilled to
    DRAM as bf16) with fp8 hi/lo DoubleRow matmuls; a second pass computes
    v [4096, 640], which stays resident in SBUF (bf16) until attention.
  - RMSNorm over the full 5120 dims needs a cross-core sum of squares:
    partial ssq per token is computed with ones-matmuls on the PE and
    allreduced across the 8 cores (32 KB collective, hidden under the V
    projection).  The 1/sqrt factors and the first attention pair's
    rope chain are also emitted under the V projection (DVE is idle there).
  - Attention per (batch, head) in scores-transposed layout
    scoresT[st, sq] = (rope(k) slice)^T @ rope(q), all in bf16: softmax
    denominators via ones-matmul column sums (interleaved PSUM accumulation
    groups), exp on the scalar engine, A@V accumulated with resident
    v-chunks stationary, and the 1/colsum normalization folded into the
    PSUM->SBUF eviction of attn_outT (quantized fp8 hi/lo for phase 4).
  - Output projection: partial_out = attn_outT^T @ Wout[rows of this core]
    -> [4096, 5120] bf16 per-core partial; host sums the 8 partials (+ bout).
"""

import numpy as np
import ml_dtypes

import concourse.bass as bass  # noqa: F401  (bass types via bacc)
import concourse.mybir as mybir
import concourse.tile as tile
from concourse import bacc
from concourse.bass_utils import run_bass_kernel_spmd

F32 = mybir.dt.float32
F32R = mybir.dt.float32r
BF16 = mybir.dt.bfloat16
F8 = mybir.dt.float8e4
E4NP = ml_dtypes.float8_e4m3
BFNP = ml_dtypes.bfloat16
DRM = mybir.MatmulPerfMode.DoubleRow

NCORES = 8
EPS = 1e-6

# fp8 hi/lo quantization scales (host-side split; 3-term DoubleRow matmuls)
SH = 16.0
SW = 1024.0
SA = 32.0
SWO = 1024.0
QKV_DESC = 1.0 / (SH * SW)
OUT_DESC = 1.0 / (SA * SWO)


def build_program(S, DIM, H, collective=True, repeat=1):
    """Emit the per-core bass program (identical on all cores; per-core data
    differences come entirely from the input tensors)."""
    HD = 128
    assert DIM == H * HD
    HPC = H // NCORES          # heads per core
    CW = HPC * HD              # per-core channel width for q/k/v
    CT = HPC                   # 128-col tiles per group
    NT = 2 * S                 # tokens across both batches
    DC = DIM // 128            # contraction chunks
    TBS = 256                  # token block in phase 1
    NTB = NT // TBS
    SQB = min(512, S)          # sq block in attention
    NSQ = S // SQB
    NST = S // 128             # st (key) chunks per batch
    NTC = NT // 128            # token chunks for resident v
    ONB = DIM // 512           # out-proj N blocks
    WOSL = 2048 if DIM >= 4096 else DIM   # prefetched Wout column prefix

    nc = bacc.Bacc("TRN2", target_bir_lowering=False, debug=False,
                   num_devices=NCORES)

    # host-pretiled inputs: per partition p everything is contiguous, so each
    # DMA is a handful of >=512 B descriptors (full DMA bandwidth)
    hP = nc.dram_tensor("hP", [128, NTB * DC * 2 * TBS], F8,
                        kind="ExternalInput")
    wq8 = nc.dram_tensor("wq8", [128, DC * 2 * CW], F8, kind="ExternalInput")
    wk8 = nc.dram_tensor("wk8", [128, DC * 2 * CW], F8, kind="ExternalInput")
    wv8 = nc.dram_tensor("wv8", [128, DC * 2 * CW], F8, kind="ExternalInput")
    bq = nc.dram_tensor("bq", [128, CT], F32, kind="ExternalInput")
    bk = nc.dram_tensor("bk", [128, CT], F32, kind="ExternalInput")
    wqn = nc.dram_tensor("wqn", [128, CT], F32, kind="ExternalInput")
    wkn = nc.dram_tensor("wkn", [128, CT], F32, kind="ExternalInput")
    cosT = nc.dram_tensor("cosT", [128, S], BF16, kind="ExternalInput")
    sinrT = nc.dram_tensor("sinrT", [128, S], BF16, kind="ExternalInput")
    wo8 = nc.dram_tensor("wo8", [128, 6 * 2 * DIM], F8, kind="ExternalInput")
    outp = nc.dram_tensor("outp", [NT, DIM], BF16, kind="ExternalOutput")

    h5 = hP.rearrange("p (b c j t) -> p b c j t", b=NTB, c=DC, j=2, t=TBS)
    wq4 = wq8.rearrange("p (c j n) -> p c j n", c=DC, j=2, n=CW)
    wk4 = wk8.rearrange("p (c j n) -> p c j n", c=DC, j=2, n=CW)
    wv4 = wv8.rearrange("p (c j n) -> p c j n", c=DC, j=2, n=CW)
    w3o = wo8.rearrange("p (s j n) -> p s j n", s=6, j=2, n=DIM)

    with tile.TileContext(nc) as tc:
        with (
            tc.tile_pool(name="dram", bufs=1, space="DRAM") as dram,
            tc.tile_pool(name="persist", bufs=1) as persist,
        ):
            for _rep in range(repeat):
              qsc = dram.tile([CW, NT], BF16, tag="qsc")
              ksc = dram.tile([CW, NT], BF16, tag="ksc")
              aosc8 = dram.tile([2 * CW, NT], F8, tag="aosc8")
              # ssq partials in partition-major layout [128 p, gi, tb, ts]:
              # produced by transposed ones-matmuls (out free size 1 => ~free
              # on the PE), allreduced as a flat 32 KB buffer
              NSS = 2 * NTB * (TBS // 128)
              cc_in = dram.tile([128, NSS], F32, tag="cc_in")
              cc_out = dram.tile([128, NSS], F32, tag="cc_out")
              rdump = dram.tile([2, NTB * (TBS // 128), 128], BF16,
                                tag="rdump")

              # constants
              ones_f = persist.tile([128, 1], F32, tag="ones_f")
              nc.vector.memset(ones_f[:], 1.0)
              ones_b = persist.tile([128, 1], BF16, tag="ones_b")
              nc.vector.tensor_copy(ones_b[:], ones_f[:])

              bq_t = persist.tile([128, CT], F32, tag="bq")
              nc.scalar.dma_start(bq_t[:], bq[:])
              bk_t = persist.tile([128, CT], F32, tag="bk")
              nc.scalar.dma_start(bk_t[:], bk[:])
              wqn_t = persist.tile([128, CT], F32, tag="wqn")
              nc.scalar.dma_start(wqn_t[:], wqn[:])
              wkn_t = persist.tile([128, CT], F32, tag="wkn")
              nc.scalar.dma_start(wkn_t[:], wkn[:])

              from contextlib import ExitStack as _ES
              _late = _ES()
              # resident v lives from the v projection through attention;
              # pre-reserved before the phase-1 pools (LIFO stack order)
              vresp = _late.enter_context(tc.tile_pool(name="vres", bufs=1))
              vall = vresp.tile([128, NTC, CW], BF16, tag="vall")

              # ---------------- phase 1: projections ----------------------
              with tc.tile_pool(name="hp", bufs=2) as hp:
                def load_hall(tb, pieces=1):
                    hall = hp.tile([128, DC, 2, TBS], F8, tag="hall")
                    cs = DC // pieces
                    for i in range(pieces):
                        nc.sync.dma_start(
                            hall[:, i * cs:(i + 1) * cs, :, :],
                            h5[:, tb, i * cs:(i + 1) * cs, :, :])
                    return hall

                # ---- phase 1a: fused q+k projections + ssq partials ------
                with (
                    tc.tile_pool(name="wqk", bufs=1) as wqk,
                    tc.tile_pool(name="ev", bufs=2) as evp,
                    tc.tile_pool(name="ps", bufs=4, space="PSUM") as psp,
                    tc.tile_pool(name="sq", bufs=2, space="PSUM") as sqp,
                ):
                    # interleave wq / hall0 quarter-loads so the first token
                    # block (which consumes chunks in order) starts as soon
                    # as the first quarter lands
                    wq_t = wqk.tile([128, DC, 2, CW], F8, tag="wq")
                    hall0 = hp.tile([128, DC, 2, TBS], F8, tag="hall")
                    for i in range(8):
                        nc.sync.dma_start(wq_t[:, i * 5:(i + 1) * 5],
                                          wq4[:, i * 5:(i + 1) * 5])
                        nc.sync.dma_start(
                            hall0[:, i * 5:(i + 1) * 5, :, :],
                            h5[:, 0, i * 5:(i + 1) * 5, :, :])
                    hall1 = load_hall(1, pieces=2)
                    wk_t = wqk.tile([128, DC, 2, CW], F8, tag="wk")
                    for i in range(4):
                        nc.sync.dma_start(wk_t[:, i * 10:(i + 1) * 10],
                                          wk4[:, i * 10:(i + 1) * 10])

                    deferred = []
                    ssq_acc = evp.tile([128, 2, NTB, TBS // 128], F32,
                                       tag="ssq_acc")

                    def qk_group(tb, gi, hall):
                        wall, bias_t, spill = (
                            (wq_t, bq_t, qsc), (wk_t, bk_t, ksc))[gi]
                        nts = TBS // 128
                        zz = sqp.tile([128, nts], F32, tag="ssq")
                        evq = evp.tile([128, CT, TBS], BF16, tag="evq")
                        sqt = evp.tile([128, CT, TBS], BF16, tag="sqt")
                        for ct in range(CT):
                            pq = psp.tile([128, TBS], F32, tag="acc")
                            csl = slice(ct * 128, (ct + 1) * 128)
                            # chunk-ordered 3-term interleave: compute
                            # streams behind the chunk-sliced DMAs
                            for cp in range(DC // 2):
                                nc.tensor.matmul(
                                    pq[:],
                                    wall[:, 2 * cp:2 * cp + 2, 0, csl],
                                    hall[:, 2 * cp:2 * cp + 2, 1, :],
                                    start=(cp == 0), stop=False,
                                    perf_mode=DRM)
                                nc.tensor.matmul(
                                    pq[:], wall[:, 2 * cp, :, csl],
                                    hall[:, 2 * cp, :, :],
                                    start=False, stop=False, perf_mode=DRM)
                                nc.tensor.matmul(
                                    pq[:], wall[:, 2 * cp + 1, :, csl],
                                    hall[:, 2 * cp + 1, :, :],
                                    start=False, stop=(cp == DC // 2 - 1),
                                    perf_mode=DRM)
                            # the ssq matmul of the PREVIOUS ct group goes
                            # here so the PE never waits on the DVE square
                            if deferred:
                                deferred.pop(0)()
                            nc.vector.tensor_scalar(
                                evq[:, ct, :], pq[:],
                                bias_t[:, ct:ct + 1], QKV_DESC,
                                mybir.AluOpType.add,
                                mybir.AluOpType.mult)
                            nc.vector.tensor_mul(
                                sqt[:, ct, :], evq[:, ct, :], evq[:, ct, :])

                            def emit_ssq(zz=zz, sqt=sqt, ct=ct,
                                         evq=evq, tb=tb, spill=spill, gi=gi,
                                         nts=nts):
                                # transposed ones-matmul: out free size is 1,
                                # so the partition-dim token sums are nearly
                                # free on the PE (vs 1 cycle/token in the
                                # row-layout version)
                                # one zero-region per PSUM bank: only the
                                # FIRST chain in the shared bank may set
                                # start (it lazily zeroes the whole bank),
                                # only the LAST may set stop
                                for ts in range(nts):
                                    nc.tensor.matmul(
                                        zz[:, ts:ts + 1],
                                        sqt[:, ct, ts * 128:(ts + 1) * 128],
                                        ones_b[:],
                                        start=(ct == 0 and ts == 0),
                                        stop=(ct == CT - 1
                                              and ts == nts - 1))
                                if ct == CT - 1:
                                    # batched bf16 spill of the whole token
                                    # block (one DMA per (tb, gi))
                                    nc.sync.dma_start(
                                        spill.rearrange(
                                            "(c p) t -> p c t", p=128)
                                        [:, :, tb * TBS:(tb + 1) * TBS],
                                        evq[:])
                                    nc.vector.tensor_copy(
                                        ssq_acc[:, gi, tb, :], zz[:])
                            deferred.append(emit_ssq)

                    # startup order q0,q1,k0,k1 hides the wk load behind the
                    # first two q groups (the serial DMA stream needs ~44us
                    # for wq+wk+h0+h1, two q groups give it ~32us of PE work)
                    qk_group(0, 0, hall0)
                    qk_group(1, 0, hall1)
                    qk_group(0, 1, hall0)
                    qk_group(1, 1, hall1)
                    for tb in range(2, NTB):
                        hall = load_hall(tb)
                        qk_group(tb, 0, hall)
                        qk_group(tb, 1, hall)
                    while deferred:
                        deferred.pop(0)()
                    nc.gpsimd.dma_start(
                        cc_in.rearrange("p (g c s) -> p g c s", g=2, c=NTB),
                        ssq_acc[:])

                # allreduce the ssq partials (overlaps with the v group)
                if collective:
                    nc.gpsimd.collective_compute(
                        "AllReduce", mybir.AluOpType.add,
                        replica_groups=[list(range(NCORES))],
                        ins=[cc_in[:].opt()], outs=[cc_out[:].opt()])
                else:
                    # single-core timing-sim variant: stand-in for the
                    # allreduce so TimelineSim (no collectives) can run
                    nc.sync.dma_start(cc_out[:], cc_in[:])

                # rms norm factors: tiny partition-major math + the b=0
                # broadcast rows, all overlapped with the v projection
                # (fold the HD**-0.5 attention scale into the q side:
                #  s/sqrt(ssq/DIM+eps) == 1/sqrt(ssq*HD/DIM + HD*eps))
                NTC2 = NTB * (TBS // 128)
                rwork = persist.tile([128, 2, NTC2], F32, tag="rwork")
                rinv = persist.tile([128, 2, NTC2], BF16, tag="rinv")
                nc.scalar.dma_start(
                    rwork[:], cc_out.rearrange("p (g c) -> p g c", g=2))
                for gi in range(2):
                    sc1 = (HD / DIM) if gi == 0 else (1.0 / DIM)
                    sc2 = (HD * EPS) if gi == 0 else EPS
                    nc.vector.tensor_scalar(
                        rwork[:, gi, :], rwork[:, gi, :], sc1, sc2,
                        mybir.AluOpType.mult, mybir.AluOpType.add)
                nc.scalar.activation(
                    rwork[:], rwork[:], mybir.ActivationFunctionType.Sqrt)
                with nc.allow_low_precision(reason="bf16 rms factor"):
                    nc.vector.reciprocal(rinv[:], rwork[:])
                nc.scalar.dma_start(rdump.rearrange("g c p -> p g c"),
                                    rinv[:])
                rd2 = rdump.rearrange("g c p -> g (c p)")
                rb = {}

                def emit_rb(b, pool):
                    for gi in range(2):
                        row = pool.tile([1, S], BF16, tag="rrow",
                                        name="rrow")
                        dma = nc.scalar.dma_start if b == 0 \
                            else nc.sync.dma_start
                        dma(row[:], rd2[gi:gi + 1, b * S:(b + 1) * S])
                        t = pool.tile([128, S], BF16, tag=f"rb{gi}{b}",
                                      name=f"rb{gi}{b}")
                        nc.gpsimd.partition_broadcast(t[:], row[:])
                        rb[(gi, b)] = t

                emit_rb(0, persist)

                # ---- phase 1b: v projection (natural layout) -------------
                # wv loads in chunk slices at v start; the PE streams
                # chunk-ordered behind them (four PSUM tiles in lockstep)
                with (
                    tc.tile_pool(name="wv", bufs=1) as wvp,
                    tc.tile_pool(name="psv", bufs=2, space="PSUM") as psv,
                ):
                    wv_t = wvp.tile([128, DC, 2, CW], F8, tag="wv")
                    for i in range(8):
                        nc.sync.dma_start(wv_t[:, i * 5:(i + 1) * 5],
                                          wv4[:, i * 5:(i + 1) * 5])
                    nsub = TBS // 128
                    nb = CW // 2
                    for tb in range(NTB):
                        hall = load_hall(tb)
                        pv = [[psv.tile([128, nb], F32, tag=f"pv{ts}{i}",
                                        name=f"pv{ts}{i}")
                               for i in range(2)] for ts in range(nsub)]
                        for cp in range(DC // 2):
                            for ts in range(nsub):
                                tsl = slice(ts * 128, (ts + 1) * 128)
                                for i in range(2):
                                    nsl = slice(i * nb, (i + 1) * nb)
                                    nc.tensor.matmul(
                                        pv[ts][i][:],
                                        hall[:, 2 * cp:2 * cp + 2, 1, tsl],
                                        wv_t[:, 2 * cp:2 * cp + 2, 0, nsl],
                                        start=(cp == 0), stop=False,
                                        perf_mode=DRM)
                                    nc.tensor.matmul(
                                        pv[ts][i][:],
                                        hall[:, 2 * cp, :, tsl],
                                        wv_t[:, 2 * cp, :, nsl],
                                        start=False, stop=False,
                                        perf_mode=DRM)
                                    nc.tensor.matmul(
                                        pv[ts][i][:],
                                        hall[:, 2 * cp + 1, :, tsl],
                                        wv_t[:, 2 * cp + 1, :, nsl],
                                        start=False,
                                        stop=(cp == DC // 2 - 1),
                                        perf_mode=DRM)
                        for ts in range(nsub):
                            for i in range(2):
                                # v bias is folded into the host-side output
                                # bias (softmax rows sum to 1); pure
                                # PSUM->SBUF convert-copy on the idle scalar
                                # engine straight into the resident v tile
                                nc.scalar.activation(
                                    vall[:, tb * nsub + ts,
                                         i * nb:(i + 1) * nb],
                                    pv[ts][i][:],
                                    mybir.ActivationFunctionType.Copy,
                                    scale=QKV_DESC)

              # ---------------- phase 3: attention per (batch, head) ------
              _late2 = _ES()
              wprep = _late2.enter_context(tc.tile_pool(name="wpre", bufs=1))
              # the full Wout block + the attn-out ping-pong tiles live in a
              # pool that predates the attention pools, so their DMAs have no
              # WAR on attention tiles and stream during attention
              wot = wprep.tile([128, 6, 2, DIM], F8, tag="wot")
              bhs = [(b, hh) for b in range(2) for hh in range(HPC)]
              with (
                  tc.tile_pool(name="attc", bufs=1) as attc,
                  tc.tile_pool(name="p3", bufs=2) as p3,
                  tc.tile_pool(name="p3e", bufs=4) as p3e,
                  tc.tile_pool(name="ps_sc", bufs=2, space="PSUM") as ps_sc,
                  tc.tile_pool(name="ps_cs", bufs=2, space="PSUM") as ps_cs,
                  tc.tile_pool(name="ps_av", bufs=2, space="PSUM") as ps_av,
              ):
                  cosT_t = attc.tile([128, S], BF16, tag="cosT")
                  nc.gpsimd.dma_start(cosT_t[:], cosT[:])
                  sinrT_t = attc.tile([128, S], BF16, tag="sinrT")
                  nc.gpsimd.dma_start(sinrT_t[:], sinrT[:])

                  def prep(i, nchunk=1):
                      """Load + norm + rope q/k for pair i (bf16 end-to-end;
                      v is already resident).  Emitted one pair ahead so the
                      DVE work overlaps the previous pair's attention.
                      nchunk>1 slices the chain column-wise so the first
                      scores matmul only waits for the first slice (used for
                      pair 0, which has nothing earlier to hide behind)."""
                      b, hh = bhs[i]
                      CS2 = S // nchunk
                      tls = {}
                      for gi, (spill, wn) in enumerate(
                              [(qsc, wqn_t), (ksc, wkn_t)]):
                          xt = p3.tile([128, S], BF16, tag="xt", name="xt")
                          tmc = p3.tile([128, S], BF16, tag="tmc",
                                        name="tmc")
                          tms = p3.tile([128, S], BF16, tag="tms",
                                        name="tms")
                          xr = p3.tile([128, S], BF16, tag=f"xr{gi}",
                                       name="xr")
                          tls[gi] = (xt, tmc, tms, xr, spill, wn)
                      # k chain first within each column slice: the first
                      # scores matmul needs kr slice 0 (stationary) + qr
                      # slice 0, so it can issue after two chain slices
                      for cc in range(nchunk):
                          sl = slice(cc * CS2, (cc + 1) * CS2)
                          for gi in (1, 0):
                              xt, tmc, tms, xr, spill, wn = tls[gi]
                              nc.sync.dma_start(
                                  xt[:, sl],
                                  spill[hh * 128:(hh + 1) * 128,
                                        b * S + cc * CS2:
                                        b * S + (cc + 1) * CS2])
                              nc.vector.tensor_mul(xt[:, sl], xt[:, sl],
                                                   rb[(gi, b)][:, sl])
                              nc.vector.tensor_scalar_mul(
                                  xt[:, sl], xt[:, sl], wn[:, hh:hh + 1])
                              nc.vector.tensor_mul(tmc[:, sl], xt[:, sl],
                                                   cosT_t[:, sl])
                              nc.vector.tensor_mul(
                                  tms[0:64, sl], xt[64:128, sl],
                                  sinrT_t[64:128, sl])
                              nc.vector.tensor_mul(
                                  tms[64:128, sl], xt[0:64, sl],
                                  sinrT_t[0:64, sl])
                              nc.vector.tensor_add(xr[:, sl], tmc[:, sl],
                                                   tms[:, sl])
                      return tls[0][3], tls[1][3]

                  preps = {0: prep(0, nchunk=4)}
                  for i in range(len(bhs)):
                      b, hh = bhs[i]
                      qr, kr = preps.pop(i)
                      aoh8 = p3.tile([128, S], F8, tag="aoh8", name="aoh8")
                      aol8 = p3.tile([128, S], F8, tag="aol8", name="aol8")
                      NSTP = NST // 2
                      NSL = SQB // 128
                      # flattened (sqb, st-pair) stream with one-pair
                      # lookahead ACROSS sqb boundaries: the scalar engine
                      # (exp) is the attention bottleneck, so the scores
                      # matmul feeding exp u+1 always issues before the
                      # av/cs consumers of exp u
                      NU = NSQ * NSTP
                      ets = {}
                      avs = {}
                      zzs = {}

                      def norm_evict(sqb):
                          # denominators are partition-major [128, NSL];
                          # reshape to a row via a DRAM bounce (SBUF APs
                          # cannot transpose partitions), then 1/z
                          av = avs.pop(sqb)
                          zz = zzs.pop(sqb)
                          z4s = p3.tile([128, NSL], F32, tag="z4s")
                          nc.vector.tensor_copy(z4s[:], zz[:])
                          zd = dram.tile([NSL, 128], F32, tag="zd")
                          nc.gpsimd.dma_start(
                              zd.rearrange("s p -> p s"), z4s[:])
                          rc = p3.tile([1, SQB], F32, tag="rc")
                          nc.gpsimd.dma_start(
                              rc[:], zd.rearrange("s p -> () (s p)"))
                          nc.vector.reciprocal(rc[:], rc[:])
                          nc.vector.tensor_scalar_mul(rc[:], rc[:], SA)
                          rb2 = p3.tile([128, SQB], F32, tag="rb2")
                          nc.gpsimd.partition_broadcast(rb2[:], rc[:])
                          sqsl = slice(sqb * SQB, (sqb + 1) * SQB)
                          ao32 = p3.tile([128, SQB], F32, tag="ao32")
                          nc.vector.tensor_mul(ao32[:], av[:], rb2[:])
                          nc.vector.tensor_copy(aoh8[:, sqsl], ao32[:])
                          nc.vector.tensor_tensor(
                              aol8[:, sqsl], ao32[:], aoh8[:, sqsl],
                              mybir.AluOpType.subtract)

                      for u in range(NU + 1):
                          if u < NU:
                              sqb, stp = divmod(u, NSTP)
                              if stp == 0:
                                  zzs[sqb] = ps_cs.tile([128, NSL], F32,
                                                        tag="zz", name="zz")
                                  avs[sqb] = ps_av.tile([128, SQB], F32,
                                                        tag="av", name="av")
                              sc2 = ps_sc.tile([128, 2, SQB], F32,
                                               tag="sc")
                              for hs in range(2):
                                  st = 2 * stp + hs
                                  nc.tensor.matmul(
                                      sc2[:, hs, :],
                                      kr[:, st * 128:(st + 1) * 128],
                                      qr[:, sqb * SQB:(sqb + 1) * SQB],
                                      start=True, stop=True)
                              et2 = p3e.tile([128, 2, SQB], BF16,
                                             tag="et")
                              nc.scalar.activation(
                                  et2[:], sc2[:],
                                  mybir.ActivationFunctionType.Exp)
                              ets[u] = et2
                          if u >= 1:
                              psqb, pstp = divmod(u - 1, NSTP)
                              et2 = ets.pop(u - 1)
                              for hs in range(2):
                                  st = 2 * pstp + hs
                                  nc.tensor.matmul(
                                      avs[psqb][:],
                                      vall[:, b * NST + st,
                                           hh * 128:(hh + 1) * 128],
                                      et2[:, hs, :],
                                      start=(st == 0),
                                      stop=(st == NST - 1))
                                  # shared-bank chains: single start
                                  # (bank zero) / single stop, see phase 1
                                  for sl in range(NSL):
                                      nc.tensor.matmul(
                                          zzs[psqb][:, sl:sl + 1],
                                          et2[:, hs,
                                              sl * 128:(sl + 1) * 128],
                                          ones_b[:],
                                          start=(st == 0 and sl == 0),
                                          stop=(st == NST - 1
                                                and sl == NSL - 1))
                              if pstp == NSTP - 1:
                                  norm_evict(psqb)
                                  if i == 0 and psqb == 0:
                                      # off the critical path: the b=1 norm
                                      # factors, the second pair's prep, and
                                      # the full Wout stream for phase 4
                                      emit_rb(1, attc)
                                      preps[1] = prep(1)
                                      for nbw in range(ONB):
                                          nc.sync.dma_start(
                                              wot[:, :, :,
                                                  nbw * 512:
                                                  (nbw + 1) * 512],
                                              w3o[:, :, :,
                                                  nbw * 512:
                                                  (nbw + 1) * 512])
                                  if i > 0 and psqb == 1 \
                                          and i + 1 < len(bhs):
                                      # next pair's rope chain emitted
                                      # mid-pair so the in-order DVE queue
                                      # does not delay this pair's
                                      # normalizations
                                      preps[i + 1] = prep(i + 1)
                      nc.gpsimd.dma_start(
                          aosc8[hh * 256 + 128:hh * 256 + 256,
                                b * S:(b + 1) * S], aoh8[:])
                      nc.gpsimd.dma_start(
                          aosc8[hh * 256:hh * 256 + 128,
                                b * S:(b + 1) * S], aol8[:])

              # -------------- phase 4: partial output projection ----------
              with (
                  tc.tile_pool(name="p4", bufs=4) as p4,
                  tc.tile_pool(name="oe", bufs=2) as oep,
                  tc.tile_pool(name="ps4", bufs=4, space="PSUM") as ps4,
              ):
                  ao4 = aosc8.rearrange("(c j p) t -> p c j t", p=128, j=2)

                  def load_aot(tt):
                      aot = wprep.tile([128, HPC, 2, 128], F8,
                                       tag=f"aot{tt % 2}", name="aot")
                      nc.sync.dma_start(
                          aot[:], ao4[:, :, :, tt * 128:(tt + 1) * 128])
                      return aot

                  aot_next = load_aot(0)
                  for tt in range(NT // 128):
                      aot = aot_next
                      if tt + 1 < NT // 128:
                          aot_next = load_aot(tt + 1)
                      orow = oep.tile([128, DIM], BF16, tag="orow")
                      for nb in range(ONB):
                          wsl = wot[:, :, :, nb * 512:(nb + 1) * 512]
                          po = ps4.tile([128, 512], F32, tag="po")
                          # 8 DoubleRow passes: 2 hi*hi chunk-pairs, 5
                          # crosses, 1 fused hi*hi+lo*lo for the odd chunk
                          nc.tensor.matmul(
                              po[:], aot[:, 0:2, 1, :], wsl[:, 0:2, 0, :],
                              start=True, stop=False, perf_mode=DRM)
                          nc.tensor.matmul(
                              po[:], aot[:, 2:4, 1, :], wsl[:, 2:4, 0, :],
                              start=False, stop=False, perf_mode=DRM)
                          for ch in range(HPC):
                              nc.tensor.matmul(
                                  po[:], aot[:, ch, :, :], wsl[:, ch, :, :],
                                  start=False, stop=False, perf_mode=DRM)
                          nc.tensor.matmul(
                              po[:], aot[:, 4, :, :], wsl[:, 5, :, :],
                              start=False, stop=True, perf_mode=DRM)
                          nc.vector.tensor_scalar_mul(
                              orow[:, nb * 512:(nb + 1) * 512], po[:],
                              OUT_DESC)
                      if tt == NT // 128 - 1:
                          # split the last row-block's writeback so the DMA
                          # overlaps the tail evictions
                          nc.sync.dma_start(
                              outp[tt * 128:(tt + 1) * 128, 0:DIM // 2],
                              orow[:, 0:DIM // 2])
                          nc.sync.dma_start(
                              outp[tt * 128:(tt + 1) * 128, DIM // 2:],
                              orow[:, DIM // 2:])
                      else:
                          nc.sync.dma_start(
                              outp[tt * 128:(tt + 1) * 128, :], orow[:])
              _late2.close()
              _late.close()
    nc.finalize()
    return nc


_PROGRAM_CACHE = {}


def _get_program(S, DIM, H):
    key = (S, DIM, H)
    if key not in _PROGRAM_CACHE:
        _PROGRAM_CACHE[key] = build_program(S, DIM, H)
    return _PROGRAM_CACHE[key]


def _split8(x, scale):
    xs = (np.asarray(x, np.float32) * np.float32(scale))
    hi = xs.astype(E4NP)
    lo = (xs - hi.astype(np.float32)).astype(E4NP)
    return hi, lo


def _pack_h(h, DIM, NT, TBS):
    # h [NT, DIM] f32 -> [128, NTB*DC*2*TBS] fp8, j: 0=lo, 1=hi
    DC = DIM // 128
    NTB = NT // TBS
    hh, hl = _split8(h.T, SH)                        # [DIM, NT]
    arr = np.stack([hl, hh])                         # [2(j), DIM, NT]
    arr = arr.reshape(2, DC, 128, NTB, TBS)
    arr = arr.transpose(2, 3, 1, 0, 4)               # [128, NTB, DC, 2, TBS]
    return np.ascontiguousarray(arr.reshape(128, -1))


def _pack_w(Wslice, scale):
    # [DIM, n] -> [128, DC*2*n] with j: 0=hi, 1=lo
    d, n = Wslice.shape
    wh, wl = _split8(Wslice, scale)
    arr = np.stack([wh, wl])                         # [2(j), DIM, n]
    arr = arr.reshape(2, d // 128, 128, n)
    arr = arr.transpose(2, 1, 0, 3)                  # [128, DC, 2, n]
    return np.ascontiguousarray(arr.reshape(128, -1))


def make_in_maps(S, DIM, H, hidden_cond, hidden_uncond, cos_freqs, sin_freqs,
                 Wqkv, bqkv, wq_norm, wk_norm, Wout, bout):
    HD = 128
    HPC = H // NCORES
    CW = HPC * HD
    NT = 2 * S
    TBS = 256
    h = np.concatenate([np.asarray(hidden_cond), np.asarray(hidden_uncond)],
                       axis=0).reshape(NT, DIM)
    hP = _pack_h(h, DIM, NT, TBS)
    cosTb = np.ascontiguousarray(
        np.asarray(cos_freqs).T.astype(BFNP))        # [128, S]
    sinT = np.asarray(sin_freqs).T                   # [128, S]
    HF = HD // 2
    sinrT = np.concatenate([sinT[HF:], -sinT[:HF]], axis=0)
    sinrTb = np.ascontiguousarray(sinrT.astype(BFNP))
    Wqkv = np.asarray(Wqkv)
    bqkv = np.asarray(bqkv)
    wq_norm = np.asarray(wq_norm)
    wk_norm = np.asarray(wk_norm)
    Wout = np.asarray(Wout)

    in_maps = []
    for c in range(NCORES):
        sl = slice(c * CW, (c + 1) * CW)
        bq_c = (bqkv[0 * DIM:1 * DIM][sl] * (SH * SW)).astype(np.float32) \
            .reshape(HPC, HD).T
        bk_c = (bqkv[1 * DIM:2 * DIM][sl] * (SH * SW)).astype(np.float32) \
            .reshape(HPC, HD).T
        # out-proj rows [128, slot, j, n]: slots 0..4 = chunk (hi,lo),
        # slot 5 = chunk 4 as (lo,hi) so hi*hi+lo*lo fuses into one DoubleRow
        woh, wol = _split8(Wout[sl, :], SWO)
        wo8 = np.empty((128, 6, 2, DIM), E4NP)
        for s in range(HPC):
            wo8[:, s, 0] = woh[s * 128:(s + 1) * 128]
            wo8[:, s, 1] = wol[s * 128:(s + 1) * 128]
        wo8[:, 5, 0] = wol[4 * 128:5 * 128]
        wo8[:, 5, 1] = woh[4 * 128:5 * 128]
        in_maps.append({
            "hP": hP,
            "wq8": _pack_w(Wqkv[:, 0 * DIM:1 * DIM][:, sl], SW),
            "wk8": _pack_w(Wqkv[:, 1 * DIM:2 * DIM][:, sl], SW),
            "wv8": _pack_w(Wqkv[:, 2 * DIM:3 * DIM][:, sl], SW),
            "bq": np.ascontiguousarray(bq_c),
            "bk": np.ascontiguousarray(bk_c),
            "wqn": np.ascontiguousarray(wq_norm[sl].reshape(HPC, HD).T
                                        .astype(np.float32)),
            "wkn": np.ascontiguousarray(wk_norm[sl].reshape(HPC, HD).T
                                        .astype(np.float32)),
            "cosT": cosTb,
            "sinrT": sinrTb,
            "wo8": np.ascontiguousarray(wo8.reshape(128, -1)),
        })
    return in_maps


def run(S, DIM, H, inputs):
    nc = _get_program(S, DIM, H)
    in_maps = make_in_maps(S, DIM, H, **inputs)
    res = run_bass_kernel_spmd(nc, in_maps, list(range(NCORES)))
    partial = np.zeros((2 * S, DIM), np.float64)
    for r in res.results:
        partial += np.asarray(r["outp"]).astype(np.float64)
    # the v-bias contribution: softmax rows sum to 1, so attn(v + 1*bv) =
    # attn(v) + 1*bv, and bv flows through Wout as a constant per-channel term
    bv_full = np.asarray(inputs["bqkv"])[2 * DIM:3 * DIM].astype(np.float64)
    const_bias = bv_full @ np.asarray(inputs["Wout"]).astype(np.float64) \
        + np.asarray(inputs["bout"])
    out = (partial + const_bias[None, :]).astype(np.float32)
    out = out.reshape(2, 1, S, DIM)
    return out[0], out[1]


def kernel(hidden_cond, hidden_uncond, cos_freqs, sin_freqs,
           Wqkv, bqkv, wq_norm, wk_norm, Wout, bout):
    B, S, DIM = np.asarray(hidden_cond).shape
    assert B == 1
    H = DIM // 128
    return run(S, DIM, H, dict(
        hidden_cond=hidden_cond, hidden_uncond=hidden_uncond,
        cos_freqs=cos_freqs, sin_freqs=sin_freqs, Wqkv=Wqkv, bqkv=bqkv,
        wq_norm=wq_norm, wk_norm=wk_norm, Wout=Wout, bout=bout))


# revision 28
# speedup vs baseline: 1.1881x; 1.0238x over previous
"""CFG dual self-attention kernel for 8 Trainium2 NeuronCores.

Strategy (tensor parallel on heads):
  - h = concat(hidden_cond, hidden_uncond) -> [4096 tokens, 5120]; host
    pre-transposes AND pre-tiles to hP [128, tb, chunk, hi/lo, tok] so every
    DMA line is a single 20 KB contiguous run per partition (full 360 GB/s;
    the naive transposed layout ran at half speed on 256 B descriptor lines).
  - Each core owns 5 heads (640 of the 5120 q/k/v channels).  One fused pass
    over hP computes qT/kT [640, 4096] (head-dim on partitions, spilled to
    DRAM as bf16) with fp8 hi/lo DoubleRow matmuls; a second pass computes
    v [4096, 640], which stays resident in SBUF (bf16) until attention.
  - RMSNorm over the full 5120 dims needs a cross-core sum of squares:
    partial ssq per token is computed with ones-matmuls on the PE and
    allreduced across the 8 cores (32 KB collective, hidden under the V
    projection).  The 1/sqrt factors and the first attention pair's
    rope chain are also emitted under the V projection (DVE is idle there).
  - Attention per (batch, head) in scores-transposed layout
    scoresT[st, sq] = (rope(k) slice)^T @ rope(q), all in bf16: softmax
    denominators via ones-matmul column sums (interleaved PSUM accumulation
    groups), exp on the scalar engine, A@V accumulated with resident
    v-chunks stationary, and the 1/colsum normalization folded into the
    PSUM->SBUF eviction of attn_outT (quantized fp8 hi/lo for phase 4).
  - Output projection: partial_out = attn_outT^T @ Wout[rows of this core]
    -> [4096, 5120] bf16 per-core partial; host sums the 8 partials (+ bout).
"""

import numpy as np
import ml_dtypes

import concourse.bass as bass  # noqa: F401  (bass types via bacc)
import concourse.mybir as mybir
import concourse.tile as tile
from concourse import bacc
from concourse.bass_utils import run_bass_kernel_spmd

F32 = mybir.dt.float32
F32R = mybir.dt.float32r
BF16 = mybir.dt.bfloat16
F8 = mybir.dt.float8e4
E4NP = ml_dtypes.float8_e4m3
BFNP = ml_dtypes.bfloat16
DRM = mybir.MatmulPerfMode.DoubleRow

NCORES = 8
EPS = 1e-6

# fp8 hi/lo quantization scales (host-side split; 3-term DoubleRow matmuls)
SH = 16.0
SW = 1024.0
SA = 32.0
SWO = 1024.0
QKV_DESC = 1.0 / (SH * SW)
OUT_DESC = 1.0 / (SA * SWO)


def build_program(S, DIM, H, collective=True, repeat=1):
    """Emit the per-core bass program (identical on all cores; per-core data
    differences come entirely from the input tensors)."""
    HD = 128
    assert DIM == H * HD
    HPC = H // NCORES          # heads per core
    CW = HPC * HD              # per-core channel width for q/k/v
    CT = HPC                   # 128-col tiles per group
    NT = 2 * S                 # tokens across both batches
    DC = DIM // 128            # contraction chunks
    TBS = 256                  # token block in phase 1
    NTB = NT // TBS
    SQB = min(512, S)          # sq block in attention
    NSQ = S // SQB
    NST = S // 128             # st (key) chunks per batch
    NTC = NT // 128            # token chunks for resident v
    ONB = DIM // 512           # out-proj N blocks
    WOSL = 2048 if DIM >= 4096 else DIM   # prefetched Wout column prefix

    nc = bacc.Bacc("TRN2", target_bir_lowering=False, debug=False,
                   num_devices=NCORES)

    # host-pretiled inputs: per partition p everything is contiguous, so each
    # DMA is a handful of >=512 B descriptors (full DMA bandwidth)
    hP = nc.dram_tensor("hP", [128, NTB * DC * 2 * TBS], F8,
                        kind="ExternalInput")
    wq8 = nc.dram_tensor("wq8", [128, DC * 2 * CW], F8, kind="ExternalInput")
    wk8 = nc.dram_tensor("wk8", [128, DC * 2 * CW], F8, kind="ExternalInput")
    wv8 = nc.dram_tensor("wv8", [128, DC * 2 * CW], F8, kind="ExternalInput")
    bq = nc.dram_tensor("bq", [128, CT], F32, kind="ExternalInput")
    bk = nc.dram_tensor("bk", [128, CT], F32, kind="ExternalInput")
    wqn = nc.dram_tensor("wqn", [128, CT], F32, kind="ExternalInput")
    wkn = nc.dram_tensor("wkn", [128, CT], F32, kind="ExternalInput")
    cosT = nc.dram_tensor("cosT", [128, S], BF16, kind="ExternalInput")
    sinrT = nc.dram_tensor("sinrT", [128, S], BF16, kind="ExternalInput")
    wo8 = nc.dram_tensor("wo8", [128, 6 * 2 * DIM], F8, kind="ExternalInput")
    outp = nc.dram_tensor("outp", [NT, DIM], BF16, kind="ExternalOutput")

    h5 = hP.rearrange("p (b c j t) -> p b c j t", b=NTB, c=DC, j=2, t=TBS)
    wq4 = wq8.rearrange("p (c j n) -> p c j n", c=DC, j=2, n=CW)
    wk4 = wk8.rearrange("p (c j n) -> p c j n", c=DC, j=2, n=CW)
    wv4 = wv8.rearrange("p (c j n) -> p c j n", c=DC, j=2, n=CW)
    w3o = wo8.rearrange("p (s j n) -> p s j n", s=6, j=2, n=DIM)

    with tile.TileContext(nc) as tc:
        with (
            tc.tile_pool(name="dram", bufs=1, space="DRAM") as dram,
            tc.tile_pool(name="persist", bufs=1) as persist,
        ):
            for _rep in range(repeat):
              qsc = dram.tile([CW, NT], BF16, tag="qsc")
              ksc = dram.tile([CW, NT], BF16, tag="ksc")
              aosc8 = dram.tile([2 * CW, NT], F8, tag="aosc8")
              # ssq partials in partition-major layout [128 p, gi, tb, ts]:
              # produced by transposed ones-matmuls (out free size 1 => ~free
              # on the PE), allreduced as a flat 32 KB buffer
              NSS = 2 * NTB * (TBS // 128)
              cc_in = dram.tile([128, NSS], F32, tag="cc_in")
              cc_out = dram.tile([128, NSS], F32, tag="cc_out")
              rdump = dram.tile([2, NTB * (TBS // 128), 128], BF16,
                                tag="rdump")

              # constants
              ones_f = persist.tile([128, 1], F32, tag="ones_f")
              nc.vector.memset(ones_f[:], 1.0)
              ones_b = persist.tile([128, 1], BF16, tag="ones_b")
              nc.vector.tensor_copy(ones_b[:], ones_f[:])

              bq_t = persist.tile([128, CT], F32, tag="bq")
              nc.scalar.dma_start(bq_t[:], bq[:])
              bk_t = persist.tile([128, CT], F32, tag="bk")
              nc.scalar.dma_start(bk_t[:], bk[:])
              wqn_t = persist.tile([128, CT], F32, tag="wqn")
              nc.scalar.dma_start(wqn_t[:], wqn[:])
              wkn_t = persist.tile([128, CT], F32, tag="wkn")
              nc.scalar.dma_start(wkn_t[:], wkn[:])

              from contextlib import ExitStack as _ES
              _late = _ES()
              # resident v lives from the v projection through attention;
              # pre-reserved before the phase-1 pools (LIFO stack order)
              vresp = _late.enter_context(tc.tile_pool(name="vres", bufs=1))
              vall = vresp.tile([128, NTC, CW], BF16, tag="vall")

              # ---------------- phase 1: projections ----------------------
              with tc.tile_pool(name="hp", bufs=2) as hp:
                def load_hall(tb, pieces=1):
                    hall = hp.tile([128, DC, 2, TBS], F8, tag="hall")
                    cs = DC // pieces
                    for i in range(pieces):
                        nc.sync.dma_start(
                            hall[:, i * cs:(i + 1) * cs, :, :],
                            h5[:, tb, i * cs:(i + 1) * cs, :, :])
                    return hall

                # ---- phase 1a: fused q+k projections + ssq partials ------
                with (
                    tc.tile_pool(name="wqk", bufs=1) as wqk,
                    tc.tile_pool(name="ev", bufs=2) as evp,
                    tc.tile_pool(name="ps", bufs=4, space="PSUM") as psp,
                    tc.tile_pool(name="sq", bufs=2, space="PSUM") as sqp,
                ):
                    # interleave wq / hall0 quarter-loads so the first token
                    # block (which consumes chunks in order) starts as soon
                    # as the first quarter lands
                    wq_t = wqk.tile([128, DC, 2, CW], F8, tag="wq")
                    hall0 = hp.tile([128, DC, 2, TBS], F8, tag="hall")
                    for i in range(8):
                        nc.sync.dma_start(wq_t[:, i * 5:(i + 1) * 5],
                                          wq4[:, i * 5:(i + 1) * 5])
                        nc.sync.dma_start(
                            hall0[:, i * 5:(i + 1) * 5, :, :],
                            h5[:, 0, i * 5:(i + 1) * 5, :, :])
                    hall1 = load_hall(1, pieces=2)
                    wk_t = wqk.tile([128, DC, 2, CW], F8, tag="wk")
                    for i in range(4):
                        nc.sync.dma_start(wk_t[:, i * 10:(i + 1) * 10],
                                          wk4[:, i * 10:(i + 1) * 10])

                    deferred = []
                    ssq_acc = evp.tile([128, 2, NTB, TBS // 128], F32,
                                       tag="ssq_acc")

                    def qk_group(tb, gi, hall):
                        wall, bias_t, spill = (
                            (wq_t, bq_t, qsc), (wk_t, bk_t, ksc))[gi]
                        nts = TBS // 128
                        zz = sqp.tile([128, nts], F32, tag="ssq")
                        evq = evp.tile([128, CT, TBS], BF16, tag="evq")
                        sqt = evp.tile([128, CT, TBS], BF16, tag="sqt")
                        for ct in range(CT):
                            pq = psp.tile([128, TBS], F32, tag="acc")
                            csl = slice(ct * 128, (ct + 1) * 128)
                            # chunk-ordered 3-term interleave: compute
                            # streams behind the chunk-sliced DMAs
                            for cp in range(DC // 2):
                                nc.tensor.matmul(
                                    pq[:],
                                    wall[:, 2 * cp:2 * cp + 2, 0, csl],
                                    hall[:, 2 * cp:2 * cp + 2, 1, :],
                                    start=(cp == 0), stop=False,
                                    perf_mode=DRM)
                                nc.tensor.matmul(
                                    pq[:], wall[:, 2 * cp, :, csl],
                                    hall[:, 2 * cp, :, :],
                                    start=False, stop=False, perf_mode=DRM)
                                nc.tensor.matmul(
                                    pq[:], wall[:, 2 * cp + 1, :, csl],
                                    hall[:, 2 * cp + 1, :, :],
                                    start=False, stop=(cp == DC // 2 - 1),
                                    perf_mode=DRM)
                            # the ssq matmul of the PREVIOUS ct group goes
                            # here so the PE never waits on the DVE square
                            if deferred:
                                deferred.pop(0)()
                            nc.vector.tensor_scalar(
                                evq[:, ct, :], pq[:],
                                bias_t[:, ct:ct + 1], QKV_DESC,
                                mybir.AluOpType.add,
                                mybir.AluOpType.mult)
                            nc.vector.tensor_mul(
                                sqt[:, ct, :], evq[:, ct, :], evq[:, ct, :])

                            def emit_ssq(zz=zz, sqt=sqt, ct=ct,
                                         evq=evq, tb=tb, spill=spill, gi=gi,
                                         nts=nts):
                                # transposed ones-matmul: out free size is 1,
                                # so the partition-dim token sums are nearly
                                # free on the PE (vs 1 cycle/token in the
                                # row-layout version)
                                # one zero-region per PSUM bank: only the
                                # FIRST chain in the shared bank may set
                                # start (it lazily zeroes the whole bank),
                                # only the LAST may set stop
                                for ts in range(nts):
                                    nc.tensor.matmul(
                                        zz[:, ts:ts + 1],
                                        sqt[:, ct, ts * 128:(ts + 1) * 128],
                                        ones_b[:],
                                        start=(ct == 0 and ts == 0),
                                        stop=(ct == CT - 1
                                              and ts == nts - 1))
                                if ct == CT - 1:
                                    # batched bf16 spill of the whole token
                                    # block (one DMA per (tb, gi))
                                    nc.sync.dma_start(
                                        spill.rearrange(
                                            "(c p) t -> p c t", p=128)
                                        [:, :, tb * TBS:(tb + 1) * TBS],
                                        evq[:])
                                    nc.vector.tensor_copy(
                                        ssq_acc[:, gi, tb, :], zz[:])
                            deferred.append(emit_ssq)

                    # startup order q0,q1,k0,k1 hides the wk load behind the
                    # first two q groups (the serial DMA stream needs ~44us
                    # for wq+wk+h0+h1, two q groups give it ~32us of PE work)
                    qk_group(0, 0, hall0)
                    qk_group(1, 0, hall1)
                    qk_group(0, 1, hall0)
                    qk_group(1, 1, hall1)
                    halls = {0: hall0, 1: hall1}
                    for tb in range(2, NTB):
                        hall = load_hall(tb)
                        halls[tb] = hall
                        qk_group(tb, 0, hall)
                        qk_group(tb, 1, hall)
                    while deferred:
                        deferred.pop(0)()
                    nc.gpsimd.dma_start(
                        cc_in.rearrange("p (g c s) -> p g c s", g=2, c=NTB),
                        ssq_acc[:])

                # allreduce the ssq partials (overlaps with the v group)
                if collective:
                    nc.gpsimd.collective_compute(
                        "AllReduce", mybir.AluOpType.add,
                        replica_groups=[list(range(NCORES))],
                        ins=[cc_in[:].opt()], outs=[cc_out[:].opt()])
                else:
                    # single-core timing-sim variant: stand-in for the
                    # allreduce so TimelineSim (no collectives) can run
                    nc.sync.dma_start(cc_out[:], cc_in[:])

                # rms norm factors: tiny partition-major math + the b=0
                # broadcast rows, all overlapped with the v projection
                # (fold the HD**-0.5 attention scale into the q side:
                #  s/sqrt(ssq/DIM+eps) == 1/sqrt(ssq*HD/DIM + HD*eps))
                NTC2 = NTB * (TBS // 128)
                rwork = persist.tile([128, 2, NTC2], F32, tag="rwork")
                rinv = persist.tile([128, 2, NTC2], BF16, tag="rinv")
                nc.scalar.dma_start(
                    rwork[:], cc_out.rearrange("p (g c) -> p g c", g=2))
                for gi in range(2):
                    sc1 = (HD / DIM) if gi == 0 else (1.0 / DIM)
                    sc2 = (HD * EPS) if gi == 0 else EPS
                    nc.vector.tensor_scalar(
                        rwork[:, gi, :], rwork[:, gi, :], sc1, sc2,
                        mybir.AluOpType.mult, mybir.AluOpType.add)
                nc.scalar.activation(
                    rwork[:], rwork[:], mybir.ActivationFunctionType.Sqrt)
                with nc.allow_low_precision(reason="bf16 rms factor"):
                    nc.vector.reciprocal(rinv[:], rwork[:])
                nc.scalar.dma_start(rdump.rearrange("g c p -> p g c"),
                                    rinv[:])
                rd2 = rdump.rearrange("g c p -> g (c p)")
                rb = {}

                def emit_rb(b, pool):
                    for gi in range(2):
                        row = pool.tile([1, S], BF16, tag="rrow",
                                        name="rrow")
                        dma = nc.scalar.dma_start if b == 0 \
                            else nc.sync.dma_start
                        dma(row[:], rd2[gi:gi + 1, b * S:(b + 1) * S])
                        t = pool.tile([128, S], BF16, tag=f"rb{gi}{b}",
                                      name=f"rb{gi}{b}")
                        nc.gpsimd.partition_broadcast(t[:], row[:])
                        rb[(gi, b)] = t

                emit_rb(0, persist)

                # ---- phase 1b: v projection (natural layout) -------------
                # wv loads in chunk slices at v start; the PE streams
                # chunk-ordered behind them (four PSUM tiles in lockstep)
                with (
                    tc.tile_pool(name="wv", bufs=1) as wvp,
                    tc.tile_pool(name="psv", bufs=2, space="PSUM") as psv,
                ):
                    wv_t = wvp.tile([128, DC, 2, CW], F8, tag="wv")
                    for i in range(8):
                        nc.sync.dma_start(wv_t[:, i * 5:(i + 1) * 5],
                                          wv4[:, i * 5:(i + 1) * 5])
                    nsub = TBS // 128
                    nb = CW // 2
                    # reverse order: the last two token blocks' hidden tiles
                    # are still resident from the q/k pass, so the v matmuls
                    # start immediately while the wv weights stream in
                    for tb in list(range(NTB))[::-1]:
                        hall = halls[tb] if tb >= NTB - 2 else load_hall(tb)
                        pv = [[psv.tile([128, nb], F32, tag=f"pv{ts}{i}",
                                        name=f"pv{ts}{i}")
                               for i in range(2)] for ts in range(nsub)]
                        for cp in range(DC // 2):
                            for ts in range(nsub):
                                tsl = slice(ts * 128, (ts + 1) * 128)
                                for i in range(2):
                                    nsl = slice(i * nb, (i + 1) * nb)
                                    nc.tensor.matmul(
                                        pv[ts][i][:],
                                        hall[:, 2 * cp:2 * cp + 2, 1, tsl],
                                        wv_t[:, 2 * cp:2 * cp + 2, 0, nsl],
                                        start=(cp == 0), stop=False,
                                        perf_mode=DRM)
                                    nc.tensor.matmul(
                                        pv[ts][i][:],
                                        hall[:, 2 * cp, :, tsl],
                                        wv_t[:, 2 * cp, :, nsl],
                                        start=False, stop=False,
                                        perf_mode=DRM)
                                    nc.tensor.matmul(
                                        pv[ts][i][:],
                                        hall[:, 2 * cp + 1, :, tsl],
                                        wv_t[:, 2 * cp + 1, :, nsl],
                                        start=False,
                                        stop=(cp == DC // 2 - 1),
                                        perf_mode=DRM)
                        for ts in range(nsub):
                            for i in range(2):
                                # v bias is folded into the host-side output
                                # bias (softmax rows sum to 1); pure
                                # PSUM->SBUF convert-copy on the idle scalar
                                # engine straight into the resident v tile
                                nc.scalar.activation(
                                    vall[:, tb * nsub + ts,
                                         i * nb:(i + 1) * nb],
                                    pv[ts][i][:],
                                    mybir.ActivationFunctionType.Copy,
                                    scale=QKV_DESC)

              # ---------------- phase 3: attention per (batch, head) ------
              _late2 = _ES()
              wprep = _late2.enter_context(tc.tile_pool(name="wpre", bufs=1))
              # the full Wout block + the attn-out ping-pong tiles live in a
              # pool that predates the attention pools, so their DMAs have no
              # WAR on attention tiles and stream during attention
              wot = wprep.tile([128, 6, 2, DIM], F8, tag="wot")
              bhs = [(b, hh) for b in range(2) for hh in range(HPC)]
              with (
                  tc.tile_pool(name="attc", bufs=1) as attc,
                  tc.tile_pool(name="p3", bufs=2) as p3,
                  tc.tile_pool(name="p3e", bufs=4) as p3e,
                  tc.tile_pool(name="ps_sc", bufs=2, space="PSUM") as ps_sc,
                  tc.tile_pool(name="ps_cs", bufs=2, space="PSUM") as ps_cs,
                  tc.tile_pool(name="ps_av", bufs=2, space="PSUM") as ps_av,
              ):
                  cosT_t = attc.tile([128, S], BF16, tag="cosT")
                  nc.gpsimd.dma_start(cosT_t[:], cosT[:])
                  sinrT_t = attc.tile([128, S], BF16, tag="sinrT")
                  nc.gpsimd.dma_start(sinrT_t[:], sinrT[:])

                  def prep(i, nchunk=1):
                      """Load + norm + rope q/k for pair i (bf16 end-to-end;
                      v is already resident).  Emitted ahead so the DVE work
                      overlaps earlier attention.  nchunk>1 interleaves
                      column slices k-first so the first scores matmul only
                      waits for the first k+q slices (used for pair 0)."""
                      b, hh = bhs[i]
                      CS2 = S // nchunk
                      tls = {}
                      for gi, (spill, wn) in enumerate(
                              [(qsc, wqn_t), (ksc, wkn_t)]):
                          xt = p3.tile([128, S], BF16, tag="xt",
                                       name="xt")
                          tmc = p3.tile([128, S], BF16, tag="tmc",
                                        name="tmc")
                          tms = p3.tile([128, S], BF16, tag="tms",
                                        name="tms")
                          xr = p3.tile([128, S], BF16, tag=f"xr{gi}",
                                       name="xr")
                          tls[gi] = (xt, tmc, tms, xr, spill, wn)
                      for cc in range(nchunk):
                          sl = slice(cc * CS2, (cc + 1) * CS2)
                          for gi in (1, 0):
                              xt, tmc, tms, xr, spill, wn = tls[gi]
                              dma = nc.scalar.dma_start if gi == 1 \
                                  else nc.sync.dma_start
                              dma(xt[:, sl],
                                  spill[hh * 128:(hh + 1) * 128,
                                        b * S + cc * CS2:
                                        b * S + (cc + 1) * CS2])
                              # fused (xt * wn) * rb in one DVE op
                              nc.vector.scalar_tensor_tensor(
                                  xt[:, sl], xt[:, sl], wn[:, hh:hh + 1],
                                  rb[(gi, b)][:, sl],
                                  mybir.AluOpType.mult,
                                  mybir.AluOpType.mult)
                              nc.vector.tensor_mul(tmc[:, sl], xt[:, sl],
                                                   cosT_t[:, sl])
                              nc.vector.tensor_mul(
                                  tms[0:64, sl], xt[64:128, sl],
                                  sinrT_t[64:128, sl])
                              nc.vector.tensor_mul(
                                  tms[64:128, sl], xt[0:64, sl],
                                  sinrT_t[0:64, sl])
                              nc.vector.tensor_add(xr[:, sl], tmc[:, sl],
                                                   tms[:, sl])
                      return tls[0][3], tls[1][3]

                  preps = {0: prep(0, nchunk=4)}
                  NSTP = NST // 2
                  NSL = SQB // 128
                  NPU = NSQ * NSTP
                  NTOT = len(bhs) * NPU
                  ets, avs, zzs, cur, aos = {}, {}, {}, {}, {}

                  def norm_evict(i, sqb):
                      # denominators are partition-major [128, NSL]; reshape
                      # to a row via a DRAM bounce (SBUF APs cannot transpose
                      # partitions), then 1/z and the fp8 hi/lo quantization
                      b, hh = bhs[i]
                      aoh8, aol8 = aos[i]
                      av = avs.pop((i, sqb))
                      zz = zzs.pop((i, sqb))
                      z4s = p3.tile([128, NSL], F32, tag="z4s")
                      nc.vector.tensor_copy(z4s[:], zz[:])
                      # copy av out of PSUM immediately so its bank is free
                      # for the sqb two steps ahead even while the z bounce
                      # and later DVE work (rope chains) are still pending
                      av_s = p3.tile([128, SQB], F32, tag="av_s",
                                     name="av_s")
                      nc.vector.tensor_copy(av_s[:], av[:])
                      zd = dram.tile([NSL, 128], F32, tag="zd")
                      nc.gpsimd.dma_start(
                          zd.rearrange("s p -> p s"), z4s[:])
                      rc = p3.tile([1, SQB], F32, tag="rc")
                      nc.gpsimd.dma_start(
                          rc[:], zd.rearrange("s p -> () (s p)"))
                      nc.vector.reciprocal(rc[:], rc[:])
                      nc.vector.tensor_scalar_mul(rc[:], rc[:], SA)
                      rb2 = p3.tile([128, SQB], F32, tag="rb2")
                      nc.gpsimd.partition_broadcast(rb2[:], rc[:])
                      sqsl = slice(sqb * SQB, (sqb + 1) * SQB)
                      ao32 = p3.tile([128, SQB], F32, tag="ao32")
                      nc.vector.tensor_mul(ao32[:], av_s[:], rb2[:])
                      nc.vector.tensor_copy(aoh8[:, sqsl], ao32[:])
                      nc.vector.tensor_tensor(
                          aol8[:, sqsl], ao32[:], aoh8[:, sqsl],
                          mybir.AluOpType.subtract)

                  # single software-pipelined stream over every
                  # (pair, sqb, st-pair) unit, one unit of lookahead: the
                  # scalar engine (exp) is the bottleneck, so the scores
                  # matmuls feeding exp x+1 always precede the av/cs
                  # consumers of exp x -- across sqb AND pair boundaries
                  for x in range(NTOT + 1):
                      if x < NTOT:
                          i, r = divmod(x, NPU)
                          sqb, stp = divmod(r, NSTP)
                          b, hh = bhs[i]
                          if r == 0:
                              cur[i] = preps.pop(i)
                              aos[i] = (
                                  p3.tile([128, S], F8, tag="aoh8",
                                          name="aoh8"),
                                  p3.tile([128, S], F8, tag="aol8",
                                          name="aol8"))
                          qr, kr = cur[i]
                          if stp == 0:
                              zzs[(i, sqb)] = ps_cs.tile(
                                  [128, NSL], F32, tag="zz", name="zz")
                              avs[(i, sqb)] = ps_av.tile(
                                  [128, SQB], F32, tag="av", name="av")
                          sc2 = ps_sc.tile([128, 2, SQB], F32, tag="sc")
                          for hs in range(2):
                              st = 2 * stp + hs
                              nc.tensor.matmul(
                                  sc2[:, hs, :],
                                  kr[:, st * 128:(st + 1) * 128],
                                  qr[:, sqb * SQB:(sqb + 1) * SQB],
                                  start=True, stop=True)
                          et2 = p3e.tile([128, 2, SQB], BF16, tag="et")
                          nc.scalar.activation(
                              et2[:], sc2[:],
                              mybir.ActivationFunctionType.Exp)
                          ets[x] = et2
                      if x >= 1:
                          i, r = divmod(x - 1, NPU)
                          sqb, stp = divmod(r, NSTP)
                          b, hh = bhs[i]
                          et2 = ets.pop(x - 1)
                          for hs in range(2):
                              st = 2 * stp + hs
                              nc.tensor.matmul(
                                  avs[(i, sqb)][:],
                                  vall[:, b * NST + st,
                                       hh * 128:(hh + 1) * 128],
                                  et2[:, hs, :],
                                  start=(st == 0),
                                  stop=(st == NST - 1))
                              # shared-bank chains: single start (bank
                              # zero) / single stop, see phase 1
                              for sl in range(NSL):
                                  nc.tensor.matmul(
                                      zzs[(i, sqb)][:, sl:sl + 1],
                                      et2[:, hs,
                                          sl * 128:(sl + 1) * 128],
                                      ones_b[:],
                                      start=(st == 0 and sl == 0),
                                      stop=(st == NST - 1
                                            and sl == NSL - 1))
                          if stp == NSTP - 1:
                              norm_evict(i, sqb)
                              if i + 1 < len(bhs):
                                  # next pair's rope chain: pair 1 right at
                                  # the first hook (the stream reaches it
                                  # early), later pairs at the second hook
                                  if sqb == (0 if i == 0 else 1):
                                      preps[i + 1] = prep(i + 1)
                              if i == 0:
                                  # off the critical path: the b=1 norm
                                  # factors and the Wout stream for phase 4
                                  # (spread so its transfers do not starve
                                  # the prep loads on the shared DMA bus)
                                  if sqb == 0:
                                      emit_rb(1, attc)
                                  nwq = [4, 2, 2, 2]
                                  base = sum(nwq[:sqb])
                                  for nbw in range(base,
                                                   base + nwq[sqb]):
                                      nc.sync.dma_start(
                                          wot[:, :, :,
                                              nbw * 512:(nbw + 1) * 512],
                                          w3o[:, :, :,
                                              nbw * 512:(nbw + 1) * 512])
                              if sqb == NSQ - 1:
                                  aoh8, aol8 = aos.pop(i)
                                  nc.gpsimd.dma_start(
                                      aosc8[hh * 256 + 128:hh * 256 + 256,
                                            b * S:(b + 1) * S], aoh8[:])
                                  nc.gpsimd.dma_start(
                                      aosc8[hh * 256:hh * 256 + 128,
                                            b * S:(b + 1) * S], aol8[:])

              # -------------- phase 4: partial output projection ----------
              with (
                  tc.tile_pool(name="p4", bufs=4) as p4,
                  tc.tile_pool(name="oe", bufs=2) as oep,
                  tc.tile_pool(name="ps4", bufs=4, space="PSUM") as ps4,
              ):
                  ao4 = aosc8.rearrange("(c j p) t -> p c j t", p=128, j=2)

                  def load_aot(tt):
                      aot = wprep.tile([128, HPC, 2, 128], F8,
                                       tag=f"aot{tt % 2}", name="aot")
                      nc.sync.dma_start(
                          aot[:], ao4[:, :, :, tt * 128:(tt + 1) * 128])
                      return aot

                  aot_next = load_aot(0)
                  for tt in range(NT // 128):
                      aot = aot_next
                      if tt + 1 < NT // 128:
                          aot_next = load_aot(tt + 1)
                      orow = oep.tile([128, DIM], BF16, tag="orow")
                      for nb in range(ONB):
                          wsl = wot[:, :, :, nb * 512:(nb + 1) * 512]
                          po = ps4.tile([128, 512], F32, tag="po")
                          # 8 DoubleRow passes: 2 hi*hi chunk-pairs, 5
                          # crosses, 1 fused hi*hi+lo*lo for the odd chunk
                          nc.tensor.matmul(
                              po[:], aot[:, 0:2, 1, :], wsl[:, 0:2, 0, :],
                              start=True, stop=False, perf_mode=DRM)
                          nc.tensor.matmul(
                              po[:], aot[:, 2:4, 1, :], wsl[:, 2:4, 0, :],
                              start=False, stop=False, perf_mode=DRM)
                          for ch in range(HPC):
                              nc.tensor.matmul(
                                  po[:], aot[:, ch, :, :], wsl[:, ch, :, :],
                                  start=False, stop=False, perf_mode=DRM)
                          nc.tensor.matmul(
                              po[:], aot[:, 4, :, :], wsl[:, 5, :, :],
                              start=False, stop=True, perf_mode=DRM)
                          nc.vector.tensor_scalar_mul(
                              orow[:, nb * 512:(nb + 1) * 512], po[:],
                              OUT_DESC)
                      if tt == NT // 128 - 1:
                          # split the last row-block's writeback so the DMA
                          # overlaps the tail evictions
                          for qq in range(4):
                              qsl = slice(qq * (DIM // 4),
                                          (qq + 1) * (DIM // 4))
                              nc.sync.dma_start(
                                  outp[tt * 128:(tt + 1) * 128, qsl],
                                  orow[:, qsl])
                      else:
                          nc.sync.dma_start(
                              outp[tt * 128:(tt + 1) * 128, :], orow[:])
              _late2.close()
              _late.close()
    nc.finalize()
    return nc


_PROGRAM_CACHE = {}


def _get_program(S, DIM, H):
    key = (S, DIM, H)
    if key not in _PROGRAM_CACHE:
        _PROGRAM_CACHE[key] = build_program(S, DIM, H)
    return _PROGRAM_CACHE[key]


def _split8(x, scale):
    xs = (np.asarray(x, np.float32) * np.float32(scale))
    hi = xs.astype(E4NP)
    lo = (xs - hi.astype(np.float32)).astype(E4NP)
    return hi, lo


def _pack_h(h, DIM, NT, TBS):
    # h [NT, DIM] f32 -> [128, NTB*DC*2*TBS] fp8, j: 0=lo, 1=hi
    DC = DIM // 128
    NTB = NT // TBS
    hh, hl = _split8(h.T, SH)                        # [DIM, NT]
    arr = np.stack([hl, hh])                         # [2(j), DIM, NT]
    arr = arr.reshape(2, DC, 128, NTB, TBS)
    arr = arr.transpose(2, 3, 1, 0, 4)               # [128, NTB, DC, 2, TBS]
    return np.ascontiguousarray(arr.reshape(128, -1))


def _pack_w(Wslice, scale):
    # [DIM, n] -> [128, DC*2*n] with j: 0=hi, 1=lo
    d, n = Wslice.shape
    wh, wl = _split8(Wslice, scale)
    arr = np.stack([wh, wl])                         # [2(j), DIM, n]
    arr = arr.reshape(2, d // 128, 128, n)
    arr = arr.transpose(2, 1, 0, 3)                  # [128, DC, 2, n]
    return np.ascontiguousarray(arr.reshape(128, -1))


def make_in_maps(S, DIM, H, hidden_cond, hidden_uncond, cos_freqs, sin_freqs,
                 Wqkv, bqkv, wq_norm, wk_norm, Wout, bout):
    HD = 128
    HPC = H // NCORES
    CW = HPC * HD
    NT = 2 * S
    TBS = 256
    h = np.concatenate([np.asarray(hidden_cond), np.asarray(hidden_uncond)],
                       axis=0).reshape(NT, DIM)
    hP = _pack_h(h, DIM, NT, TBS)
    cosTb = np.ascontiguousarray(
        np.asarray(cos_freqs).T.astype(BFNP))        # [128, S]
    sinT = np.asarray(sin_freqs).T                   # [128, S]
    HF = HD // 2
    sinrT = np.concatenate([sinT[HF:], -sinT[:HF]], axis=0)
    sinrTb = np.ascontiguousarray(sinrT.astype(BFNP))
    Wqkv = np.asarray(Wqkv)
    bqkv = np.asarray(bqkv)
    wq_norm = np.asarray(wq_norm)
    wk_norm = np.asarray(wk_norm)
    Wout = np.asarray(Wout)

    in_maps = []
    for c in range(NCORES):
        sl = slice(c * CW, (c + 1) * CW)
        bq_c = (bqkv[0 * DIM:1 * DIM][sl] * (SH * SW)).astype(np.float32) \
            .reshape(HPC, HD).T
        bk_c = (bqkv[1 * DIM:2 * DIM][sl] * (SH * SW)).astype(np.float32) \
            .reshape(HPC, HD).T
        # out-proj rows [128, slot, j, n]: slots 0..4 = chunk (hi,lo),
        # slot 5 = chunk 4 as (lo,hi) so hi*hi+lo*lo fuses into one DoubleRow
        woh, wol = _split8(Wout[sl, :], SWO)
        wo8 = np.empty((128, 6, 2, DIM), E4NP)
        for s in range(HPC):
            wo8[:, s, 0] = woh[s * 128:(s + 1) * 128]
            wo8[:, s, 1] = wol[s * 128:(s + 1) * 128]
        wo8[:, 5, 0] = wol[4 * 128:5 * 128]
        wo8[:, 5, 1] = woh[4 * 128:5 * 128]
        in_maps.append({
            "hP": hP,
            "wq8": _pack_w(Wqkv[:, 0 * DIM:1 * DIM][:, sl], SW),
            "wk8": _pack_w(Wqkv[:, 1 * DIM:2 * DIM][:, sl], SW),
            "wv8": _pack_w(Wqkv[:, 2 * DIM:3 * DIM][:, sl], SW),
            "bq": np.ascontiguousarray(bq_c),
            "bk": np.ascontiguousarray(bk_c),
            "wqn": np.ascontiguousarray(wq_norm[sl].reshape(HPC, HD).T
                                        .astype(np.float32)),
            "wkn": np.ascontiguousarray(wk_norm[sl].reshape(HPC, HD).T
                                        .astype(np.float32)),
            "cosT": cosTb,
            "sinrT": sinrTb,
            "wo8": np.ascontiguousarray(wo8.reshape(128, -1)),
        })
    return in_maps


def run(S, DIM, H, inputs):
    nc = _get_program(S, DIM, H)
    in_maps = make_in_maps(S, DIM, H, **inputs)
    res = run_bass_kernel_spmd(nc, in_maps, list(range(NCORES)))
    partial = np.zeros((2 * S, DIM), np.float64)
    for r in res.results:
        partial += np.asarray(r["outp"]).astype(np.float64)
    # the v-bias contribution: softmax rows sum to 1, so attn(v + 1*bv) =
    # attn(v) + 1*bv, and bv flows through Wout as a constant per-channel term
    bv_full = np.asarray(inputs["bqkv"])[2 * DIM:3 * DIM].astype(np.float64)
    const_bias = bv_full @ np.asarray(inputs["Wout"]).astype(np.float64) \
        + np.asarray(inputs["bout"])
    out = (partial + const_bias[None, :]).astype(np.float32)
    out = out.reshape(2, 1, S, DIM)
    return out[0], out[1]


def kernel(hidden_cond, hidden_uncond, cos_freqs, sin_freqs,
           Wqkv, bqkv, wq_norm, wk_norm, Wout, bout):
    B, S, DIM = np.asarray(hidden_cond).shape
    assert B == 1
    H = DIM // 128
    return run(S, DIM, H, dict(
        hidden_cond=hidden_cond, hidden_uncond=hidden_uncond,
        cos_freqs=cos_freqs, sin_freqs=sin_freqs, Wqkv=Wqkv, bqkv=bqkv,
        wq_norm=wq_norm, wk_norm=wk_norm, Wout=Wout, bout=bout))


# revision 36
# speedup vs baseline: 1.1896x; 1.0013x over previous
"""CFG dual self-attention kernel for 8 Trainium2 NeuronCores.

Strategy (tensor parallel on heads):
  - h = concat(hidden_cond, hidden_uncond) -> [4096 tokens, 5120]; host
    pre-transposes AND pre-tiles to hP [128, tb, chunk, hi/lo, tok] so every
    DMA line is a single 20 KB contiguous run per partition (full 360 GB/s;
    the naive transposed layout ran at half speed on 256 B descriptor lines).
  - Each core owns 5 heads (640 of the 5120 q/k/v channels).  One fused pass
    over hP computes qT/kT [640, 4096] (head-dim on partitions, spilled to
    DRAM as bf16) with fp8 hi/lo DoubleRow matmuls; a second pass computes
    v [4096, 640], which stays resident in SBUF (bf16) until attention.
  - RMSNorm over the full 5120 dims needs a cross-core sum of squares:
    partial ssq per token is computed with ones-matmuls on the PE and
    allreduced across the 8 cores (32 KB collective, hidden under the V
    projection).  The 1/sqrt factors and the first attention pair's
    rope chain are also emitted under the V projection (DVE is idle there).
  - Attention per (batch, head) in scores-transposed layout
    scoresT[st, sq] = (rope(k) slice)^T @ rope(q), all in bf16: softmax
    denominators via ones-matmul column sums (interleaved PSUM accumulation
    groups), exp on the scalar engine, A@V accumulated with resident
    v-chunks stationary, and the 1/colsum normalization folded into the
    PSUM->SBUF eviction of attn_outT (quantized fp8 hi/lo for phase 4).
  - Output projection: partial_out = attn_outT^T @ Wout[rows of this core]
    -> [4096, 5120] bf16 per-core partial; host sums the 8 partials (+ bout).
"""

import numpy as np
import ml_dtypes

import concourse.bass as bass  # noqa: F401  (bass types via bacc)
import concourse.mybir as mybir
import concourse.tile as tile
from concourse import bacc
from concourse.bass_utils import run_bass_kernel_spmd

F32 = mybir.dt.float32
F32R = mybir.dt.float32r
BF16 = mybir.dt.bfloat16
F8 = mybir.dt.float8e4
E4NP = ml_dtypes.float8_e4m3
BFNP = ml_dtypes.bfloat16
DRM = mybir.MatmulPerfMode.DoubleRow

NCORES = 8
EPS = 1e-6

# fp8 hi/lo quantization scales (host-side split; 3-term DoubleRow matmuls)
SH = 16.0
SW = 1024.0
SA = 32.0
SWO = 1024.0
QKV_DESC = 1.0 / (SH * SW)
OUT_DESC = 1.0 / (SA * SWO)


def build_program(S, DIM, H, collective=True, repeat=1):
    """Emit the per-core bass program (identical on all cores; per-core data
    differences come entirely from the input tensors)."""
    HD = 128
    assert DIM == H * HD
    HPC = H // NCORES          # heads per core
    CW = HPC * HD              # per-core channel width for q/k/v
    CT = HPC                   # 128-col tiles per group
    NT = 2 * S                 # tokens across both batches
    DC = DIM // 128            # contraction chunks
    TBS = 256                  # token block in phase 1
    NTB = NT // TBS
    SQB = min(512, S)          # sq block in attention
    NSQ = S // SQB
    NST = S // 128             # st (key) chunks per batch
    NTC = NT // 128            # token chunks for resident v
    ONB = DIM // 512           # out-proj N blocks
    WOSL = 2048 if DIM >= 4096 else DIM   # prefetched Wout column prefix

    nc = bacc.Bacc("TRN2", target_bir_lowering=False, debug=False,
                   num_devices=NCORES)

    # host-pretiled inputs: per partition p everything is contiguous, so each
    # DMA is a handful of >=512 B descriptors (full DMA bandwidth)
    hP = nc.dram_tensor("hP", [128, NTB * DC * 2 * TBS], F8,
                        kind="ExternalInput")
    wq8 = nc.dram_tensor("wq8", [128, DC * 2 * CW], F8, kind="ExternalInput")
    wk8 = nc.dram_tensor("wk8", [128, DC * 2 * CW], F8, kind="ExternalInput")
    wv8 = nc.dram_tensor("wv8", [128, DC * 2 * CW], F8, kind="ExternalInput")
    bq = nc.dram_tensor("bq", [128, CT], F32, kind="ExternalInput")
    bk = nc.dram_tensor("bk", [128, CT], F32, kind="ExternalInput")
    wqn = nc.dram_tensor("wqn", [128, CT], F32, kind="ExternalInput")
    wkn = nc.dram_tensor("wkn", [128, CT], F32, kind="ExternalInput")
    cosT = nc.dram_tensor("cosT", [128, S], BF16, kind="ExternalInput")
    sinrT = nc.dram_tensor("sinrT", [128, S], BF16, kind="ExternalInput")
    wo8 = nc.dram_tensor("wo8", [128, 6 * 2 * DIM], F8, kind="ExternalInput")
    outp = nc.dram_tensor("outp", [NT, DIM], BF16, kind="ExternalOutput")

    h5 = hP.rearrange("p (b c j t) -> p b c j t", b=NTB, c=DC, j=2, t=TBS)
    wq4 = wq8.rearrange("p (c j n) -> p c j n", c=DC, j=2, n=CW)
    wk4 = wk8.rearrange("p (c j n) -> p c j n", c=DC, j=2, n=CW)
    wv4 = wv8.rearrange("p (c j n) -> p c j n", c=DC, j=2, n=CW)
    w3o = wo8.rearrange("p (s j n) -> p s j n", s=6, j=2, n=DIM)

    with tile.TileContext(nc) as tc:
        with (
            tc.tile_pool(name="dram", bufs=1, space="DRAM") as dram,
            tc.tile_pool(name="persist", bufs=1) as persist,
        ):
            for _rep in range(repeat):
              qsc = dram.tile([CW, NT], BF16, tag="qsc")
              ksc = dram.tile([CW, NT], BF16, tag="ksc")
              aosc8 = dram.tile([2 * CW, NT], F8, tag="aosc8")
              # ssq partials in partition-major layout [128 p, gi, tb, ts]:
              # produced by transposed ones-matmuls (out free size 1 => ~free
              # on the PE), allreduced as a flat 32 KB buffer
              NSS = 2 * NTB * (TBS // 128)
              cc_in = dram.tile([128, NSS], F32, tag="cc_in")
              cc_out = dram.tile([128, NSS], F32, tag="cc_out")
              rdump = dram.tile([2, NTB * (TBS // 128), 128], BF16,
                                tag="rdump")

              # constants
              ones_f = persist.tile([128, 1], F32, tag="ones_f")
              nc.vector.memset(ones_f[:], 1.0)
              ones_b = persist.tile([128, 1], BF16, tag="ones_b")
              nc.vector.tensor_copy(ones_b[:], ones_f[:])

              bq_t = persist.tile([128, CT], F32, tag="bq")
              nc.scalar.dma_start(bq_t[:], bq[:])
              bk_t = persist.tile([128, CT], F32, tag="bk")
              nc.scalar.dma_start(bk_t[:], bk[:])
              wqn_t = persist.tile([128, CT], F32, tag="wqn")
              nc.scalar.dma_start(wqn_t[:], wqn[:])
              wkn_t = persist.tile([128, CT], F32, tag="wkn")
              nc.scalar.dma_start(wkn_t[:], wkn[:])

              from contextlib import ExitStack as _ES
              _late = _ES()
              # resident v lives from the v projection through attention;
              # pre-reserved before the phase-1 pools (LIFO stack order)
              vresp = _late.enter_context(tc.tile_pool(name="vres", bufs=1))
              vall = vresp.tile([128, NTC, CW], BF16, tag="vall")

              # ---------------- phase 1: projections ----------------------
              with tc.tile_pool(name="hp", bufs=2) as hp:
                def load_hall(tb, pieces=1):
                    hall = hp.tile([128, DC, 2, TBS], F8, tag="hall")
                    cs = DC // pieces
                    for i in range(pieces):
                        nc.sync.dma_start(
                            hall[:, i * cs:(i + 1) * cs, :, :],
                            h5[:, tb, i * cs:(i + 1) * cs, :, :])
                    return hall

                # ---- phase 1a: fused q+k projections + ssq partials ------
                with (
                    tc.tile_pool(name="wqk", bufs=1) as wqk,
                    tc.tile_pool(name="ev", bufs=2) as evp,
                    tc.tile_pool(name="ps", bufs=4, space="PSUM") as psp,
                    tc.tile_pool(name="sq", bufs=2, space="PSUM") as sqp,
                ):
                    # interleave wq / hall0 quarter-loads so the first token
                    # block (which consumes chunks in order) starts as soon
                    # as the first quarter lands
                    wq_t = wqk.tile([128, DC, 2, CW], F8, tag="wq")
                    hall0 = hp.tile([128, DC, 2, TBS], F8, tag="hall")
                    for i in range(8):
                        nc.sync.dma_start(wq_t[:, i * 5:(i + 1) * 5],
                                          wq4[:, i * 5:(i + 1) * 5])
                        nc.sync.dma_start(
                            hall0[:, i * 5:(i + 1) * 5, :, :],
                            h5[:, 0, i * 5:(i + 1) * 5, :, :])
                    hall1 = load_hall(1, pieces=2)
                    wk_t = wqk.tile([128, DC, 2, CW], F8, tag="wk")
                    for i in range(4):
                        nc.sync.dma_start(wk_t[:, i * 10:(i + 1) * 10],
                                          wk4[:, i * 10:(i + 1) * 10])

                    deferred = []
                    ssq_acc = evp.tile([128, 2, NTB, TBS // 128], F32,
                                       tag="ssq_acc")

                    def qk_group(tb, gi, hall):
                        wall, bias_t, spill = (
                            (wq_t, bq_t, qsc), (wk_t, bk_t, ksc))[gi]
                        nts = TBS // 128
                        zz = sqp.tile([128, nts], F32, tag="ssq")
                        evq = evp.tile([128, CT, TBS], BF16, tag="evq")
                        sqt = evp.tile([128, CT, TBS], BF16, tag="sqt")
                        for ct in range(CT):
                            pq = psp.tile([128, TBS], F32, tag="acc")
                            csl = slice(ct * 128, (ct + 1) * 128)
                            # chunk-ordered 3-term interleave: compute
                            # streams behind the chunk-sliced DMAs
                            for cp in range(DC // 2):
                                nc.tensor.matmul(
                                    pq[:],
                                    wall[:, 2 * cp:2 * cp + 2, 0, csl],
                                    hall[:, 2 * cp:2 * cp + 2, 1, :],
                                    start=(cp == 0), stop=False,
                                    perf_mode=DRM)
                                nc.tensor.matmul(
                                    pq[:], wall[:, 2 * cp, :, csl],
                                    hall[:, 2 * cp, :, :],
                                    start=False, stop=False, perf_mode=DRM)
                                nc.tensor.matmul(
                                    pq[:], wall[:, 2 * cp + 1, :, csl],
                                    hall[:, 2 * cp + 1, :, :],
                                    start=False, stop=(cp == DC // 2 - 1),
                                    perf_mode=DRM)
                            # the ssq matmul of the PREVIOUS ct group goes
                            # here so the PE never waits on the DVE square
                            if deferred:
                                deferred.pop(0)()
                            nc.vector.tensor_scalar(
                                evq[:, ct, :], pq[:],
                                bias_t[:, ct:ct + 1], QKV_DESC,
                                mybir.AluOpType.add,
                                mybir.AluOpType.mult)
                            nc.vector.tensor_mul(
                                sqt[:, ct, :], evq[:, ct, :], evq[:, ct, :])

                            def emit_ssq(zz=zz, sqt=sqt, ct=ct,
                                         evq=evq, tb=tb, spill=spill, gi=gi,
                                         nts=nts):
                                # transposed ones-matmul: out free size is 1,
                                # so the partition-dim token sums are nearly
                                # free on the PE (vs 1 cycle/token in the
                                # row-layout version)
                                # one zero-region per PSUM bank: only the
                                # FIRST chain in the shared bank may set
                                # start (it lazily zeroes the whole bank),
                                # only the LAST may set stop
                                for ts in range(nts):
                                    nc.tensor.matmul(
                                        zz[:, ts:ts + 1],
                                        sqt[:, ct, ts * 128:(ts + 1) * 128],
                                        ones_b[:],
                                        start=(ct == 0 and ts == 0),
                                        stop=(ct == CT - 1
                                              and ts == nts - 1))
                                if ct == CT - 1:
                                    # batched bf16 spill of the whole token
                                    # block (one DMA per (tb, gi))
                                    nc.sync.dma_start(
                                        spill.rearrange(
                                            "(c p) t -> p c t", p=128)
                                        [:, :, tb * TBS:(tb + 1) * TBS],
                                        evq[:])
                                    nc.vector.tensor_copy(
                                        ssq_acc[:, gi, tb, :], zz[:])
                            deferred.append(emit_ssq)

                    # startup order q0,q1,k0,k1 hides the wk load behind the
                    # first two q groups (the serial DMA stream needs ~44us
                    # for wq+wk+h0+h1, two q groups give it ~32us of PE work)
                    qk_group(0, 0, hall0)
                    qk_group(1, 0, hall1)
                    qk_group(0, 1, hall0)
                    qk_group(1, 1, hall1)
                    halls = {0: hall0, 1: hall1}
                    for tb in range(2, NTB):
                        hall = load_hall(tb)
                        halls[tb] = hall
                        qk_group(tb, 0, hall)
                        qk_group(tb, 1, hall)
                    while deferred:
                        deferred.pop(0)()
                    nc.gpsimd.dma_start(
                        cc_in.rearrange("p (g c s) -> p g c s", g=2, c=NTB),
                        ssq_acc[:])

                # allreduce the ssq partials (overlaps with the v group)
                if collective:
                    nc.gpsimd.collective_compute(
                        "AllReduce", mybir.AluOpType.add,
                        replica_groups=[list(range(NCORES))],
                        ins=[cc_in[:].opt()], outs=[cc_out[:].opt()])
                else:
                    # single-core timing-sim variant: stand-in for the
                    # allreduce so TimelineSim (no collectives) can run
                    nc.sync.dma_start(cc_out[:], cc_in[:])

                # rms norm factors: tiny partition-major math + the b=0
                # broadcast rows, all overlapped with the v projection
                # (fold the HD**-0.5 attention scale into the q side:
                #  s/sqrt(ssq/DIM+eps) == 1/sqrt(ssq*HD/DIM + HD*eps))
                NTC2 = NTB * (TBS // 128)
                rwork = persist.tile([128, 2, NTC2], F32, tag="rwork")
                rinv = persist.tile([128, 2, NTC2], BF16, tag="rinv")
                nc.scalar.dma_start(
                    rwork[:], cc_out.rearrange("p (g c) -> p g c", g=2))
                for gi in range(2):
                    sc1 = (HD / DIM) if gi == 0 else (1.0 / DIM)
                    sc2 = (HD * EPS) if gi == 0 else EPS
                    nc.vector.tensor_scalar(
                        rwork[:, gi, :], rwork[:, gi, :], sc1, sc2,
                        mybir.AluOpType.mult, mybir.AluOpType.add)
                nc.scalar.activation(
                    rwork[:], rwork[:], mybir.ActivationFunctionType.Sqrt)
                with nc.allow_low_precision(reason="bf16 rms factor"):
                    nc.vector.reciprocal(rinv[:], rwork[:])
                nc.scalar.dma_start(rdump.rearrange("g c p -> p g c"),
                                    rinv[:])
                rd2 = rdump.rearrange("g c p -> g (c p)")
                rb = {}

                def emit_rb(b, pool):
                    for gi in range(2):
                        row = pool.tile([1, S], BF16, tag="rrow",
                                        name="rrow")
                        dma = nc.scalar.dma_start if b == 0 \
                            else nc.sync.dma_start
                        dma(row[:], rd2[gi:gi + 1, b * S:(b + 1) * S])
                        t = pool.tile([128, S], BF16, tag=f"rb{gi}{b}",
                                      name=f"rb{gi}{b}")
                        nc.gpsimd.partition_broadcast(t[:], row[:])
                        rb[(gi, b)] = t

                emit_rb(0, persist)

                # ---- phase 1b: v projection (natural layout) -------------
                # wv loads in chunk slices at v start; the PE streams
                # chunk-ordered behind them (four PSUM tiles in lockstep)
                with (
                    tc.tile_pool(name="wv", bufs=1) as wvp,
                    tc.tile_pool(name="psv", bufs=2, space="PSUM") as psv,
                ):
                    wv_t = wvp.tile([128, DC, 2, CW], F8, tag="wv")
                    for i in range(8):
                        nc.sync.dma_start(wv_t[:, i * 5:(i + 1) * 5],
                                          wv4[:, i * 5:(i + 1) * 5])
                    nsub = TBS // 128
                    nb = CW // 2
                    # reverse order: the last two token blocks' hidden tiles
                    # are still resident from the q/k pass, so the v matmuls
                    # start immediately while the wv weights stream in
                    for tb in list(range(NTB))[::-1]:
                        hall = halls[tb] if tb >= NTB - 2 else load_hall(tb)
                        pv = [[psv.tile([128, nb], F32, tag=f"pv{ts}{i}",
                                        name=f"pv{ts}{i}")
                               for i in range(2)] for ts in range(nsub)]
                        for cp in range(DC // 2):
                            for ts in range(nsub):
                                tsl = slice(ts * 128, (ts + 1) * 128)
                                for i in range(2):
                                    nsl = slice(i * nb, (i + 1) * nb)
                                    nc.tensor.matmul(
                                        pv[ts][i][:],
                                        hall[:, 2 * cp:2 * cp + 2, 1, tsl],
                                        wv_t[:, 2 * cp:2 * cp + 2, 0, nsl],
                                        start=(cp == 0), stop=False,
                                        perf_mode=DRM)
                                    nc.tensor.matmul(
                                        pv[ts][i][:],
                                        hall[:, 2 * cp, :, tsl],
                                        wv_t[:, 2 * cp, :, nsl],
                                        start=False, stop=False,
                                        perf_mode=DRM)
                                    nc.tensor.matmul(
                                        pv[ts][i][:],
                                        hall[:, 2 * cp + 1, :, tsl],
                                        wv_t[:, 2 * cp + 1, :, nsl],
                                        start=False,
                                        stop=(cp == DC // 2 - 1),
                                        perf_mode=DRM)
                        for ts in range(nsub):
                            for i in range(2):
                                # v bias is folded into the host-side output
                                # bias (softmax rows sum to 1); pure
                                # PSUM->SBUF convert-copy on the idle scalar
                                # engine straight into the resident v tile
                                nc.scalar.activation(
                                    vall[:, tb * nsub + ts,
                                         i * nb:(i + 1) * nb],
                                    pv[ts][i][:],
                                    mybir.ActivationFunctionType.Copy,
                                    scale=QKV_DESC)

              # ---------------- phase 3: attention per (batch, head) ------
              _late2 = _ES()
              wprep = _late2.enter_context(tc.tile_pool(name="wpre", bufs=1))
              # the full Wout block + the attn-out ping-pong tiles live in a
              # pool that predates the attention pools, so their DMAs have no
              # WAR on attention tiles and stream during attention
              wot = wprep.tile([128, 6, 2, DIM], F8, tag="wot")
              bhs = [(b, hh) for b in range(2) for hh in range(HPC)]
              with (
                  tc.tile_pool(name="attc", bufs=1) as attc,
                  tc.tile_pool(name="p3", bufs=2) as p3,
                  tc.tile_pool(name="p3e", bufs=4) as p3e,
                  tc.tile_pool(name="ps_sc", bufs=2, space="PSUM") as ps_sc,
                  tc.tile_pool(name="ps_cs", bufs=2, space="PSUM") as ps_cs,
                  tc.tile_pool(name="ps_av", bufs=2, space="PSUM") as ps_av,
              ):
                  cosT_t = attc.tile([128, S], BF16, tag="cosT")
                  nc.gpsimd.dma_start(cosT_t[:], cosT[:])
                  sinrT_t = attc.tile([128, S], BF16, tag="sinrT")
                  nc.gpsimd.dma_start(sinrT_t[:], sinrT[:])

                  def prep_gi(i, gi):
                      """One of pair i's two rope chains (see prep); emitted
                      separately so each in-order DVE insertion stays short
                      enough to not starve the eviction cadence."""
                      b, hh = bhs[i]
                      spill, wn = ((qsc, wqn_t), (ksc, wkn_t))[gi]
                      xt = p3.tile([128, S], BF16, tag="xt", name="xt")
                      tmc = p3.tile([128, S], BF16, tag="tmc", name="tmc")
                      tms = p3.tile([128, S], BF16, tag="tms", name="tms")
                      xr = p3.tile([128, S], BF16, tag=f"xr{gi}", name="xr")
                      dma = nc.scalar.dma_start if gi == 1 \
                          else nc.sync.dma_start
                      dma(xt[:], spill[hh * 128:(hh + 1) * 128,
                                       b * S:(b + 1) * S])
                      nc.vector.scalar_tensor_tensor(
                          xt[:], xt[:], wn[:, hh:hh + 1], rb[(gi, b)][:],
                          mybir.AluOpType.mult, mybir.AluOpType.mult)
                      nc.vector.tensor_mul(tmc[:], xt[:], cosT_t[:])
                      nc.vector.tensor_mul(
                          tms[0:64, :], xt[64:128, :], sinrT_t[64:128, :])
                      nc.vector.tensor_mul(
                          tms[64:128, :], xt[0:64, :], sinrT_t[0:64, :])
                      nc.vector.tensor_add(xr[:], tmc[:], tms[:])
                      return xr

                  def prep(i, nchunk=1):
                      """Load + norm + rope q/k for pair i (bf16 end-to-end;
                      v is already resident).  Emitted ahead so the DVE work
                      overlaps earlier attention.  nchunk>1 interleaves
                      column slices k-first so the first scores matmul only
                      waits for the first k+q slices (used for pair 0)."""
                      b, hh = bhs[i]
                      CS2 = S // nchunk
                      tls = {}
                      for gi, (spill, wn) in enumerate(
                              [(qsc, wqn_t), (ksc, wkn_t)]):
                          xt = p3.tile([128, S], BF16, tag="xt",
                                       name="xt")
                          tmc = p3.tile([128, S], BF16, tag="tmc",
                                        name="tmc")
                          tms = p3.tile([128, S], BF16, tag="tms",
                                        name="tms")
                          xr = p3.tile([128, S], BF16, tag=f"xr{gi}",
                                       name="xr")
                          tls[gi] = (xt, tmc, tms, xr, spill, wn)
                      for cc in range(nchunk):
                          sl = slice(cc * CS2, (cc + 1) * CS2)
                          for gi in (1, 0):
                              xt, tmc, tms, xr, spill, wn = tls[gi]
                              dma = nc.scalar.dma_start if gi == 1 \
                                  else nc.sync.dma_start
                              dma(xt[:, sl],
                                  spill[hh * 128:(hh + 1) * 128,
                                        b * S + cc * CS2:
                                        b * S + (cc + 1) * CS2])
                              # fused (xt * wn) * rb in one DVE op
                              nc.vector.scalar_tensor_tensor(
                                  xt[:, sl], xt[:, sl], wn[:, hh:hh + 1],
                                  rb[(gi, b)][:, sl],
                                  mybir.AluOpType.mult,
                                  mybir.AluOpType.mult)
                              nc.vector.tensor_mul(tmc[:, sl], xt[:, sl],
                                                   cosT_t[:, sl])
                              nc.vector.tensor_mul(
                                  tms[0:64, sl], xt[64:128, sl],
                                  sinrT_t[64:128, sl])
                              nc.vector.tensor_mul(
                                  tms[64:128, sl], xt[0:64, sl],
                                  sinrT_t[0:64, sl])
                              nc.vector.tensor_add(xr[:, sl], tmc[:, sl],
                                                   tms[:, sl])
                      return tls[0][3], tls[1][3]

                  preps = {0: prep(0, nchunk=4)}
                  NSTP = NST // 2
                  NSL = SQB // 128
                  NPU = NSQ * NSTP
                  NTOT = len(bhs) * NPU
                  ets, avs, zzs, cur, aos = {}, {}, {}, {}, {}

                  def norm_evict(i, sqb):
                      # denominators are partition-major [128, NSL]; reshape
                      # to a row via a DRAM bounce (SBUF APs cannot transpose
                      # partitions), then 1/z and the fp8 hi/lo quantization
                      b, hh = bhs[i]
                      aoh8, aol8 = aos[i]
                      av = avs.pop((i, sqb))
                      zz = zzs.pop((i, sqb))
                      z4s = p3.tile([128, NSL], F32, tag="z4s")
                      nc.vector.tensor_copy(z4s[:], zz[:])
                      # copy av out of PSUM immediately so its bank is free
                      # for the sqb two steps ahead even while the z bounce
                      # and later DVE work (rope chains) are still pending
                      av_s = p3.tile([128, SQB], F32, tag="av_s",
                                     name="av_s")
                      nc.vector.tensor_copy(av_s[:], av[:])
                      zd = dram.tile([NSL, 128], F32, tag="zd")
                      nc.gpsimd.dma_start(
                          zd.rearrange("s p -> p s"), z4s[:])
                      rc = p3.tile([1, SQB], F32, tag="rc")
                      nc.gpsimd.dma_start(
                          rc[:], zd.rearrange("s p -> () (s p)"))
                      nc.vector.reciprocal(rc[:], rc[:])
                      nc.vector.tensor_scalar_mul(rc[:], rc[:], SA)
                      rb2 = p3.tile([128, SQB], F32, tag="rb2")
                      nc.gpsimd.partition_broadcast(rb2[:], rc[:])
                      sqsl = slice(sqb * SQB, (sqb + 1) * SQB)
                      ao32 = p3.tile([128, SQB], F32, tag="ao32")
                      nc.vector.tensor_mul(ao32[:], av_s[:], rb2[:])
                      nc.vector.tensor_copy(aoh8[:, sqsl], ao32[:])
                      nc.vector.tensor_tensor(
                          aol8[:, sqsl], ao32[:], aoh8[:, sqsl],
                          mybir.AluOpType.subtract)

                  # single software-pipelined stream over every
                  # (pair, sqb, st-pair) unit, one unit of lookahead: the
                  # scalar engine (exp) is the bottleneck, so the scores
                  # matmuls feeding exp x+1 always precede the av/cs
                  # consumers of exp x -- across sqb AND pair boundaries
                  for x in range(NTOT + 1):
                      if x < NTOT:
                          i, r = divmod(x, NPU)
                          sqb, stp = divmod(r, NSTP)
                          b, hh = bhs[i]
                          if r == 0:
                              cur[i] = preps.pop(i)
                              aos[i] = (
                                  p3.tile([128, S], F8, tag="aoh8",
                                          name="aoh8"),
                                  p3.tile([128, S], F8, tag="aol8",
                                          name="aol8"))
                          qr, kr = cur[i]
                          if stp == 0:
                              zzs[(i, sqb)] = ps_cs.tile(
                                  [128, NSL], F32, tag="zz", name="zz")
                              avs[(i, sqb)] = ps_av.tile(
                                  [128, SQB], F32, tag="av", name="av")
                          sc2 = ps_sc.tile([128, 2, SQB], F32, tag="sc")
                          for hs in range(2):
                              st = 2 * stp + hs
                              nc.tensor.matmul(
                                  sc2[:, hs, :],
                                  kr[:, st * 128:(st + 1) * 128],
                                  qr[:, sqb * SQB:(sqb + 1) * SQB],
                                  start=True, stop=True)
                          et2 = p3e.tile([128, 2, SQB], BF16, tag="et")
                          nc.scalar.activation(
                              et2[:], sc2[:],
                              mybir.ActivationFunctionType.Exp)
                          ets[x] = et2
                      if x >= 1:
                          i, r = divmod(x - 1, NPU)
                          sqb, stp = divmod(r, NSTP)
                          b, hh = bhs[i]
                          et2 = ets.pop(x - 1)
                          for hs in range(2):
                              st = 2 * stp + hs
                              nc.tensor.matmul(
                                  avs[(i, sqb)][:],
                                  vall[:, b * NST + st,
                                       hh * 128:(hh + 1) * 128],
                                  et2[:, hs, :],
                                  start=(st == 0),
                                  stop=(st == NST - 1))
                              # shared-bank chains: single start (bank
                              # zero) / single stop, see phase 1
                              for sl in range(NSL):
                                  nc.tensor.matmul(
                                      zzs[(i, sqb)][:, sl:sl + 1],
                                      et2[:, hs,
                                          sl * 128:(sl + 1) * 128],
                                      ones_b[:],
                                      start=(st == 0 and sl == 0),
                                      stop=(st == NST - 1
                                            and sl == NSL - 1))
                          if stp == NSTP - 1:
                              norm_evict(i, sqb)
                              if i + 1 < len(bhs):
                                  # next pair's rope chains, k then q split
                                  # across the first two hooks (pair 1 all
                                  # at once -- the stream reaches it early):
                                  # each DVE insertion is ~5us so neither
                                  # the eviction cadence nor the next
                                  # pair's scores starve
                                  if i == 0:
                                      if sqb == 0:
                                          preps[1] = prep(1)
                                  elif sqb == 0:
                                      preps[i + 1] = [None,
                                                      prep_gi(i + 1, 1)]
                                  elif sqb == 1:
                                      preps[i + 1][0] = prep_gi(i + 1, 0)
                              if i == 0:
                                  # off the critical path: the b=1 norm
                                  # factors and the Wout stream for phase 4
                                  # (spread so its transfers do not starve
                                  # the prep loads on the shared DMA bus)
                                  if sqb == 0:
                                      emit_rb(1, attc)
                                  nwq = [4, 2, 2, 2]
                                  base = sum(nwq[:sqb])
                                  for nbw in range(base,
                                                   base + nwq[sqb]):
                                      nc.sync.dma_start(
                                          wot[:, :, :,
                                              nbw * 512:(nbw + 1) * 512],
                                          w3o[:, :, :,
                                              nbw * 512:(nbw + 1) * 512])
                              if sqb == NSQ - 1:
                                  # SP queue: the pool queue carries the
                                  # z bounces and would delay these, and
                                  # their completion releases the aoh8
                                  # buffers two pairs later
                                  aoh8, aol8 = aos.pop(i)
                                  nc.sync.dma_start(
                                      aosc8[hh * 256 + 128:hh * 256 + 256,
                                            b * S:(b + 1) * S], aoh8[:])
                                  nc.sync.dma_start(
                                      aosc8[hh * 256:hh * 256 + 128,
                                            b * S:(b + 1) * S], aol8[:])

              # -------------- phase 4: partial output projection ----------
              with (
                  tc.tile_pool(name="p4", bufs=4) as p4,
                  tc.tile_pool(name="oe", bufs=2) as oep,
                  tc.tile_pool(name="ps4", bufs=4, space="PSUM") as ps4,
              ):
                  ao4 = aosc8.rearrange("(c j p) t -> p c j t", p=128, j=2)

                  def load_aot(tt):
                      aot = wprep.tile([128, HPC, 2, 128], F8,
                                       tag=f"aot{tt % 2}", name="aot")
                      nc.sync.dma_start(
                          aot[:], ao4[:, :, :, tt * 128:(tt + 1) * 128])
                      return aot

                  aot_next = load_aot(0)
                  for tt in range(NT // 128):
                      aot = aot_next
                      if tt + 1 < NT // 128:
                          aot_next = load_aot(tt + 1)
                      orow = oep.tile([128, DIM], BF16, tag="orow")
                      for nb in range(ONB):
                          wsl = wot[:, :, :, nb * 512:(nb + 1) * 512]
                          po = ps4.tile([128, 512], F32, tag="po")
                          # 8 DoubleRow passes: 2 hi*hi chunk-pairs, 5
                          # crosses, 1 fused hi*hi+lo*lo for the odd chunk
                          nc.tensor.matmul(
                              po[:], aot[:, 0:2, 1, :], wsl[:, 0:2, 0, :],
                              start=True, stop=False, perf_mode=DRM)
                          nc.tensor.matmul(
                              po[:], aot[:, 2:4, 1, :], wsl[:, 2:4, 0, :],
                              start=False, stop=False, perf_mode=DRM)
                          for ch in range(HPC):
                              nc.tensor.matmul(
                                  po[:], aot[:, ch, :, :], wsl[:, ch, :, :],
                                  start=False, stop=False, perf_mode=DRM)
                          nc.tensor.matmul(
                              po[:], aot[:, 4, :, :], wsl[:, 5, :, :],
                              start=False, stop=True, perf_mode=DRM)
                          nc.vector.tensor_scalar_mul(
                              orow[:, nb * 512:(nb + 1) * 512], po[:],
                              OUT_DESC)
                      if tt == NT // 128 - 1:
                          # split the last row-block's writeback so the DMA
                          # overlaps the tail evictions
                          for qq in range(4):
                              qsl = slice(qq * (DIM // 4),
                                          (qq + 1) * (DIM // 4))
                              nc.sync.dma_start(
                                  outp[tt * 128:(tt + 1) * 128, qsl],
                                  orow[:, qsl])
                      else:
                          nc.sync.dma_start(
                              outp[tt * 128:(tt + 1) * 128, :], orow[:])
              _late2.close()
              _late.close()
    nc.finalize()
    return nc


_PROGRAM_CACHE = {}


def _get_program(S, DIM, H):
    key = (S, DIM, H)
    if key not in _PROGRAM_CACHE:
        _PROGRAM_CACHE[key] = build_program(S, DIM, H)
    return _PROGRAM_CACHE[key]


def _split8(x, scale):
    xs = (np.asarray(x, np.float32) * np.float32(scale))
    hi = xs.astype(E4NP)
    lo = (xs - hi.astype(np.float32)).astype(E4NP)
    return hi, lo


def _pack_h(h, DIM, NT, TBS):
    # h [NT, DIM] f32 -> [128, NTB*DC*2*TBS] fp8, j: 0=lo, 1=hi
    DC = DIM // 128
    NTB = NT // TBS
    hh, hl = _split8(h.T, SH)                        # [DIM, NT]
    arr = np.stack([hl, hh])                         # [2(j), DIM, NT]
    arr = arr.reshape(2, DC, 128, NTB, TBS)
    arr = arr.transpose(2, 3, 1, 0, 4)               # [128, NTB, DC, 2, TBS]
    return np.ascontiguousarray(arr.reshape(128, -1))


def _pack_w(Wslice, scale):
    # [DIM, n] -> [128, DC*2*n] with j: 0=hi, 1=lo
    d, n = Wslice.shape
    wh, wl = _split8(Wslice, scale)
    arr = np.stack([wh, wl])                         # [2(j), DIM, n]
    arr = arr.reshape(2, d // 128, 128, n)
    arr = arr.transpose(2, 1, 0, 3)                  # [128, DC, 2, n]
    return np.ascontiguousarray(arr.reshape(128, -1))


def make_in_maps(S, DIM, H, hidden_cond, hidden_uncond, cos_freqs, sin_freqs,
                 Wqkv, bqkv, wq_norm, wk_norm, Wout, bout):
    HD = 128
    HPC = H // NCORES
    CW = HPC * HD
    NT = 2 * S
    TBS = 256
    h = np.concatenate([np.asarray(hidden_cond), np.asarray(hidden_uncond)],
                       axis=0).reshape(NT, DIM)
    hP = _pack_h(h, DIM, NT, TBS)
    cosTb = np.ascontiguousarray(
        np.asarray(cos_freqs).T.astype(BFNP))        # [128, S]
    sinT = np.asarray(sin_freqs).T                   # [128, S]
    HF = HD // 2
    sinrT = np.concatenate([sinT[HF:], -sinT[:HF]], axis=0)
    sinrTb = np.ascontiguousarray(sinrT.astype(BFNP))
    Wqkv = np.asarray(Wqkv)
    bqkv = np.asarray(bqkv)
    wq_norm = np.asarray(wq_norm)
    wk_norm = np.asarray(wk_norm)
    Wout = np.asarray(Wout)

    in_maps = []
    for c in range(NCORES):
        sl = slice(c * CW, (c + 1) * CW)
        bq_c = (bqkv[0 * DIM:1 * DIM][sl] * (SH * SW)).astype(np.float32) \
            .reshape(HPC, HD).T
        bk_c = (bqkv[1 * DIM:2 * DIM][sl] * (SH * SW)).astype(np.float32) \
            .reshape(HPC, HD).T
        # out-proj rows [128, slot, j, n]: slots 0..4 = chunk (hi,lo),
        # slot 5 = chunk 4 as (lo,hi) so hi*hi+lo*lo fuses into one DoubleRow
        woh, wol = _split8(Wout[sl, :], SWO)
        wo8 = np.empty((128, 6, 2, DIM), E4NP)
        for s in range(HPC):
            wo8[:, s, 0] = woh[s * 128:(s + 1) * 128]
            wo8[:, s, 1] = wol[s * 128:(s + 1) * 128]
        wo8[:, 5, 0] = wol[4 * 128:5 * 128]
        wo8[:, 5, 1] = woh[4 * 128:5 * 128]
        in_maps.append({
            "hP": hP,
            "wq8": _pack_w(Wqkv[:, 0 * DIM:1 * DIM][:, sl], SW),
            "wk8": _pack_w(Wqkv[:, 1 * DIM:2 * DIM][:, sl], SW),
            "wv8": _pack_w(Wqkv[:, 2 * DIM:3 * DIM][:, sl], SW),
            "bq": np.ascontiguousarray(bq_c),
            "bk": np.ascontiguousarray(bk_c),
            "wqn": np.ascontiguousarray(wq_norm[sl].reshape(HPC, HD).T
                                        .astype(np.float32)),
            "wkn": np.ascontiguousarray(wk_norm[sl].reshape(HPC, HD).T
                                        .astype(np.float32)),
            "cosT": cosTb,
            "sinrT": sinrTb,
            "wo8": np.ascontiguousarray(wo8.reshape(128, -1)),
        })
    return in_maps


def run(S, DIM, H, inputs):
    nc = _get_program(S, DIM, H)
    in_maps = make_in_maps(S, DIM, H, **inputs)
    res = run_bass_kernel_spmd(nc, in_maps, list(range(NCORES)))
    partial = np.zeros((2 * S, DIM), np.float64)
    for r in res.results:
        partial += np.asarray(r["outp"]).astype(np.float64)
    # the v-bias contribution: softmax rows sum to 1, so attn(v + 1*bv) =
    # attn(v) + 1*bv, and bv flows through Wout as a constant per-channel term
    bv_full = np.asarray(inputs["bqkv"])[2 * DIM:3 * DIM].astype(np.float64)
    const_bias = bv_full @ np.asarray(inputs["Wout"]).astype(np.float64) \
        + np.asarray(inputs["bout"])
    out = (partial + const_bias[None, :]).astype(np.float32)
    out = out.reshape(2, 1, S, DIM)
    return out[0], out[1]


def kernel(hidden_cond, hidden_uncond, cos_freqs, sin_freqs,
           Wqkv, bqkv, wq_norm, wk_norm, Wout, bout):
    B, S, DIM = np.asarray(hidden_cond).shape
    assert B == 1
    H = DIM // 128
    return run(S, DIM, H, dict(
        hidden_cond=hidden_cond, hidden_uncond=hidden_uncond,
        cos_freqs=cos_freqs, sin_freqs=sin_freqs, Wqkv=Wqkv, bqkv=bqkv,
        wq_norm=wq_norm, wk_norm=wk_norm, Wout=Wout, bout=bout))


# revision 39
# speedup vs baseline: 1.2055x; 1.0134x over previous
"""CFG dual self-attention kernel for 8 Trainium2 NeuronCores.

Strategy (tensor parallel on heads):
  - h = concat(hidden_cond, hidden_uncond) -> [4096 tokens, 5120]; host
    pre-transposes AND pre-tiles to hP [128, tb, chunk, hi/lo, tok] so every
    DMA line is a single 20 KB contiguous run per partition (full 360 GB/s;
    the naive transposed layout ran at half speed on 256 B descriptor lines).
  - Each core owns 5 heads (640 of the 5120 q/k/v channels).  One fused pass
    over hP computes qT/kT [640, 4096] (head-dim on partitions, spilled to
    DRAM as bf16) with fp8 hi/lo DoubleRow matmuls; a second pass computes
    v [4096, 640], which stays resident in SBUF (bf16) until attention.
  - RMSNorm over the full 5120 dims needs a cross-core sum of squares:
    partial ssq per token is computed with ones-matmuls on the PE and
    allreduced across the 8 cores (32 KB collective, hidden under the V
    projection).  The 1/sqrt factors and the first attention pair's
    rope chain are also emitted under the V projection (DVE is idle there).
  - Attention per (batch, head) in scores-transposed layout
    scoresT[st, sq] = (rope(k) slice)^T @ rope(q), all in bf16: softmax
    denominators via ones-matmul column sums (interleaved PSUM accumulation
    groups), exp on the scalar engine, A@V accumulated with resident
    v-chunks stationary, and the 1/colsum normalization folded into the
    PSUM->SBUF eviction of attn_outT (quantized fp8 hi/lo for phase 4).
  - Output projection: partial_out = attn_outT^T @ Wout[rows of this core]
    -> [4096, 5120] bf16 per-core partial; host sums the 8 partials (+ bout).
"""

import numpy as np
import ml_dtypes

import concourse.bass as bass  # noqa: F401  (bass types via bacc)
import concourse.mybir as mybir
import concourse.tile as tile
from concourse import bacc
from concourse.bass_utils import run_bass_kernel_spmd

F32 = mybir.dt.float32
F32R = mybir.dt.float32r
BF16 = mybir.dt.bfloat16
F8 = mybir.dt.float8e4
E4NP = ml_dtypes.float8_e4m3
BFNP = ml_dtypes.bfloat16
DRM = mybir.MatmulPerfMode.DoubleRow

NCORES = 8
EPS = 1e-6

# fp8 hi/lo quantization scales (host-side split; 3-term DoubleRow matmuls)
SH = 16.0
SW = 1024.0
SA = 32.0
SWO = 1024.0
QKV_DESC = 1.0 / (SH * SW)
OUT_DESC = 1.0 / (SA * SWO)


def build_program(S, DIM, H, collective=True, repeat=1):
    """Emit the per-core bass program (identical on all cores; per-core data
    differences come entirely from the input tensors)."""
    HD = 128
    assert DIM == H * HD
    HPC = H // NCORES          # heads per core
    CW = HPC * HD              # per-core channel width for q/k/v
    CT = HPC                   # 128-col tiles per group
    NT = 2 * S                 # tokens across both batches
    DC = DIM // 128            # contraction chunks
    TBS = 256                  # token block in phase 1
    NTB = NT // TBS
    SQB = min(512, S)          # sq block in attention
    NSQ = S // SQB
    NST = S // 128             # st (key) chunks per batch
    NTC = NT // 128            # token chunks for resident v
    ONB = DIM // 512           # out-proj N blocks
    WOSL = 2048 if DIM >= 4096 else DIM   # prefetched Wout column prefix

    nc = bacc.Bacc("TRN2", target_bir_lowering=False, debug=False,
                   num_devices=NCORES)

    # host-pretiled inputs: per partition p everything is contiguous, so each
    # DMA is a handful of >=512 B descriptors (full DMA bandwidth)
    hP = nc.dram_tensor("hP", [128, NTB * DC * 2 * TBS], F8,
                        kind="ExternalInput")
    wq8 = nc.dram_tensor("wq8", [128, DC * 2 * CW], F8, kind="ExternalInput")
    wk8 = nc.dram_tensor("wk8", [128, DC * 2 * CW], F8, kind="ExternalInput")
    wv8 = nc.dram_tensor("wv8", [128, DC * 2 * CW], F8, kind="ExternalInput")
    bq = nc.dram_tensor("bq", [128, CT], F32, kind="ExternalInput")
    bk = nc.dram_tensor("bk", [128, CT], F32, kind="ExternalInput")
    wqn = nc.dram_tensor("wqn", [128, CT], F32, kind="ExternalInput")
    wkn = nc.dram_tensor("wkn", [128, CT], F32, kind="ExternalInput")
    cosT = nc.dram_tensor("cosT", [128, S], BF16, kind="ExternalInput")
    sinrT = nc.dram_tensor("sinrT", [128, S], BF16, kind="ExternalInput")
    wo8 = nc.dram_tensor("wo8", [128, 6 * 2 * DIM], F8, kind="ExternalInput")
    outp = nc.dram_tensor("outp", [NT, DIM], BF16, kind="ExternalOutput")

    h5 = hP.rearrange("p (b c j t) -> p b c j t", b=NTB, c=DC, j=2, t=TBS)
    wq4 = wq8.rearrange("p (c j n) -> p c j n", c=DC, j=2, n=CW)
    wk4 = wk8.rearrange("p (c j n) -> p c j n", c=DC, j=2, n=CW)
    wv4 = wv8.rearrange("p (c j n) -> p c j n", c=DC, j=2, n=CW)
    w3o = wo8.rearrange("p (s j n) -> p s j n", s=6, j=2, n=DIM)

    with tile.TileContext(nc) as tc:
        with (
            tc.tile_pool(name="dram", bufs=1, space="DRAM") as dram,
            tc.tile_pool(name="persist", bufs=1) as persist,
        ):
            for _rep in range(repeat):
              qsc = dram.tile([CW, NT], BF16, tag="qsc")
              ksc = dram.tile([CW, NT], BF16, tag="ksc")
              aosc8 = dram.tile([2 * CW, NT], F8, tag="aosc8")
              # ssq partials in partition-major layout [128 p, gi, tb, ts]:
              # produced by transposed ones-matmuls (out free size 1 => ~free
              # on the PE), allreduced as a flat 32 KB buffer
              NSS = 2 * NTB * (TBS // 128)
              cc_in = dram.tile([128, NSS], F32, tag="cc_in")
              cc_out = dram.tile([128, NSS], F32, tag="cc_out")
              rdump = dram.tile([2, NTB * (TBS // 128), 128], BF16,
                                tag="rdump")

              # constants
              ones_f = persist.tile([128, 1], F32, tag="ones_f")
              nc.vector.memset(ones_f[:], 1.0)
              ones_b = persist.tile([128, 1], BF16, tag="ones_b")
              nc.vector.tensor_copy(ones_b[:], ones_f[:])

              bq_t = persist.tile([128, CT], F32, tag="bq")
              nc.scalar.dma_start(bq_t[:], bq[:])
              bk_t = persist.tile([128, CT], F32, tag="bk")
              nc.scalar.dma_start(bk_t[:], bk[:])
              wqn_t = persist.tile([128, CT], F32, tag="wqn")
              nc.scalar.dma_start(wqn_t[:], wqn[:])
              wkn_t = persist.tile([128, CT], F32, tag="wkn")
              nc.scalar.dma_start(wkn_t[:], wkn[:])

              from contextlib import ExitStack as _ES
              _late = _ES()
              # resident v lives from the v projection through attention;
              # pre-reserved before the phase-1 pools (LIFO stack order)
              vresp = _late.enter_context(tc.tile_pool(name="vres", bufs=1))
              vall = vresp.tile([128, NTC, CW], BF16, tag="vall")

              # ---------------- phase 1: projections ----------------------
              with tc.tile_pool(name="hp", bufs=2) as hp:
                def load_hall(tb, pieces=1):
                    hall = hp.tile([128, DC, 2, TBS], F8, tag="hall")
                    cs = DC // pieces
                    for i in range(pieces):
                        nc.sync.dma_start(
                            hall[:, i * cs:(i + 1) * cs, :, :],
                            h5[:, tb, i * cs:(i + 1) * cs, :, :])
                    return hall

                # ---- phase 1a: fused q+k projections + ssq partials ------
                with (
                    tc.tile_pool(name="wqk", bufs=1) as wqk,
                    tc.tile_pool(name="ev", bufs=2) as evp,
                    tc.tile_pool(name="ps", bufs=4, space="PSUM") as psp,
                    tc.tile_pool(name="sq", bufs=2, space="PSUM") as sqp,
                ):
                    # interleave wq / hall0 quarter-loads so the first token
                    # block (which consumes chunks in order) starts as soon
                    # as the first quarter lands
                    wq_t = wqk.tile([128, DC, 2, CW], F8, tag="wq")
                    hall0 = hp.tile([128, DC, 2, TBS], F8, tag="hall")
                    for i in range(8):
                        nc.sync.dma_start(wq_t[:, i * 5:(i + 1) * 5],
                                          wq4[:, i * 5:(i + 1) * 5])
                        nc.sync.dma_start(
                            hall0[:, i * 5:(i + 1) * 5, :, :],
                            h5[:, 0, i * 5:(i + 1) * 5, :, :])
                    hall1 = load_hall(1, pieces=2)
                    wk_t = wqk.tile([128, DC, 2, CW], F8, tag="wk")
                    for i in range(4):
                        nc.sync.dma_start(wk_t[:, i * 10:(i + 1) * 10],
                                          wk4[:, i * 10:(i + 1) * 10])

                    deferred = []
                    ssq_acc = evp.tile([128, 2, NTB, TBS // 128], F32,
                                       tag="ssq_acc")

                    def qk_group(tb, gi, hall):
                        wall, bias_t, spill = (
                            (wq_t, bq_t, qsc), (wk_t, bk_t, ksc))[gi]
                        nts = TBS // 128
                        zz = sqp.tile([128, nts], F32, tag="ssq")
                        evq = evp.tile([128, CT, TBS], BF16, tag="evq")
                        sqt = evp.tile([128, CT, TBS], BF16, tag="sqt")
                        for ct in range(CT):
                            pq = psp.tile([128, TBS], F32, tag="acc")
                            csl = slice(ct * 128, (ct + 1) * 128)
                            # chunk-ordered 3-term interleave: compute
                            # streams behind the chunk-sliced DMAs
                            for cp in range(DC // 2):
                                nc.tensor.matmul(
                                    pq[:],
                                    wall[:, 2 * cp:2 * cp + 2, 0, csl],
                                    hall[:, 2 * cp:2 * cp + 2, 1, :],
                                    start=(cp == 0), stop=False,
                                    perf_mode=DRM)
                                nc.tensor.matmul(
                                    pq[:], wall[:, 2 * cp, :, csl],
                                    hall[:, 2 * cp, :, :],
                                    start=False, stop=False, perf_mode=DRM)
                                nc.tensor.matmul(
                                    pq[:], wall[:, 2 * cp + 1, :, csl],
                                    hall[:, 2 * cp + 1, :, :],
                                    start=False, stop=(cp == DC // 2 - 1),
                                    perf_mode=DRM)
                            # the ssq matmul of the PREVIOUS ct group goes
                            # here so the PE never waits on the DVE square
                            if deferred:
                                deferred.pop(0)()
                            nc.vector.tensor_scalar(
                                evq[:, ct, :], pq[:],
                                bias_t[:, ct:ct + 1], QKV_DESC,
                                mybir.AluOpType.add,
                                mybir.AluOpType.mult)
                            nc.vector.tensor_mul(
                                sqt[:, ct, :], evq[:, ct, :], evq[:, ct, :])

                            def emit_ssq(zz=zz, sqt=sqt, ct=ct,
                                         evq=evq, tb=tb, spill=spill, gi=gi,
                                         nts=nts):
                                # transposed ones-matmul: out free size is 1,
                                # so the partition-dim token sums are nearly
                                # free on the PE (vs 1 cycle/token in the
                                # row-layout version)
                                # one zero-region per PSUM bank: only the
                                # FIRST chain in the shared bank may set
                                # start (it lazily zeroes the whole bank),
                                # only the LAST may set stop
                                for ts in range(nts):
                                    nc.tensor.matmul(
                                        zz[:, ts:ts + 1],
                                        sqt[:, ct, ts * 128:(ts + 1) * 128],
                                        ones_b[:],
                                        start=(ct == 0 and ts == 0),
                                        stop=(ct == CT - 1
                                              and ts == nts - 1))
                                if ct == CT - 1:
                                    # batched bf16 spill of the whole token
                                    # block (one DMA per (tb, gi))
                                    nc.sync.dma_start(
                                        spill.rearrange(
                                            "(c p) t -> p c t", p=128)
                                        [:, :, tb * TBS:(tb + 1) * TBS],
                                        evq[:])
                                    nc.vector.tensor_copy(
                                        ssq_acc[:, gi, tb, :], zz[:])
                            deferred.append(emit_ssq)

                    # startup order q0,q1,k0,k1 hides the wk load behind the
                    # first two q groups (the serial DMA stream needs ~44us
                    # for wq+wk+h0+h1, two q groups give it ~32us of PE work)
                    qk_group(0, 0, hall0)
                    qk_group(1, 0, hall1)
                    qk_group(0, 1, hall0)
                    qk_group(1, 1, hall1)
                    halls = {0: hall0, 1: hall1}
                    for tb in range(2, NTB):
                        hall = load_hall(tb)
                        halls[tb] = hall
                        qk_group(tb, 0, hall)
                        qk_group(tb, 1, hall)
                    while deferred:
                        deferred.pop(0)()
                    nc.gpsimd.dma_start(
                        cc_in.rearrange("p (g c s) -> p g c s", g=2, c=NTB),
                        ssq_acc[:])

                # allreduce the ssq partials (overlaps with the v group)
                if collective:
                    nc.gpsimd.collective_compute(
                        "AllReduce", mybir.AluOpType.add,
                        replica_groups=[list(range(NCORES))],
                        ins=[cc_in[:].opt()], outs=[cc_out[:].opt()])
                else:
                    # single-core timing-sim variant: stand-in for the
                    # allreduce so TimelineSim (no collectives) can run
                    nc.sync.dma_start(cc_out[:], cc_in[:])

                # rms norm factors: tiny partition-major math + the b=0
                # broadcast rows, all overlapped with the v projection
                # (fold the HD**-0.5 attention scale into the q side:
                #  s/sqrt(ssq/DIM+eps) == 1/sqrt(ssq*HD/DIM + HD*eps))
                NTC2 = NTB * (TBS // 128)
                rwork = persist.tile([128, 2, NTC2], F32, tag="rwork")
                rinv = persist.tile([128, 2, NTC2], BF16, tag="rinv")
                nc.scalar.dma_start(
                    rwork[:], cc_out.rearrange("p (g c) -> p g c", g=2))
                for gi in range(2):
                    sc1 = (HD / DIM) if gi == 0 else (1.0 / DIM)
                    sc2 = (HD * EPS) if gi == 0 else EPS
                    nc.vector.tensor_scalar(
                        rwork[:, gi, :], rwork[:, gi, :], sc1, sc2,
                        mybir.AluOpType.mult, mybir.AluOpType.add)
                nc.scalar.activation(
                    rwork[:], rwork[:], mybir.ActivationFunctionType.Sqrt)
                with nc.allow_low_precision(reason="bf16 rms factor"):
                    nc.vector.reciprocal(rinv[:], rwork[:])
                nc.scalar.dma_start(rdump.rearrange("g c p -> p g c"),
                                    rinv[:])
                rd2 = rdump.rearrange("g c p -> g (c p)")
                rb = {}

                def emit_rb(b, pool):
                    for gi in range(2):
                        row = pool.tile([1, S], BF16, tag="rrow",
                                        name="rrow")
                        dma = nc.scalar.dma_start if b == 0 \
                            else nc.sync.dma_start
                        dma(row[:], rd2[gi:gi + 1, b * S:(b + 1) * S])
                        t = pool.tile([128, S], BF16, tag=f"rb{gi}{b}",
                                      name=f"rb{gi}{b}")
                        nc.gpsimd.partition_broadcast(t[:], row[:])
                        rb[(gi, b)] = t

                emit_rb(0, persist)

                # ---- phase 1b: v projection (natural layout) -------------
                # wv loads in chunk slices at v start; the PE streams
                # chunk-ordered behind them (four PSUM tiles in lockstep)
                with (
                    tc.tile_pool(name="wv", bufs=1) as wvp,
                    tc.tile_pool(name="psv", bufs=2, space="PSUM") as psv,
                ):
                    wv_t = wvp.tile([128, DC, 2, CW], F8, tag="wv")
                    for i in range(8):
                        nc.sync.dma_start(wv_t[:, i * 5:(i + 1) * 5],
                                          wv4[:, i * 5:(i + 1) * 5])
                    nsub = TBS // 128
                    nb = CW // 2
                    # reverse order: the last two token blocks' hidden tiles
                    # are still resident from the q/k pass, so the v matmuls
                    # start immediately while the wv weights stream in
                    for tb in list(range(NTB))[::-1]:
                        hall = halls[tb] if tb >= NTB - 2 else load_hall(tb)
                        pv = [[psv.tile([128, nb], F32, tag=f"pv{ts}{i}",
                                        name=f"pv{ts}{i}")
                               for i in range(2)] for ts in range(nsub)]
                        for cp in range(DC // 2):
                            for ts in range(nsub):
                                tsl = slice(ts * 128, (ts + 1) * 128)
                                for i in range(2):
                                    nsl = slice(i * nb, (i + 1) * nb)
                                    nc.tensor.matmul(
                                        pv[ts][i][:],
                                        hall[:, 2 * cp:2 * cp + 2, 1, tsl],
                                        wv_t[:, 2 * cp:2 * cp + 2, 0, nsl],
                                        start=(cp == 0), stop=False,
                                        perf_mode=DRM)
                                    nc.tensor.matmul(
                                        pv[ts][i][:],
                                        hall[:, 2 * cp, :, tsl],
                                        wv_t[:, 2 * cp, :, nsl],
                                        start=False, stop=False,
                                        perf_mode=DRM)
                                    nc.tensor.matmul(
                                        pv[ts][i][:],
                                        hall[:, 2 * cp + 1, :, tsl],
                                        wv_t[:, 2 * cp + 1, :, nsl],
                                        start=False,
                                        stop=(cp == DC // 2 - 1),
                                        perf_mode=DRM)
                        for ts in range(nsub):
                            for i in range(2):
                                # v bias is folded into the host-side output
                                # bias (softmax rows sum to 1); pure
                                # PSUM->SBUF convert-copy on the idle scalar
                                # engine straight into the resident v tile
                                nc.scalar.activation(
                                    vall[:, tb * nsub + ts,
                                         i * nb:(i + 1) * nb],
                                    pv[ts][i][:],
                                    mybir.ActivationFunctionType.Copy,
                                    scale=QKV_DESC)

              # ---------------- phase 3: attention per (batch, head) ------
              _late2 = _ES()
              wprep = _late2.enter_context(tc.tile_pool(name="wpre", bufs=1))
              # the full Wout block + the attn-out ping-pong tiles live in a
              # pool that predates the attention pools, so their DMAs have no
              # WAR on attention tiles and stream during attention
              wot = wprep.tile([128, 6, 2, DIM], F8, tag="wot")
              bhs = [(b, hh) for b in range(2) for hh in range(HPC)]
              with (
                  tc.tile_pool(name="attc", bufs=1) as attc,
                  tc.tile_pool(name="p3", bufs=2) as p3,
                  tc.tile_pool(name="p3e", bufs=4) as p3e,
                  tc.tile_pool(name="ps_sc", bufs=2, space="PSUM") as ps_sc,
                  tc.tile_pool(name="ps_cs", bufs=2, space="PSUM") as ps_cs,
                  tc.tile_pool(name="ps_av", bufs=2, space="PSUM") as ps_av,
              ):
                  cosT_t = attc.tile([128, S], BF16, tag="cosT")
                  nc.gpsimd.dma_start(cosT_t[:], cosT[:])
                  sinrT_t = attc.tile([128, S], BF16, tag="sinrT")
                  nc.gpsimd.dma_start(sinrT_t[:], sinrT[:])

                  def prep_gi(i, gi):
                      """One of pair i's two rope chains (see prep); emitted
                      separately so each in-order DVE insertion stays short
                      enough to not starve the eviction cadence."""
                      b, hh = bhs[i]
                      spill, wn = ((qsc, wqn_t), (ksc, wkn_t))[gi]
                      xt = p3.tile([128, S], BF16, tag="xt", name="xt")
                      tmc = p3.tile([128, S], BF16, tag="tmc", name="tmc")
                      tms = p3.tile([128, S], BF16, tag="tms", name="tms")
                      xr = p3.tile([128, S], BF16, tag=f"xr{gi}", name="xr")
                      dma = nc.scalar.dma_start if gi == 1 \
                          else nc.sync.dma_start
                      dma(xt[:], spill[hh * 128:(hh + 1) * 128,
                                       b * S:(b + 1) * S])
                      nc.vector.scalar_tensor_tensor(
                          xt[:], xt[:], wn[:, hh:hh + 1], rb[(gi, b)][:],
                          mybir.AluOpType.mult, mybir.AluOpType.mult)
                      nc.vector.tensor_mul(tmc[:], xt[:], cosT_t[:])
                      nc.vector.tensor_mul(
                          tms[0:64, :], xt[64:128, :], sinrT_t[64:128, :])
                      nc.vector.tensor_mul(
                          tms[64:128, :], xt[0:64, :], sinrT_t[0:64, :])
                      nc.vector.tensor_add(xr[:], tmc[:], tms[:])
                      return xr

                  def prep(i, nchunk=1):
                      """Load + norm + rope q/k for pair i (bf16 end-to-end;
                      v is already resident).  Emitted ahead so the DVE work
                      overlaps earlier attention.  nchunk>1 interleaves
                      column slices k-first so the first scores matmul only
                      waits for the first k+q slices (used for pair 0)."""
                      b, hh = bhs[i]
                      CS2 = S // nchunk
                      tls = {}
                      for gi, (spill, wn) in enumerate(
                              [(qsc, wqn_t), (ksc, wkn_t)]):
                          xt = p3.tile([128, S], BF16, tag="xt",
                                       name="xt")
                          tmc = p3.tile([128, S], BF16, tag="tmc",
                                        name="tmc")
                          tms = p3.tile([128, S], BF16, tag="tms",
                                        name="tms")
                          xr = p3.tile([128, S], BF16, tag=f"xr{gi}",
                                       name="xr")
                          tls[gi] = (xt, tmc, tms, xr, spill, wn)
                      for cc in range(nchunk):
                          sl = slice(cc * CS2, (cc + 1) * CS2)
                          for gi in (1, 0):
                              xt, tmc, tms, xr, spill, wn = tls[gi]
                              dma = nc.scalar.dma_start if gi == 1 \
                                  else nc.sync.dma_start
                              dma(xt[:, sl],
                                  spill[hh * 128:(hh + 1) * 128,
                                        b * S + cc * CS2:
                                        b * S + (cc + 1) * CS2])
                              # fused (xt * wn) * rb in one DVE op
                              nc.vector.scalar_tensor_tensor(
                                  xt[:, sl], xt[:, sl], wn[:, hh:hh + 1],
                                  rb[(gi, b)][:, sl],
                                  mybir.AluOpType.mult,
                                  mybir.AluOpType.mult)
                              nc.vector.tensor_mul(tmc[:, sl], xt[:, sl],
                                                   cosT_t[:, sl])
                              nc.vector.tensor_mul(
                                  tms[0:64, sl], xt[64:128, sl],
                                  sinrT_t[64:128, sl])
                              nc.vector.tensor_mul(
                                  tms[64:128, sl], xt[0:64, sl],
                                  sinrT_t[0:64, sl])
                              nc.vector.tensor_add(xr[:, sl], tmc[:, sl],
                                                   tms[:, sl])
                      return tls[0][3], tls[1][3]

                  preps = {0: prep(0, nchunk=4)}
                  NSTP = NST // 2
                  NSL = SQB // 128
                  NPU = NSQ * NSTP
                  NTOT = len(bhs) * NPU
                  ets, avs, zzs, cur, aos = {}, {}, {}, {}, {}

                  def norm_evict(i, sqb):
                      # denominators are partition-major [128, NSL]; reshape
                      # to a row via a DRAM bounce (SBUF APs cannot transpose
                      # partitions), then 1/z and the fp8 hi/lo quantization
                      b, hh = bhs[i]
                      aoh8, aol8 = aos[i]
                      av = avs.pop((i, sqb))
                      zz = zzs.pop((i, sqb))
                      z4s = p3.tile([128, NSL], F32, tag="z4s")
                      nc.vector.tensor_copy(z4s[:], zz[:])
                      # copy av out of PSUM immediately so its bank is free
                      # for the sqb two steps ahead even while the z bounce
                      # and later DVE work (rope chains) are still pending
                      av_s = p3.tile([128, SQB], F32, tag="av_s",
                                     name="av_s")
                      nc.vector.tensor_copy(av_s[:], av[:])
                      zd = dram.tile([NSL, 128], F32, tag="zd")
                      nc.sync.dma_start(
                          zd.rearrange("s p -> p s"), z4s[:])
                      rc = p3.tile([1, SQB], F32, tag="rc")
                      nc.sync.dma_start(
                          rc[:], zd.rearrange("s p -> () (s p)"))
                      nc.vector.reciprocal(rc[:], rc[:])
                      nc.vector.tensor_scalar_mul(rc[:], rc[:], SA)
                      rb2 = p3.tile([128, SQB], F32, tag="rb2")
                      nc.gpsimd.partition_broadcast(rb2[:], rc[:])
                      sqsl = slice(sqb * SQB, (sqb + 1) * SQB)
                      ao32 = p3.tile([128, SQB], F32, tag="ao32")
                      nc.vector.tensor_mul(ao32[:], av_s[:], rb2[:])
                      nc.vector.tensor_copy(aoh8[:, sqsl], ao32[:])
                      nc.vector.tensor_tensor(
                          aol8[:, sqsl], ao32[:], aoh8[:, sqsl],
                          mybir.AluOpType.subtract)

                  # single software-pipelined stream over every
                  # (pair, sqb, st-pair) unit, one unit of lookahead: the
                  # scalar engine (exp) is the bottleneck, so the scores
                  # matmuls feeding exp x+1 always precede the av/cs
                  # consumers of exp x -- across sqb AND pair boundaries
                  for x in range(NTOT + 1):
                      if x < NTOT:
                          i, r = divmod(x, NPU)
                          sqb, stp = divmod(r, NSTP)
                          b, hh = bhs[i]
                          if r == 0:
                              cur[i] = preps.pop(i)
                              aos[i] = (
                                  p3.tile([128, S], F8, tag="aoh8",
                                          name="aoh8"),
                                  p3.tile([128, S], F8, tag="aol8",
                                          name="aol8"))
                          qr, kr = cur[i]
                          if stp == 0:
                              zzs[(i, sqb)] = ps_cs.tile(
                                  [128, NSL], F32, tag="zz", name="zz")
                              avs[(i, sqb)] = ps_av.tile(
                                  [128, SQB], F32, tag="av", name="av")
                          sc2 = ps_sc.tile([128, 2, SQB], F32, tag="sc")
                          for hs in range(2):
                              st = 2 * stp + hs
                              nc.tensor.matmul(
                                  sc2[:, hs, :],
                                  kr[:, st * 128:(st + 1) * 128],
                                  qr[:, sqb * SQB:(sqb + 1) * SQB],
                                  start=True, stop=True)
                          et2 = p3e.tile([128, 2, SQB], BF16, tag="et")
                          nc.scalar.activation(
                              et2[:], sc2[:],
                              mybir.ActivationFunctionType.Exp)
                          ets[x] = et2
                      if x >= 1:
                          i, r = divmod(x - 1, NPU)
                          sqb, stp = divmod(r, NSTP)
                          b, hh = bhs[i]
                          et2 = ets.pop(x - 1)
                          for hs in range(2):
                              st = 2 * stp + hs
                              nc.tensor.matmul(
                                  avs[(i, sqb)][:],
                                  vall[:, b * NST + st,
                                       hh * 128:(hh + 1) * 128],
                                  et2[:, hs, :],
                                  start=(st == 0),
                                  stop=(st == NST - 1))
                              # shared-bank chains: single start (bank
                              # zero) / single stop, see phase 1
                              for sl in range(NSL):
                                  nc.tensor.matmul(
                                      zzs[(i, sqb)][:, sl:sl + 1],
                                      et2[:, hs,
                                          sl * 128:(sl + 1) * 128],
                                      ones_b[:],
                                      start=(st == 0 and sl == 0),
                                      stop=(st == NST - 1
                                            and sl == NSL - 1))
                          if stp == NSTP - 1:
                              norm_evict(i, sqb)
                              if i + 1 < len(bhs):
                                  # next pair's rope chains, k then q split
                                  # across the first two hooks (pair 1 all
                                  # at once -- the stream reaches it early):
                                  # each DVE insertion is ~5us so neither
                                  # the eviction cadence nor the next
                                  # pair's scores starve
                                  if i == 0:
                                      if sqb == 0:
                                          preps[1] = prep(1)
                                  elif sqb == 0:
                                      preps[i + 1] = [None,
                                                      prep_gi(i + 1, 1)]
                                  elif sqb == 1:
                                      preps[i + 1][0] = prep_gi(i + 1, 0)
                              if i == 0:
                                  # off the critical path: the b=1 norm
                                  # factors and the Wout stream for phase 4
                                  # (spread so its transfers do not starve
                                  # the prep loads on the shared DMA bus)
                                  if sqb == 0:
                                      emit_rb(1, attc)
                                  nwq = [4, 2, 2, 2]
                                  base = sum(nwq[:sqb])
                                  for nbw in range(base,
                                                   base + nwq[sqb]):
                                      nc.sync.dma_start(
                                          wot[:, :, :,
                                              nbw * 512:(nbw + 1) * 512],
                                          w3o[:, :, :,
                                              nbw * 512:(nbw + 1) * 512])
                              if sqb == NSQ - 1:
                                  # SP queue: the pool queue carries the
                                  # z bounces and would delay these, and
                                  # their completion releases the aoh8
                                  # buffers two pairs later
                                  aoh8, aol8 = aos.pop(i)
                                  nc.sync.dma_start(
                                      aosc8[hh * 256 + 128:hh * 256 + 256,
                                            b * S:(b + 1) * S], aoh8[:])
                                  nc.sync.dma_start(
                                      aosc8[hh * 256:hh * 256 + 128,
                                            b * S:(b + 1) * S], aol8[:])

              # -------------- phase 4: partial output projection ----------
              with (
                  tc.tile_pool(name="p4", bufs=4) as p4,
                  tc.tile_pool(name="oe", bufs=2) as oep,
                  tc.tile_pool(name="ps4", bufs=4, space="PSUM") as ps4,
              ):
                  ao4 = aosc8.rearrange("(c j p) t -> p c j t", p=128, j=2)

                  def load_aot(tt):
                      aot = wprep.tile([128, HPC, 2, 128], F8,
                                       tag=f"aot{tt % 2}", name="aot")
                      nc.sync.dma_start(
                          aot[:], ao4[:, :, :, tt * 128:(tt + 1) * 128])
                      return aot

                  aot_next = load_aot(0)
                  for tt in range(NT // 128):
                      aot = aot_next
                      if tt + 1 < NT // 128:
                          aot_next = load_aot(tt + 1)
                      orow = oep.tile([128, DIM], BF16, tag="orow")
                      for nb in range(ONB):
                          wsl = wot[:, :, :, nb * 512:(nb + 1) * 512]
                          po = ps4.tile([128, 512], F32, tag="po")
                          # 8 DoubleRow passes: 2 hi*hi chunk-pairs, 5
                          # crosses, 1 fused hi*hi+lo*lo for the odd chunk
                          nc.tensor.matmul(
                              po[:], aot[:, 0:2, 1, :], wsl[:, 0:2, 0, :],
                              start=True, stop=False, perf_mode=DRM)
                          nc.tensor.matmul(
                              po[:], aot[:, 2:4, 1, :], wsl[:, 2:4, 0, :],
                              start=False, stop=False, perf_mode=DRM)
                          for ch in range(HPC):
                              nc.tensor.matmul(
                                  po[:], aot[:, ch, :, :], wsl[:, ch, :, :],
                                  start=False, stop=False, perf_mode=DRM)
                          nc.tensor.matmul(
                              po[:], aot[:, 4, :, :], wsl[:, 5, :, :],
                              start=False, stop=True, perf_mode=DRM)
                          nc.vector.tensor_scalar_mul(
                              orow[:, nb * 512:(nb + 1) * 512], po[:],
                              OUT_DESC)
                      if tt == NT // 128 - 1:
                          # split the last row-block's writeback so the DMA
                          # overlaps the tail evictions
                          for qq in range(4):
                              qsl = slice(qq * (DIM // 4),
                                          (qq + 1) * (DIM // 4))
                              nc.sync.dma_start(
                                  outp[tt * 128:(tt + 1) * 128, qsl],
                                  orow[:, qsl])
                      else:
                          nc.sync.dma_start(
                              outp[tt * 128:(tt + 1) * 128, :], orow[:])
              _late2.close()
              _late.close()
    nc.finalize()
    return nc


_PROGRAM_CACHE = {}


def _get_program(S, DIM, H):
    key = (S, DIM, H)
    if key not in _PROGRAM_CACHE:
        _PROGRAM_CACHE[key] = build_program(S, DIM, H)
    return _PROGRAM_CACHE[key]


def _split8(x, scale):
    xs = (np.asarray(x, np.float32) * np.float32(scale))
    hi = xs.astype(E4NP)
    lo = (xs - hi.astype(np.float32)).astype(E4NP)
    return hi, lo


def _pack_h(h, DIM, NT, TBS):
    # h [NT, DIM] f32 -> [128, NTB*DC*2*TBS] fp8, j: 0=lo, 1=hi
    DC = DIM // 128
    NTB = NT // TBS
    hh, hl = _split8(h.T, SH)                        # [DIM, NT]
    arr = np.stack([hl, hh])                         # [2(j), DIM, NT]
    arr = arr.reshape(2, DC, 128, NTB, TBS)
    arr = arr.transpose(2, 3, 1, 0, 4)               # [128, NTB, DC, 2, TBS]
    return np.ascontiguousarray(arr.reshape(128, -1))


def _pack_w(Wslice, scale):
    # [DIM, n] -> [128, DC*2*n] with j: 0=hi, 1=lo
    d, n = Wslice.shape
    wh, wl = _split8(Wslice, scale)
    arr = np.stack([wh, wl])                         # [2(j), DIM, n]
    arr = arr.reshape(2, d // 128, 128, n)
    arr = arr.transpose(2, 1, 0, 3)                  # [128, DC, 2, n]
    return np.ascontiguousarray(arr.reshape(128, -1))


def make_in_maps(S, DIM, H, hidden_cond, hidden_uncond, cos_freqs, sin_freqs,
                 Wqkv, bqkv, wq_norm, wk_norm, Wout, bout):
    HD = 128
    HPC = H // NCORES
    CW = HPC * HD
    NT = 2 * S
    TBS = 256
    h = np.concatenate([np.asarray(hidden_cond), np.asarray(hidden_uncond)],
                       axis=0).reshape(NT, DIM)
    hP = _pack_h(h, DIM, NT, TBS)
    cosTb = np.ascontiguousarray(
        np.asarray(cos_freqs).T.astype(BFNP))        # [128, S]
    sinT = np.asarray(sin_freqs).T                   # [128, S]
    HF = HD // 2
    sinrT = np.concatenate([sinT[HF:], -sinT[:HF]], axis=0)
    sinrTb = np.ascontiguousarray(sinrT.astype(BFNP))
    Wqkv = np.asarray(Wqkv)
    bqkv = np.asarray(bqkv)
    wq_norm = np.asarray(wq_norm)
    wk_norm = np.asarray(wk_norm)
    Wout = np.asarray(Wout)

    in_maps = []
    for c in range(NCORES):
        sl = slice(c * CW, (c + 1) * CW)
        bq_c = (bqkv[0 * DIM:1 * DIM][sl] * (SH * SW)).astype(np.float32) \
            .reshape(HPC, HD).T
        bk_c = (bqkv[1 * DIM:2 * DIM][sl] * (SH * SW)).astype(np.float32) \
            .reshape(HPC, HD).T
        # out-proj rows [128, slot, j, n]: slots 0..4 = chunk (hi,lo),
        # slot 5 = chunk 4 as (lo,hi) so hi*hi+lo*lo fuses into one DoubleRow
        woh, wol = _split8(Wout[sl, :], SWO)
        wo8 = np.empty((128, 6, 2, DIM), E4NP)
        for s in range(HPC):
            wo8[:, s, 0] = woh[s * 128:(s + 1) * 128]
            wo8[:, s, 1] = wol[s * 128:(s + 1) * 128]
        wo8[:, 5, 0] = wol[4 * 128:5 * 128]
        wo8[:, 5, 1] = woh[4 * 128:5 * 128]
        in_maps.append({
            "hP": hP,
            "wq8": _pack_w(Wqkv[:, 0 * DIM:1 * DIM][:, sl], SW),
            "wk8": _pack_w(Wqkv[:, 1 * DIM:2 * DIM][:, sl], SW),
            "wv8": _pack_w(Wqkv[:, 2 * DIM:3 * DIM][:, sl], SW),
            "bq": np.ascontiguousarray(bq_c),
            "bk": np.ascontiguousarray(bk_c),
            "wqn": np.ascontiguousarray(wq_norm[sl].reshape(HPC, HD).T
                                        .astype(np.float32)),
            "wkn": np.ascontiguousarray(wk_norm[sl].reshape(HPC, HD).T
                                        .astype(np.float32)),
            "cosT": cosTb,
            "sinrT": sinrTb,
            "wo8": np.ascontiguousarray(wo8.reshape(128, -1)),
        })
    return in_maps


def run(S, DIM, H, inputs):
    nc = _get_program(S, DIM, H)
    in_maps = make_in_maps(S, DIM, H, **inputs)
    res = run_bass_kernel_spmd(nc, in_maps, list(range(NCORES)))
    partial = np.zeros((2 * S, DIM), np.float64)
    for r in res.results:
        partial += np.asarray(r["outp"]).astype(np.float64)
    # the v-bias contribution: softmax rows sum to 1, so attn(v + 1*bv) =
    # attn(v) + 1*bv, and bv flows through Wout as a constant per-channel term
    bv_full = np.asarray(inputs["bqkv"])[2 * DIM:3 * DIM].astype(np.float64)
    const_bias = bv_full @ np.asarray(inputs["Wout"]).astype(np.float64) \
        + np.asarray(inputs["bout"])
    out = (partial + const_bias[None, :]).astype(np.float32)
    out = out.reshape(2, 1, S, DIM)
    return out[0], out[1]


def kernel(hidden_cond, hidden_uncond, cos_freqs, sin_freqs,
           Wqkv, bqkv, wq_norm, wk_norm, Wout, bout):
    B, S, DIM = np.asarray(hidden_cond).shape
    assert B == 1
    H = DIM // 128
    return run(S, DIM, H, dict(
        hidden_cond=hidden_cond, hidden_uncond=hidden_uncond,
        cos_freqs=cos_freqs, sin_freqs=sin_freqs, Wqkv=Wqkv, bqkv=bqkv,
        wq_norm=wq_norm, wk_norm=wk_norm, Wout=Wout, bout=bout))


# revision 41
# speedup vs baseline: 1.2069x; 1.0011x over previous
"""CFG dual self-attention kernel for 8 Trainium2 NeuronCores.

Strategy (tensor parallel on heads):
  - h = concat(hidden_cond, hidden_uncond) -> [4096 tokens, 5120]; host
    pre-transposes AND pre-tiles to hP [128, tb, chunk, hi/lo, tok] so every
    DMA line is a single 20 KB contiguous run per partition (full 360 GB/s;
    the naive transposed layout ran at half speed on 256 B descriptor lines).
  - Each core owns 5 heads (640 of the 5120 q/k/v channels).  One fused pass
    over hP computes qT/kT [640, 4096] (head-dim on partitions, spilled to
    DRAM as bf16) with fp8 hi/lo DoubleRow matmuls; a second pass computes
    v [4096, 640], which stays resident in SBUF (bf16) until attention.
  - RMSNorm over the full 5120 dims needs a cross-core sum of squares:
    partial ssq per token is computed with ones-matmuls on the PE and
    allreduced across the 8 cores (32 KB collective, hidden under the V
    projection).  The 1/sqrt factors and the first attention pair's
    rope chain are also emitted under the V projection (DVE is idle there).
  - Attention per (batch, head) in scores-transposed layout
    scoresT[st, sq] = (rope(k) slice)^T @ rope(q), all in bf16: softmax
    denominators via ones-matmul column sums (interleaved PSUM accumulation
    groups), exp on the scalar engine, A@V accumulated with resident
    v-chunks stationary, and the 1/colsum normalization folded into the
    PSUM->SBUF eviction of attn_outT (quantized fp8 hi/lo for phase 4).
  - Output projection: partial_out = attn_outT^T @ Wout[rows of this core]
    -> [4096, 5120] bf16 per-core partial; host sums the 8 partials (+ bout).
"""

import numpy as np
import ml_dtypes

import concourse.bass as bass  # noqa: F401  (bass types via bacc)
import concourse.mybir as mybir
import concourse.tile as tile
from concourse import bacc
from concourse.bass_utils import run_bass_kernel_spmd

F32 = mybir.dt.float32
F32R = mybir.dt.float32r
BF16 = mybir.dt.bfloat16
F8 = mybir.dt.float8e4
E4NP = ml_dtypes.float8_e4m3
BFNP = ml_dtypes.bfloat16
DRM = mybir.MatmulPerfMode.DoubleRow

NCORES = 8
EPS = 1e-6

# fp8 hi/lo quantization scales (host-side split; 3-term DoubleRow matmuls)
SH = 16.0
SW = 1024.0
SA = 32.0
SWO = 1024.0
QKV_DESC = 1.0 / (SH * SW)
OUT_DESC = 1.0 / (SA * SWO)


def build_program(S, DIM, H, collective=True, repeat=1):
    """Emit the per-core bass program (identical on all cores; per-core data
    differences come entirely from the input tensors)."""
    HD = 128
    assert DIM == H * HD
    HPC = H // NCORES          # heads per core
    CW = HPC * HD              # per-core channel width for q/k/v
    CT = HPC                   # 128-col tiles per group
    NT = 2 * S                 # tokens across both batches
    DC = DIM // 128            # contraction chunks
    TBS = 256                  # token block in phase 1
    NTB = NT // TBS
    SQB = min(512, S)          # sq block in attention
    NSQ = S // SQB
    NST = S // 128             # st (key) chunks per batch
    NTC = NT // 128            # token chunks for resident v
    ONB = DIM // 512           # out-proj N blocks
    WOSL = 2048 if DIM >= 4096 else DIM   # prefetched Wout column prefix

    nc = bacc.Bacc("TRN2", target_bir_lowering=False, debug=False,
                   num_devices=NCORES)

    # host-pretiled inputs: per partition p everything is contiguous, so each
    # DMA is a handful of >=512 B descriptors (full DMA bandwidth)
    hP = nc.dram_tensor("hP", [128, NTB * DC * 2 * TBS], F8,
                        kind="ExternalInput")
    wq8 = nc.dram_tensor("wq8", [128, DC * 2 * CW], F8, kind="ExternalInput")
    wk8 = nc.dram_tensor("wk8", [128, DC * 2 * CW], F8, kind="ExternalInput")
    wv8 = nc.dram_tensor("wv8", [128, DC * 2 * CW], F8, kind="ExternalInput")
    bq = nc.dram_tensor("bq", [128, CT], F32, kind="ExternalInput")
    bk = nc.dram_tensor("bk", [128, CT], F32, kind="ExternalInput")
    wqn = nc.dram_tensor("wqn", [128, CT], F32, kind="ExternalInput")
    wkn = nc.dram_tensor("wkn", [128, CT], F32, kind="ExternalInput")
    cosT = nc.dram_tensor("cosT", [128, S], BF16, kind="ExternalInput")
    sinrT = nc.dram_tensor("sinrT", [128, S], BF16, kind="ExternalInput")
    wo8 = nc.dram_tensor("wo8", [128, 6 * 2 * DIM], F8, kind="ExternalInput")
    outp = nc.dram_tensor("outp", [NT, DIM], BF16, kind="ExternalOutput")

    h5 = hP.rearrange("p (b c j t) -> p b c j t", b=NTB, c=DC, j=2, t=TBS)
    wq4 = wq8.rearrange("p (c j n) -> p c j n", c=DC, j=2, n=CW)
    wk4 = wk8.rearrange("p (c j n) -> p c j n", c=DC, j=2, n=CW)
    wv4 = wv8.rearrange("p (c j n) -> p c j n", c=DC, j=2, n=CW)
    w3o = wo8.rearrange("p (s j n) -> p s j n", s=6, j=2, n=DIM)

    with tile.TileContext(nc) as tc:
        with (
            tc.tile_pool(name="dram", bufs=1, space="DRAM") as dram,
            tc.tile_pool(name="persist", bufs=1) as persist,
        ):
            for _rep in range(repeat):
              qsc = dram.tile([CW, NT], BF16, tag="qsc")
              ksc = dram.tile([CW, NT], BF16, tag="ksc")
              aosc8 = dram.tile([2 * CW, NT], F8, tag="aosc8")
              # ssq partials in partition-major layout [128 p, gi, tb, ts]:
              # produced by transposed ones-matmuls (out free size 1 => ~free
              # on the PE), allreduced as a flat 32 KB buffer
              NSS = 2 * NTB * (TBS // 128)
              cc_in = dram.tile([128, NSS], F32, tag="cc_in")
              cc_out = dram.tile([128, NSS], F32, tag="cc_out")
              rdump = dram.tile([2, NTB * (TBS // 128), 128], BF16,
                                tag="rdump")

              # constants
              ones_f = persist.tile([128, 1], F32, tag="ones_f")
              nc.vector.memset(ones_f[:], 1.0)
              ones_b = persist.tile([128, 1], BF16, tag="ones_b")
              nc.vector.tensor_copy(ones_b[:], ones_f[:])

              bq_t = persist.tile([128, CT], F32, tag="bq")
              nc.scalar.dma_start(bq_t[:], bq[:])
              bk_t = persist.tile([128, CT], F32, tag="bk")
              nc.scalar.dma_start(bk_t[:], bk[:])
              wqn_t = persist.tile([128, CT], F32, tag="wqn")
              nc.scalar.dma_start(wqn_t[:], wqn[:])
              wkn_t = persist.tile([128, CT], F32, tag="wkn")
              nc.scalar.dma_start(wkn_t[:], wkn[:])

              from contextlib import ExitStack as _ES
              _late = _ES()
              # resident v lives from the v projection through attention;
              # pre-reserved before the phase-1 pools (LIFO stack order)
              vresp = _late.enter_context(tc.tile_pool(name="vres", bufs=1))
              vall = vresp.tile([128, NTC, CW], BF16, tag="vall")

              # ---------------- phase 1: projections ----------------------
              with tc.tile_pool(name="hp", bufs=2) as hp:
                def load_hall(tb, pieces=1):
                    hall = hp.tile([128, DC, 2, TBS], F8, tag="hall")
                    cs = DC // pieces
                    for i in range(pieces):
                        nc.sync.dma_start(
                            hall[:, i * cs:(i + 1) * cs, :, :],
                            h5[:, tb, i * cs:(i + 1) * cs, :, :])
                    return hall

                # ---- phase 1a: fused q+k projections + ssq partials ------
                with (
                    tc.tile_pool(name="wqk", bufs=1) as wqk,
                    tc.tile_pool(name="ev", bufs=2) as evp,
                    tc.tile_pool(name="ps", bufs=4, space="PSUM") as psp,
                    tc.tile_pool(name="sq", bufs=2, space="PSUM") as sqp,
                ):
                    # interleave wq / hall0 quarter-loads so the first token
                    # block (which consumes chunks in order) starts as soon
                    # as the first quarter lands
                    wq_t = wqk.tile([128, DC, 2, CW], F8, tag="wq")
                    hall0 = hp.tile([128, DC, 2, TBS], F8, tag="hall")
                    for i in range(8):
                        nc.sync.dma_start(wq_t[:, i * 5:(i + 1) * 5],
                                          wq4[:, i * 5:(i + 1) * 5])
                        nc.sync.dma_start(
                            hall0[:, i * 5:(i + 1) * 5, :, :],
                            h5[:, 0, i * 5:(i + 1) * 5, :, :])
                    hall1 = load_hall(1, pieces=2)
                    wk_t = wqk.tile([128, DC, 2, CW], F8, tag="wk")
                    for i in range(4):
                        nc.sync.dma_start(wk_t[:, i * 10:(i + 1) * 10],
                                          wk4[:, i * 10:(i + 1) * 10])

                    deferred = []
                    ssq_acc = evp.tile([128, 2, NTB, TBS // 128], F32,
                                       tag="ssq_acc")

                    def qk_group(tb, gi, hall):
                        wall, bias_t, spill = (
                            (wq_t, bq_t, qsc), (wk_t, bk_t, ksc))[gi]
                        nts = TBS // 128
                        zz = sqp.tile([128, nts], F32, tag="ssq")
                        evq = evp.tile([128, CT, TBS], BF16, tag="evq")
                        sqt = evp.tile([128, CT, TBS], BF16, tag="sqt")
                        for ct in range(CT):
                            pq = psp.tile([128, TBS], F32, tag="acc")
                            csl = slice(ct * 128, (ct + 1) * 128)
                            # chunk-ordered 3-term interleave: compute
                            # streams behind the chunk-sliced DMAs
                            for cp in range(DC // 2):
                                nc.tensor.matmul(
                                    pq[:],
                                    wall[:, 2 * cp:2 * cp + 2, 0, csl],
                                    hall[:, 2 * cp:2 * cp + 2, 1, :],
                                    start=(cp == 0), stop=False,
                                    perf_mode=DRM)
                                nc.tensor.matmul(
                                    pq[:], wall[:, 2 * cp, :, csl],
                                    hall[:, 2 * cp, :, :],
                                    start=False, stop=False, perf_mode=DRM)
                                nc.tensor.matmul(
                                    pq[:], wall[:, 2 * cp + 1, :, csl],
                                    hall[:, 2 * cp + 1, :, :],
                                    start=False, stop=(cp == DC // 2 - 1),
                                    perf_mode=DRM)
                            # the ssq matmul of the PREVIOUS ct group goes
                            # here so the PE never waits on the DVE square
                            if deferred:
                                deferred.pop(0)()
                            nc.vector.tensor_scalar(
                                evq[:, ct, :], pq[:],
                                bias_t[:, ct:ct + 1], QKV_DESC,
                                mybir.AluOpType.add,
                                mybir.AluOpType.mult)
                            nc.vector.tensor_mul(
                                sqt[:, ct, :], evq[:, ct, :], evq[:, ct, :])

                            def emit_ssq(zz=zz, sqt=sqt, ct=ct,
                                         evq=evq, tb=tb, spill=spill, gi=gi,
                                         nts=nts):
                                # transposed ones-matmul: out free size is 1,
                                # so the partition-dim token sums are nearly
                                # free on the PE (vs 1 cycle/token in the
                                # row-layout version)
                                # one zero-region per PSUM bank: only the
                                # FIRST chain in the shared bank may set
                                # start (it lazily zeroes the whole bank),
                                # only the LAST may set stop
                                for ts in range(nts):
                                    nc.tensor.matmul(
                                        zz[:, ts:ts + 1],
                                        sqt[:, ct, ts * 128:(ts + 1) * 128],
                                        ones_b[:],
                                        start=(ct == 0 and ts == 0),
                                        stop=(ct == CT - 1
                                              and ts == nts - 1))
                                if ct == CT - 1:
                                    # batched bf16 spill of the whole token
                                    # block (one DMA per (tb, gi))
                                    nc.sync.dma_start(
                                        spill.rearrange(
                                            "(c p) t -> p c t", p=128)
                                        [:, :, tb * TBS:(tb + 1) * TBS],
                                        evq[:])
                                    nc.vector.tensor_copy(
                                        ssq_acc[:, gi, tb, :], zz[:])
                            deferred.append(emit_ssq)

                    # startup order q0,q1,k0,k1 hides the wk load behind the
                    # first two q groups (the serial DMA stream needs ~44us
                    # for wq+wk+h0+h1, two q groups give it ~32us of PE work)
                    qk_group(0, 0, hall0)
                    qk_group(1, 0, hall1)
                    qk_group(0, 1, hall0)
                    qk_group(1, 1, hall1)
                    halls = {0: hall0, 1: hall1}
                    for tb in range(2, NTB):
                        hall = load_hall(tb)
                        halls[tb] = hall
                        qk_group(tb, 0, hall)
                        qk_group(tb, 1, hall)
                    while deferred:
                        deferred.pop(0)()
                    nc.gpsimd.dma_start(
                        cc_in.rearrange("p (g c s) -> p g c s", g=2, c=NTB),
                        ssq_acc[:])

                # allreduce the ssq partials (overlaps with the v group)
                if collective:
                    nc.gpsimd.collective_compute(
                        "AllReduce", mybir.AluOpType.add,
                        replica_groups=[list(range(NCORES))],
                        ins=[cc_in[:].opt()], outs=[cc_out[:].opt()])
                else:
                    # single-core timing-sim variant: stand-in for the
                    # allreduce so TimelineSim (no collectives) can run
                    nc.sync.dma_start(cc_out[:], cc_in[:])

                # rms norm factors: tiny partition-major math + the b=0
                # broadcast rows, all overlapped with the v projection
                # (fold the HD**-0.5 attention scale into the q side:
                #  s/sqrt(ssq/DIM+eps) == 1/sqrt(ssq*HD/DIM + HD*eps))
                NTC2 = NTB * (TBS // 128)
                rwork = persist.tile([128, 2, NTC2], F32, tag="rwork")
                rinv = persist.tile([128, 2, NTC2], BF16, tag="rinv")
                nc.scalar.dma_start(
                    rwork[:], cc_out.rearrange("p (g c) -> p g c", g=2))
                for gi in range(2):
                    sc1 = (HD / DIM) if gi == 0 else (1.0 / DIM)
                    sc2 = (HD * EPS) if gi == 0 else EPS
                    nc.vector.tensor_scalar(
                        rwork[:, gi, :], rwork[:, gi, :], sc1, sc2,
                        mybir.AluOpType.mult, mybir.AluOpType.add)
                nc.scalar.activation(
                    rwork[:], rwork[:], mybir.ActivationFunctionType.Sqrt)
                with nc.allow_low_precision(reason="bf16 rms factor"):
                    nc.vector.reciprocal(rinv[:], rwork[:])
                nc.scalar.dma_start(rdump.rearrange("g c p -> p g c"),
                                    rinv[:])
                rd2 = rdump.rearrange("g c p -> g (c p)")
                rb = {}

                def emit_rb(b, pool):
                    for gi in range(2):
                        row = pool.tile([1, S], BF16, tag="rrow",
                                        name="rrow")
                        dma = nc.scalar.dma_start if b == 0 \
                            else nc.sync.dma_start
                        dma(row[:], rd2[gi:gi + 1, b * S:(b + 1) * S])
                        t = pool.tile([128, S], BF16, tag=f"rb{gi}{b}",
                                      name=f"rb{gi}{b}")
                        nc.gpsimd.partition_broadcast(t[:], row[:])
                        rb[(gi, b)] = t

                emit_rb(0, persist)

                # ---- phase 1b: v projection (natural layout) -------------
                # wv loads in chunk slices at v start; the PE streams
                # chunk-ordered behind them (four PSUM tiles in lockstep)
                with (
                    tc.tile_pool(name="wv", bufs=1) as wvp,
                    tc.tile_pool(name="psv", bufs=2, space="PSUM") as psv,
                ):
                    wv_t = wvp.tile([128, DC, 2, CW], F8, tag="wv")
                    for i in range(8):
                        nc.sync.dma_start(wv_t[:, i * 5:(i + 1) * 5],
                                          wv4[:, i * 5:(i + 1) * 5])
                    nsub = TBS // 128
                    nb = CW // 2
                    # reverse order: the last two token blocks' hidden tiles
                    # are still resident from the q/k pass, so the v matmuls
                    # start immediately while the wv weights stream in
                    for tb in list(range(NTB))[::-1]:
                        hall = halls[tb] if tb >= NTB - 2 else load_hall(tb)
                        pv = [[psv.tile([128, nb], F32, tag=f"pv{ts}{i}",
                                        name=f"pv{ts}{i}")
                               for i in range(2)] for ts in range(nsub)]
                        for cp in range(DC // 2):
                            for ts in range(nsub):
                                tsl = slice(ts * 128, (ts + 1) * 128)
                                for i in range(2):
                                    nsl = slice(i * nb, (i + 1) * nb)
                                    nc.tensor.matmul(
                                        pv[ts][i][:],
                                        hall[:, 2 * cp:2 * cp + 2, 1, tsl],
                                        wv_t[:, 2 * cp:2 * cp + 2, 0, nsl],
                                        start=(cp == 0), stop=False,
                                        perf_mode=DRM)
                                    nc.tensor.matmul(
                                        pv[ts][i][:],
                                        hall[:, 2 * cp, :, tsl],
                                        wv_t[:, 2 * cp, :, nsl],
                                        start=False, stop=False,
                                        perf_mode=DRM)
                                    nc.tensor.matmul(
                                        pv[ts][i][:],
                                        hall[:, 2 * cp + 1, :, tsl],
                                        wv_t[:, 2 * cp + 1, :, nsl],
                                        start=False,
                                        stop=(cp == DC // 2 - 1),
                                        perf_mode=DRM)
                        for ts in range(nsub):
                            for i in range(2):
                                # v bias is folded into the host-side output
                                # bias (softmax rows sum to 1); pure
                                # PSUM->SBUF convert-copy on the idle scalar
                                # engine straight into the resident v tile
                                nc.scalar.activation(
                                    vall[:, tb * nsub + ts,
                                         i * nb:(i + 1) * nb],
                                    pv[ts][i][:],
                                    mybir.ActivationFunctionType.Copy,
                                    scale=QKV_DESC)

                # pre-warm the Exp activation table while the scalar
                # engine is idle at the v tail (ones_f is dead by now), so
                # the 1.3us table load is off the first exp's critical path
                nc.scalar.activation(ones_f[:], ones_f[:],
                                     mybir.ActivationFunctionType.Exp)

              # ---------------- phase 3: attention per (batch, head) ------
              _late2 = _ES()
              wprep = _late2.enter_context(tc.tile_pool(name="wpre", bufs=1))
              # the full Wout block + the attn-out ping-pong tiles live in a
              # pool that predates the attention pools, so their DMAs have no
              # WAR on attention tiles and stream during attention
              wot = wprep.tile([128, 6, 2, DIM], F8, tag="wot")
              bhs = [(b, hh) for b in range(2) for hh in range(HPC)]
              with (
                  tc.tile_pool(name="attc", bufs=1) as attc,
                  tc.tile_pool(name="p3", bufs=2) as p3,
                  tc.tile_pool(name="p3e", bufs=4) as p3e,
                  tc.tile_pool(name="ps_sc", bufs=2, space="PSUM") as ps_sc,
                  tc.tile_pool(name="ps_cs", bufs=2, space="PSUM") as ps_cs,
                  tc.tile_pool(name="ps_av", bufs=2, space="PSUM") as ps_av,
              ):
                  cosT_t = attc.tile([128, S], BF16, tag="cosT")
                  nc.gpsimd.dma_start(cosT_t[:], cosT[:])
                  sinrT_t = attc.tile([128, S], BF16, tag="sinrT")
                  nc.gpsimd.dma_start(sinrT_t[:], sinrT[:])

                  def prep_gi(i, gi):
                      """One of pair i's two rope chains (see prep); emitted
                      separately so each in-order DVE insertion stays short
                      enough to not starve the eviction cadence."""
                      b, hh = bhs[i]
                      spill, wn = ((qsc, wqn_t), (ksc, wkn_t))[gi]
                      xt = p3.tile([128, S], BF16, tag="xt", name="xt")
                      tmc = p3.tile([128, S], BF16, tag="tmc", name="tmc")
                      tms = p3.tile([128, S], BF16, tag="tms", name="tms")
                      xr = p3.tile([128, S], BF16, tag=f"xr{gi}", name="xr")
                      nc.sync.dma_start(
                          xt[:], spill[hh * 128:(hh + 1) * 128,
                                       b * S:(b + 1) * S])
                      nc.vector.scalar_tensor_tensor(
                          xt[:], xt[:], wn[:, hh:hh + 1], rb[(gi, b)][:],
                          mybir.AluOpType.mult, mybir.AluOpType.mult)
                      nc.vector.tensor_mul(tmc[:], xt[:], cosT_t[:])
                      nc.vector.tensor_mul(
                          tms[0:64, :], xt[64:128, :], sinrT_t[64:128, :])
                      nc.vector.tensor_mul(
                          tms[64:128, :], xt[0:64, :], sinrT_t[0:64, :])
                      nc.vector.tensor_add(xr[:], tmc[:], tms[:])
                      return xr

                  def prep(i, nchunk=1):
                      """Load + norm + rope q/k for pair i (bf16 end-to-end;
                      v is already resident).  Emitted ahead so the DVE work
                      overlaps earlier attention.  nchunk>1 interleaves
                      column slices k-first so the first scores matmul only
                      waits for the first k+q slices (used for pair 0)."""
                      b, hh = bhs[i]
                      CS2 = S // nchunk
                      tls = {}
                      for gi, (spill, wn) in enumerate(
                              [(qsc, wqn_t), (ksc, wkn_t)]):
                          xt = p3.tile([128, S], BF16, tag="xt",
                                       name="xt")
                          tmc = p3.tile([128, S], BF16, tag="tmc",
                                        name="tmc")
                          tms = p3.tile([128, S], BF16, tag="tms",
                                        name="tms")
                          xr = p3.tile([128, S], BF16, tag=f"xr{gi}",
                                       name="xr")
                          tls[gi] = (xt, tmc, tms, xr, spill, wn)
                      for cc in range(nchunk):
                          sl = slice(cc * CS2, (cc + 1) * CS2)
                          for gi in (1, 0):
                              xt, tmc, tms, xr, spill, wn = tls[gi]
                              dma = nc.sync.dma_start
                              dma(xt[:, sl],
                                  spill[hh * 128:(hh + 1) * 128,
                                        b * S + cc * CS2:
                                        b * S + (cc + 1) * CS2])
                              # fused (xt * wn) * rb in one DVE op
                              nc.vector.scalar_tensor_tensor(
                                  xt[:, sl], xt[:, sl], wn[:, hh:hh + 1],
                                  rb[(gi, b)][:, sl],
                                  mybir.AluOpType.mult,
                                  mybir.AluOpType.mult)
                              nc.vector.tensor_mul(tmc[:, sl], xt[:, sl],
                                                   cosT_t[:, sl])
                              nc.vector.tensor_mul(
                                  tms[0:64, sl], xt[64:128, sl],
                                  sinrT_t[64:128, sl])
                              nc.vector.tensor_mul(
                                  tms[64:128, sl], xt[0:64, sl],
                                  sinrT_t[0:64, sl])
                              nc.vector.tensor_add(xr[:, sl], tmc[:, sl],
                                                   tms[:, sl])
                      return tls[0][3], tls[1][3]

                  preps = {0: prep(0, nchunk=4)}
                  NSTP = NST // 2
                  NSL = SQB // 128
                  NPU = NSQ * NSTP
                  NTOT = len(bhs) * NPU
                  ets, avs, zzs, cur, aos = {}, {}, {}, {}, {}

                  def norm_evict(i, sqb):
                      # denominators are partition-major [128, NSL]; reshape
                      # to a row via a DRAM bounce (SBUF APs cannot transpose
                      # partitions), then 1/z and the fp8 hi/lo quantization
                      b, hh = bhs[i]
                      aoh8, aol8 = aos[i]
                      av = avs.pop((i, sqb))
                      zz = zzs.pop((i, sqb))
                      z4s = p3.tile([128, NSL], F32, tag="z4s")
                      nc.vector.tensor_copy(z4s[:], zz[:])
                      # copy av out of PSUM immediately so its bank is free
                      # for the sqb two steps ahead even while the z bounce
                      # and later DVE work (rope chains) are still pending
                      av_s = p3.tile([128, SQB], F32, tag="av_s",
                                     name="av_s")
                      nc.vector.tensor_copy(av_s[:], av[:])
                      zd = dram.tile([NSL, 128], F32, tag="zd")
                      nc.sync.dma_start(
                          zd.rearrange("s p -> p s"), z4s[:])
                      rc = p3.tile([1, SQB], F32, tag="rc")
                      nc.sync.dma_start(
                          rc[:], zd.rearrange("s p -> () (s p)"))
                      nc.vector.reciprocal(rc[:], rc[:])
                      nc.vector.tensor_scalar_mul(rc[:], rc[:], SA)
                      rb2 = p3.tile([128, SQB], F32, tag="rb2")
                      nc.gpsimd.partition_broadcast(rb2[:], rc[:])
                      sqsl = slice(sqb * SQB, (sqb + 1) * SQB)
                      ao32 = p3.tile([128, SQB], F32, tag="ao32")
                      nc.vector.tensor_mul(ao32[:], av_s[:], rb2[:])
                      nc.vector.tensor_copy(aoh8[:, sqsl], ao32[:])
                      nc.vector.tensor_tensor(
                          aol8[:, sqsl], ao32[:], aoh8[:, sqsl],
                          mybir.AluOpType.subtract)

                  # single software-pipelined stream over every
                  # (pair, sqb, st-pair) unit, one unit of lookahead: the
                  # scalar engine (exp) is the bottleneck, so the scores
                  # matmuls feeding exp x+1 always precede the av/cs
                  # consumers of exp x -- across sqb AND pair boundaries
                  for x in range(NTOT + 1):
                      if x < NTOT:
                          i, r = divmod(x, NPU)
                          sqb, stp = divmod(r, NSTP)
                          b, hh = bhs[i]
                          if r == 0:
                              cur[i] = preps.pop(i)
                              aos[i] = (
                                  p3.tile([128, S], F8, tag="aoh8",
                                          name="aoh8"),
                                  p3.tile([128, S], F8, tag="aol8",
                                          name="aol8"))
                          qr, kr = cur[i]
                          if stp == 0:
                              zzs[(i, sqb)] = ps_cs.tile(
                                  [128, NSL], F32, tag="zz", name="zz")
                              avs[(i, sqb)] = ps_av.tile(
                                  [128, SQB], F32, tag="av", name="av")
                          sc2 = ps_sc.tile([128, 2, SQB], F32, tag="sc")
                          for hs in range(2):
                              st = 2 * stp + hs
                              nc.tensor.matmul(
                                  sc2[:, hs, :],
                                  kr[:, st * 128:(st + 1) * 128],
                                  qr[:, sqb * SQB:(sqb + 1) * SQB],
                                  start=True, stop=True)
                          et2 = p3e.tile([128, 2, SQB], BF16, tag="et")
                          nc.scalar.activation(
                              et2[:], sc2[:],
                              mybir.ActivationFunctionType.Exp)
                          ets[x] = et2
                      if x >= 1:
                          i, r = divmod(x - 1, NPU)
                          sqb, stp = divmod(r, NSTP)
                          b, hh = bhs[i]
                          et2 = ets.pop(x - 1)
                          for hs in range(2):
                              st = 2 * stp + hs
                              nc.tensor.matmul(
                                  avs[(i, sqb)][:],
                                  vall[:, b * NST + st,
                                       hh * 128:(hh + 1) * 128],
                                  et2[:, hs, :],
                                  start=(st == 0),
                                  stop=(st == NST - 1))
                              # shared-bank chains: single start (bank
                              # zero) / single stop, see phase 1
                              for sl in range(NSL):
                                  nc.tensor.matmul(
                                      zzs[(i, sqb)][:, sl:sl + 1],
                                      et2[:, hs,
                                          sl * 128:(sl + 1) * 128],
                                      ones_b[:],
                                      start=(st == 0 and sl == 0),
                                      stop=(st == NST - 1
                                            and sl == NSL - 1))
                          if stp == NSTP - 1:
                              norm_evict(i, sqb)
                              if i + 1 < len(bhs):
                                  # next pair's rope chains, k then q split
                                  # across the first two hooks (pair 1 all
                                  # at once -- the stream reaches it early):
                                  # each DVE insertion is ~5us so neither
                                  # the eviction cadence nor the next
                                  # pair's scores starve
                                  if i == 0:
                                      if sqb == 0:
                                          preps[1] = prep(1)
                                  elif sqb == 0:
                                      preps[i + 1] = [None,
                                                      prep_gi(i + 1, 1)]
                                  elif sqb == 1:
                                      preps[i + 1][0] = prep_gi(i + 1, 0)
                              if i == 0:
                                  # off the critical path: the b=1 norm
                                  # factors and the Wout stream for phase 4
                                  # (spread so its transfers do not starve
                                  # the prep loads on the shared DMA bus)
                                  if sqb == 0:
                                      emit_rb(1, attc)
                                  nwq = [4, 2, 2, 2]
                                  base = sum(nwq[:sqb])
                                  for nbw in range(base,
                                                   base + nwq[sqb]):
                                      nc.sync.dma_start(
                                          wot[:, :, :,
                                              nbw * 512:(nbw + 1) * 512],
                                          w3o[:, :, :,
                                              nbw * 512:(nbw + 1) * 512])
                              if sqb == NSQ - 1:
                                  # SP queue: the pool queue carries the
                                  # z bounces and would delay these, and
                                  # their completion releases the aoh8
                                  # buffers two pairs later
                                  aoh8, aol8 = aos.pop(i)
                                  nc.sync.dma_start(
                                      aosc8[hh * 256 + 128:hh * 256 + 256,
                                            b * S:(b + 1) * S], aoh8[:])
                                  nc.sync.dma_start(
                                      aosc8[hh * 256:hh * 256 + 128,
                                            b * S:(b + 1) * S], aol8[:])

              # -------------- phase 4: partial output projection ----------
              with (
                  tc.tile_pool(name="p4", bufs=4) as p4,
                  tc.tile_pool(name="oe", bufs=2) as oep,
                  tc.tile_pool(name="ps4", bufs=4, space="PSUM") as ps4,
              ):
                  ao4 = aosc8.rearrange("(c j p) t -> p c j t", p=128, j=2)

                  def load_aot(tt):
                      aot = wprep.tile([128, HPC, 2, 128], F8,
                                       tag=f"aot{tt % 2}", name="aot")
                      nc.sync.dma_start(
                          aot[:], ao4[:, :, :, tt * 128:(tt + 1) * 128])
                      return aot

                  aot_next = load_aot(0)
                  for tt in range(NT // 128):
                      aot = aot_next
                      if tt + 1 < NT // 128:
                          aot_next = load_aot(tt + 1)
                      orow = oep.tile([128, DIM], BF16, tag="orow")
                      for nb in range(ONB):
                          wsl = wot[:, :, :, nb * 512:(nb + 1) * 512]
                          po = ps4.tile([128, 512], F32, tag="po")
                          # 8 DoubleRow passes: 2 hi*hi chunk-pairs, 5
                          # crosses, 1 fused hi*hi+lo*lo for the odd chunk
                          nc.tensor.matmul(
                              po[:], aot[:, 0:2, 1, :], wsl[:, 0:2, 0, :],
                              start=True, stop=False, perf_mode=DRM)
                          nc.tensor.matmul(
                              po[:], aot[:, 2:4, 1, :], wsl[:, 2:4, 0, :],
                              start=False, stop=False, perf_mode=DRM)
                          for ch in range(HPC):
                              nc.tensor.matmul(
                                  po[:], aot[:, ch, :, :], wsl[:, ch, :, :],
                                  start=False, stop=False, perf_mode=DRM)
                          nc.tensor.matmul(
                              po[:], aot[:, 4, :, :], wsl[:, 5, :, :],
                              start=False, stop=True, perf_mode=DRM)
                          # scalar engine is idle in phase 4; evicting
                          # there keeps the DVE queue out of the po-buffer
                          # reuse chain
                          nc.scalar.activation(
                              orow[:, nb * 512:(nb + 1) * 512], po[:],
                              mybir.ActivationFunctionType.Copy,
                              scale=OUT_DESC)
                      if tt == NT // 128 - 1:
                          # split the last row-block's writeback so the DMA
                          # overlaps the tail evictions
                          for qq in range(4):
                              qsl = slice(qq * (DIM // 4),
                                          (qq + 1) * (DIM // 4))
                              nc.sync.dma_start(
                                  outp[tt * 128:(tt + 1) * 128, qsl],
                                  orow[:, qsl])
                      else:
                          nc.sync.dma_start(
                              outp[tt * 128:(tt + 1) * 128, :], orow[:])
              _late2.close()
              _late.close()
    nc.finalize()
    return nc


_PROGRAM_CACHE = {}


def _get_program(S, DIM, H):
    key = (S, DIM, H)
    if key not in _PROGRAM_CACHE:
        _PROGRAM_CACHE[key] = build_program(S, DIM, H)
    return _PROGRAM_CACHE[key]


def _split8(x, scale):
    xs = (np.asarray(x, np.float32) * np.float32(scale))
    hi = xs.astype(E4NP)
    lo = (xs - hi.astype(np.float32)).astype(E4NP)
    return hi, lo


def _pack_h(h, DIM, NT, TBS):
    # h [NT, DIM] f32 -> [128, NTB*DC*2*TBS] fp8, j: 0=lo, 1=hi
    DC = DIM // 128
    NTB = NT // TBS
    hh, hl = _split8(h.T, SH)                        # [DIM, NT]
    arr = np.stack([hl, hh])                         # [2(j), DIM, NT]
    arr = arr.reshape(2, DC, 128, NTB, TBS)
    arr = arr.transpose(2, 3, 1, 0, 4)               # [128, NTB, DC, 2, TBS]
    return np.ascontiguousarray(arr.reshape(128, -1))


def _pack_w(Wslice, scale):
    # [DIM, n] -> [128, DC*2*n] with j: 0=hi, 1=lo
    d, n = Wslice.shape
    wh, wl = _split8(Wslice, scale)
    arr = np.stack([wh, wl])                         # [2(j), DIM, n]
    arr = arr.reshape(2, d // 128, 128, n)
    arr = arr.transpose(2, 1, 0, 3)                  # [128, DC, 2, n]
    return np.ascontiguousarray(arr.reshape(128, -1))


def make_in_maps(S, DIM, H, hidden_cond, hidden_uncond, cos_freqs, sin_freqs,
                 Wqkv, bqkv, wq_norm, wk_norm, Wout, bout):
    HD = 128
    HPC = H // NCORES
    CW = HPC * HD
    NT = 2 * S
    TBS = 256
    h = np.concatenate([np.asarray(hidden_cond), np.asarray(hidden_uncond)],
                       axis=0).reshape(NT, DIM)
    hP = _pack_h(h, DIM, NT, TBS)
    cosTb = np.ascontiguousarray(
        np.asarray(cos_freqs).T.astype(BFNP))        # [128, S]
    sinT = np.asarray(sin_freqs).T                   # [128, S]
    HF = HD // 2
    sinrT = np.concatenate([sinT[HF:], -sinT[:HF]], axis=0)
    sinrTb = np.ascontiguousarray(sinrT.astype(BFNP))
    Wqkv = np.asarray(Wqkv)
    bqkv = np.asarray(bqkv)
    wq_norm = np.asarray(wq_norm)
    wk_norm = np.asarray(wk_norm)
    Wout = np.asarray(Wout)

    in_maps = []
    for c in range(NCORES):
        sl = slice(c * CW, (c + 1) * CW)
        bq_c = (bqkv[0 * DIM:1 * DIM][sl] * (SH * SW)).astype(np.float32) \
            .reshape(HPC, HD).T
        bk_c = (bqkv[1 * DIM:2 * DIM][sl] * (SH * SW)).astype(np.float32) \
            .reshape(HPC, HD).T
        # out-proj rows [128, slot, j, n]: slots 0..4 = chunk (hi,lo),
        # slot 5 = chunk 4 as (lo,hi) so hi*hi+lo*lo fuses into one DoubleRow
        woh, wol = _split8(Wout[sl, :], SWO)
        wo8 = np.empty((128, 6, 2, DIM), E4NP)
        for s in range(HPC):
            wo8[:, s, 0] = woh[s * 128:(s + 1) * 128]
            wo8[:, s, 1] = wol[s * 128:(s + 1) * 128]
        wo8[:, 5, 0] = wol[4 * 128:5 * 128]
        wo8[:, 5, 1] = woh[4 * 128:5 * 128]
        in_maps.append({
            "hP": hP,
            "wq8": _pack_w(Wqkv[:, 0 * DIM:1 * DIM][:, sl], SW),
            "wk8": _pack_w(Wqkv[:, 1 * DIM:2 * DIM][:, sl], SW),
            "wv8": _pack_w(Wqkv[:, 2 * DIM:3 * DIM][:, sl], SW),
            "bq": np.ascontiguousarray(bq_c),
            "bk": np.ascontiguousarray(bk_c),
            "wqn": np.ascontiguousarray(wq_norm[sl].reshape(HPC, HD).T
                                        .astype(np.float32)),
            "wkn": np.ascontiguousarray(wk_norm[sl].reshape(HPC, HD).T
                                        .astype(np.float32)),
            "cosT": cosTb,
            "sinrT": sinrTb,
            "wo8": np.ascontiguousarray(wo8.reshape(128, -1)),
        })
    return in_maps


def run(S, DIM, H, inputs):
    nc = _get_program(S, DIM, H)
    in_maps = make_in_maps(S, DIM, H, **inputs)
    res = run_bass_kernel_spmd(nc, in_maps, list(range(NCORES)))
    partial = np.zeros((2 * S, DIM), np.float64)
    for r in res.results:
        partial += np.asarray(r["outp"]).astype(np.float64)
    # the v-bias contribution: softmax rows sum to 1, so attn(v + 1*bv) =
    # attn(v) + 1*bv, and bv flows through Wout as a constant per-channel term
    bv_full = np.asarray(inputs["bqkv"])[2 * DIM:3 * DIM].astype(np.float64)
    const_bias = bv_full @ np.asarray(inputs["Wout"]).astype(np.float64) \
        + np.asarray(inputs["bout"])
    out = (partial + const_bias[None, :]).astype(np.float32)
    out = out.reshape(2, 1, S, DIM)
    return out[0], out[1]


def kernel(hidden_cond, hidden_uncond, cos_freqs, sin_freqs,
           Wqkv, bqkv, wq_norm, wk_norm, Wout, bout):
    B, S, DIM = np.asarray(hidden_cond).shape
    assert B == 1
    H = DIM // 128
    return run(S, DIM, H, dict(
        hidden_cond=hidden_cond, hidden_uncond=hidden_uncond,
        cos_freqs=cos_freqs, sin_freqs=sin_freqs, Wqkv=Wqkv, bqkv=bqkv,
        wq_norm=wq_norm, wk_norm=wk_norm, Wout=Wout, bout=bout))
